# revision 24
# baseline (speedup 1.0000x reference)
"""CrossAttention kernel for 8 TRN2 NeuronCores.

Reference computation (B=2, Lq=4096, Lkv=1024, query_dim=512, cross_dim=768,
heads=8, dim_head=64, inner=512):
    q = hs @ Wq; k = enc @ Wk; v = enc @ Wv          (per batch)
    attn = softmax(q_h @ k_h^T * scale) per head
    out = concat_h(attn @ v_h) @ Wo + bo

Sharding: 8 cores = 2 batches x 4 query-slices of 1024 queries.  Each core
computes its full slice of the output (all heads), so outputs are disjoint
and no collective is needed.

Per-core dataflow (all matmuls f16 operands, fp32 PSUM accumulate):
  - host passes hs-slice and encoder transposed (hsT [512,1024], encT
    [768,1024]) already cast to f16, weights in f16
  - qT = Wq^T-weighted hsT   -> [inner=512, q=1024]  (heads along partitions)
  - kT likewise              -> [inner=512, kv=1024]
  - v natural                -> [kv=1024, slots]  slot h = 128 cols holding
        v_h (64) + a ones column + zero padding, arranged so the AV matmul
        output lands partition-aligned with head h's rows of outT and the
        softmax denominator (sum_kv exp) falls out of the same matmul.
  - scoresT_h = k_h qT_h     -> [kv, q] (kv on partitions; head pairs use
        base-partition row tiling of the 128x128 PE array)
  - expT = exp(scale * scoresT) on ScalarE, f16 out (no max-subtraction:
        |scaled scores| < ~3)
  - outT_unnorm_h = v_slot^T @ expT accumulated over kv chunks (PSUM),
        one row of which is the softmax denominator
  - normalize: reciprocal (DVE) + PE ones-column broadcast matmul + multiply
  - final = outT^T @ Wo + bo -> [1024, 512], DMA out per 128-row tile

Program order is pipelined for the Tile scheduler: k/v/q projections are
emitted ahead of the attention blocks that consume them, exp(t) is emitted
before AV(t-1) so the PE never waits in-order on ScalarE, and the final
projection m-tiles are interleaved between the last attention blocks.
"""

import sys

if "/opt/trn_rl_repo" not in sys.path:
    sys.path.insert(0, "/opt/trn_rl_repo")

import numpy as np

B, LQ, LKV = 2, 4096, 1024
QD, CD = 512, 768
H, DH = 8, 64
INNER = H * DH  # 512
SCALE = DH ** -0.5
NCORES = 8
WSCALE = 8.0  # host-side pre-scale on Wq and Wk (fp8 range centering)
QSH = LQ // 4  # 1024 queries per core
P = 128

_CACHE: dict = {}
LAST_RESULTS = None  # test harness introspection (exec_time_ns etc.)

# schedule-tuning knobs (sweepable from bench tooling)
CFG = {
    "W1": 12,       # warmup matmuls bridging the input-DMA head
    "B0_PRE": 8,    # block (0,0) pre-loop extra pops
    "B0_PER": 9,    # block (0,0) per-iter extra pops
    "BK_PRE": 2,    # later n=0 blocks pre-loop pops
    "BK_PER": 2,    # later n=0 blocks per-iter pops
    "B21_PER": 1,   # block (2,1) per-iter pops
    "B31_PER": 1,   # block (3,1) per-iter pops
}


def _build_nc():
    from contextlib import ExitStack

    import concourse.bass as bass
    import concourse.tile as tile
    from concourse import bacc, mybir

    f32 = mybir.dt.float32
    f16 = mybir.dt.float16
    f8 = mybir.dt.float8e4
    DR = mybir.MatmulPerfMode.DoubleRow
    Exp = mybir.ActivationFunctionType.Exp

    nc = bacc.Bacc(trn_type="TRN2")

    hsT_d = nc.declare_dram_parameter("hsT", [QD, QSH], f16, isOutput=False)
    encT_d = nc.declare_dram_parameter("encT", [CD, LKV], f16, isOutput=False)
    wq_d = nc.declare_dram_parameter("wq", [QD, INNER], f16, isOutput=False)
    wk_d = nc.declare_dram_parameter("wk", [CD, INNER], f16, isOutput=False)
    wv_d = nc.declare_dram_parameter("wv", [CD, INNER], f16, isOutput=False)
    wo_d = nc.declare_dram_parameter("wo", [INNER, QD], f16, isOutput=False)
    bo_d = nc.declare_dram_parameter("bo", [1, QD], f32, isOutput=False)
    out_d = nc.declare_dram_parameter("out", [QSH, QD], f32, isOutput=True)

    KC_Q = QD // P   # 4 contraction chunks for q projection
    KC_KV = CD // P  # 6 for k/v projections
    AT = INNER // P  # 4 inner tiles (2 heads each)
    NT = LKV // P    # 8 kv chunks
    QN = QSH // 512  # 2 q slices of 512

    with ExitStack() as ctx:
        tc = ctx.enter_context(tile.TileContext(nc))
        const = ctx.enter_context(tc.tile_pool(name="const", bufs=1))
        acts = ctx.enter_context(tc.tile_pool(name="acts", bufs=1))
        expp = ctx.enter_context(tc.tile_pool(name="expp", bufs=4))
        outp = ctx.enter_context(tc.tile_pool(name="outp", bufs=4))
        small = ctx.enter_context(tc.tile_pool(name="small", bufs=6))
        psA = ctx.enter_context(tc.tile_pool(name="psA", bufs=4, space="PSUM"))
        psS = ctx.enter_context(tc.tile_pool(name="psS", bufs=2, space="PSUM"))
        drp = ctx.enter_context(tc.tile_pool(name="drp", bufs=4, space="DRAM"))

        # ---- input DMA, ordered by first use: the q projections (hsT+wq)
        # run during the PE warmup window, then kT (encT+wk), then v (wv);
        # the second encT half only gates scores t>=4 of the first block
        hsT_sb = acts.tile([P, KC_Q, QSH], f16)
        nc.sync.dma_start(hsT_sb[:], hsT_d.rearrange("(c p) n -> p c n", p=P))
        wq_sb = const.tile([P, KC_Q, INNER], f16)
        nc.sync.dma_start(wq_sb[:], wq_d.rearrange("(c p) n -> p c n", p=P))
        encT_sb = acts.tile([P, KC_KV, LKV], f16)
        encT_r = encT_d.rearrange("(c p) n -> p c n", p=P)
        nc.sync.dma_start(encT_sb[:, :, 0:512], encT_r[:, :, 0:512])
        wk_sb = const.tile([P, KC_KV, INNER], f16)
        nc.sync.dma_start(wk_sb[:], wk_d.rearrange("(c p) n -> p c n", p=P))
        nc.sync.dma_start(encT_sb[:, :, 512:1024], encT_r[:, :, 512:1024])
        wv_sb = const.tile([P, KC_KV, INNER], f16)
        nc.sync.dma_start(wv_sb[:], wv_d.rearrange("(c p) n -> p c n", p=P))
        wo_sb = const.tile([P, AT, QD], f16)
        nc.sync.dma_start(wo_sb[:], wo_d.rearrange("(c p) n -> p c n", p=P))
        bo_sb = const.tile([P, QD], f32)
        nc.sync.dma_start(bo_sb[:], bo_d.ap().to_broadcast((P, QD)))

        qT8 = acts.tile([P, AT, 2, QSH], f8)       # planes: (hi, lo)
        kT8 = acts.tile([P, AT, LKV], f8)          # single plane, broadcast in DR
        v_sb = acts.tile([P, NT, H * P], f16)
        outT_sb = acts.tile([P, AT, QSH], f16)
        vv4 = v_sb.rearrange("p t (s c) -> p t s c", c=P)

        # ---- PE warmup: dummy matmuls on zeroed scratch fill the DMA head
        # so the first real matmuls run at full clock (psD is never read)
        scratch = acts.tile([P, 512], f16)
        nc.gpsimd.memset(scratch[:], 0.0)

        # ones column for the PE-side partition broadcast in normalize
        ones_sb = const.tile([1, P], f16)
        nc.vector.memset(ones_sb[:], 1.0)

        def warmup(nmm):
            psD = psA.tile([P, 512], f32, tag="acc")
            for i in range(nmm):
                nc.tensor.matmul(
                    psD[:], scratch[:, 0:P], scratch[:],
                    start=(i == 0), stop=(i == nmm - 1),
                )

        # Generators yield once per emitted PE matmul so attention blocks can
        # interleave them into PE slack at a controlled rate (the per-engine
        # instruction streams execute strictly in program order).
        def gen_proj_k(a, nns=(0, 1)):
            # trailing copies are emitted BEFORE the final yield so that a
            # fully-popped generator has fully emitted its writes
            for nn in nns:
                ps = psA.tile([P, 512], f32, tag="acc")
                for c in range(KC_KV):
                    nc.tensor.matmul(
                        ps[:],
                        wk_sb[:, c, a * P:(a + 1) * P],
                        encT_sb[:, c, nn * 512:(nn + 1) * 512],
                        start=(c == 0),
                        stop=(c == KC_KV - 1),
                    )
                    if c < KC_KV - 1:
                        yield
                with nc.allow_low_precision(reason="k stored fp8 for DR scores"):
                    nc.vector.tensor_copy(
                        kT8[:, a, nn * 512:(nn + 1) * 512], ps[:]
                    )
                yield

        def gen_proj_q(a, n):
            ps = psA.tile([P, 512], f32, tag="acc")
            for c in range(KC_Q):
                nc.tensor.matmul(
                    ps[:],
                    wq_sb[:, c, a * P:(a + 1) * P],
                    hsT_sb[:, c, n * 512:(n + 1) * 512],
                    start=(c == 0),
                    stop=(c == KC_Q - 1),
                )
                if c < KC_Q - 1:
                    yield
            sl = slice(n * 512, (n + 1) * 512)
            with nc.allow_low_precision(reason="q stored as fp8 hi/lo pair"):
                nc.vector.tensor_copy(qT8[:, a, 0, sl], ps[:])
                nc.vector.tensor_sub(qT8[:, a, 1, sl], ps[:], qT8[:, a, 0, sl])
            yield

        # v natural [kv, slots]: slot h (128 wide):
        #   h even: [v_h (0:64) | 1.0 at 64 | 0 at 65:128]   -> out rows 0:64, denom row 64
        #   h odd : [1.0 at 0 | 0 at 1:64 | v_h at 64:128]   -> out rows 64:128, denom row 0
        def v_memsets():
            nc.gpsimd.memset(vv4[:, :, 0::2, 64:65], 1.0)
            nc.gpsimd.memset(vv4[:, :, 1::2, 0:1], 1.0)
            nc.gpsimd.memset(vv4[:, :, 0::2, 65:P], 0.0)
            nc.gpsimd.memset(vv4[:, :, 1::2, 1:DH], 0.0)

        def gen_proj_v(t):
            ps = psA.tile([P, 512], f32, tag="acc")
            for c in range(KC_KV):
                nc.tensor.matmul(
                    ps[:],
                    encT_sb[:, c, t * P:(t + 1) * P],
                    wv_sb[:, c, :],
                    start=(c == 0),
                    stop=(c == KC_KV - 1),
                )
                if c < KC_KV - 1:
                    yield
            pv = ps.rearrange("p (s c) -> p s c", c=DH)
            nc.vector.tensor_copy(vv4[:, t, 0::2, 0:DH], pv[:, 0::2, :])
            nc.vector.tensor_copy(vv4[:, t, 1::2, DH:P], pv[:, 1::2, :])
            yield

        def gen_final(m):
            ps = psA.tile([P, 512], f32, tag="acc")
            for a in range(AT):
                nc.tensor.matmul(
                    ps[:],
                    outT_sb[:, a, m * P:(m + 1) * P],
                    wo_sb[:, a, :],
                    start=(a == 0),
                    stop=(a == AT - 1),
                )
                if a < AT - 1:
                    yield
            ob = outp.tile([P, QD], f32)
            nc.vector.tensor_add(ob[:], ps[:], bo_sb[:])
            nc.sync.dma_start(out_d[m * P:(m + 1) * P, :], ob[:])
            yield

        # final projection split for the tail m-tiles: partA (heads 0-1)
        # accumulates into an SBUF staging tile during earlier blocks; partB
        # (heads 2-3) only trails the last attention block
        facc = acts.tile([P, QSH // P, QD], f32)

        def gen_final_a(m):
            ps = psA.tile([P, 512], f32, tag="acc")
            for a in (0, 1):
                nc.tensor.matmul(
                    ps[:],
                    outT_sb[:, a, m * P:(m + 1) * P],
                    wo_sb[:, a, :],
                    start=(a == 0),
                    stop=(a == 1),
                )
                if a == 0:
                    yield
            nc.vector.tensor_add(facc[:, m, :], ps[:], bo_sb[:])
            yield

        def gen_final_b(m):
            ps = psA.tile([P, 512], f32, tag="acc")
            for a in (2, 3):
                nc.tensor.matmul(
                    ps[:],
                    outT_sb[:, a, m * P:(m + 1) * P],
                    wo_sb[:, a, :],
                    start=(a == 2),
                    stop=(a == 3),
                )
                if a == 2:
                    yield
            ob = outp.tile([P, QD], f32)
            nc.vector.tensor_add(ob[:], ps[:], facc[:, m, :])
            nc.sync.dma_start(out_d[m * P:(m + 1) * P, :], ob[:])
            yield

        def gen_chain(*gens):
            for g in gens:
                yield from g

        def run_gen(g):
            for _ in g:
                pass

        def attn(hp, n, extras=None, pre_pop=0, per_iter=0, prev_tail=None,
                 drain=True, act_copy_norm=False):
            """Emit one attention block.  Returns a closure that emits the
            block's last two AV matmuls + normalize; the caller passes it to
            the NEXT block so those trail instructions interleave with the
            next block's leading scores (removes the block-boundary bubble).
            """
            if extras is None:
                extras = iter(())

            def pop(k):
                for _ in range(k):
                    if next(extras, StopIteration) is StopIteration:
                        break

            av0 = psA.tile([P, 512], f32, tag="acc")
            av1 = psA.tile([P, 512], f32, tag="acc")
            av = (av0, av1)
            exs = []

            def s_(t):
                ss = psS.tile([P, 1024], f32)
                for i in range(2):
                    pr = slice(i * 64, (i + 1) * 64)
                    nc.tensor.matmul(
                        ss[:, i * 512:(i + 1) * 512],
                        kT8[pr, hp, t * P:(t + 1) * P]
                        .unsqueeze(1).broadcast_to((64, 2, P)),
                        qT8[pr, hp, :, n * 512:(n + 1) * 512],
                        start=True,
                        stop=True,
                        perf_mode=DR,
                    )
                ex = expp.tile([P, 1024], f16)
                nc.scalar.activation(
                    ex[:], ss[:], Exp, scale=SCALE / (WSCALE * WSCALE)
                )
                exs.append(ex)

            def A_(t):
                for i in range(2):
                    s = 2 * hp + i
                    nc.tensor.matmul(
                        av[i][:],
                        v_sb[:, t, s * P:(s + 1) * P],
                        exs[t][:, i * 512:(i + 1) * 512],
                        start=(t == 0),
                        stop=(t == NT - 1),
                    )

            s_(0)
            s_(1)
            pop(pre_pop)
            if prev_tail is not None:
                prev_tail()
            for t in range(2, NT):
                s_(t)
                A_(t - 2)
                pop(per_iter)
            if drain:  # drain leftovers so every generator completes
                for _ in extras:
                    pass

            def tail():
                A_(NT - 2)
                A_(NT - 1)
                # partition broadcast of 1/denom via a PE ones-column matmul
                # (GpSimd partition_broadcast proved flaky on HW; the DMA
                # round-trip costs ~4us per block).  The reciprocal lands on
                # partition 0 in f16, ones.T @ recip fills a PSUM tile,
                # which is copied to SBUF for the multiply (ScalarE for the
                # last block where it is idle, DVE elsewhere).
                for i in range(2):
                    drow = 64 if i == 0 else 0
                    dst = slice(0, 64) if i == 0 else slice(64, 128)
                    rc = small.tile([1, 512], f16, tag="rc")
                    with nc.allow_low_precision(
                        reason="softmax denom reciprocal, f16 suffices"
                    ):
                        nc.vector.reciprocal(
                            rc[0:1, :], av[i][drow:drow + 1, :]
                        )
                    rcps = psA.tile([P, 512], f32, tag="acc")
                    nc.tensor.matmul(
                        rcps[:], ones_sb[0:1, :], rc[0:1, :],
                        start=True, stop=True,
                    )
                    rcb = small.tile([P, 512], f32, tag="rcb")
                    if act_copy_norm:
                        nc.scalar.copy(rcb[:], rcps[:])
                    else:
                        nc.vector.tensor_copy(rcb[:], rcps[:])
                    nc.vector.tensor_mul(
                        outT_sb[dst, hp, n * 512:(n + 1) * 512],
                        av[i][dst, :],
                        rcb[dst, :],
                    )

            return tail

        # ---- emission = per-engine execution order.  Warmup dummies bridge
        # the DMA head up to qT(0,0); kT(0) kv-half 0 slots into the gap as
        # soon as its DMA lands; everything else (v, kT second half, later
        # k/q projections, finals) interleaves into attention-block PE slack.
        v_memsets()
        warmup(CFG["W1"])
        for a in range(AT):
            run_gen(gen_proj_q(a, 0))
        run_gen(gen_proj_q(0, 1))
        run_gen(gen_proj_k(0))
        tail = attn(
            0, 0,
            extras=gen_chain(
                *[gen_proj_v(t) for t in range(NT)],
                gen_proj_k(1),
            ),
            pre_pop=CFG["B0_PRE"], per_iter=CFG["B0_PER"],
        )
        tail = attn(1, 0, extras=gen_chain(gen_proj_k(2), gen_proj_q(1, 1)),
                    pre_pop=CFG["BK_PRE"], per_iter=CFG["BK_PER"],
                    prev_tail=tail)
        tail = attn(2, 0, extras=gen_chain(gen_proj_k(3), gen_proj_q(2, 1)),
                    pre_pop=CFG["BK_PRE"], per_iter=CFG["BK_PER"],
                    prev_tail=tail)
        tail = attn(3, 0, extras=gen_proj_q(3, 1), pre_pop=0, per_iter=1,
                    prev_tail=tail)
        f01 = gen_chain(gen_final(0), gen_final(1))
        tail = attn(0, 1, extras=f01, pre_pop=0, per_iter=1,
                    prev_tail=tail, drain=False)
        tail = attn(1, 1, extras=gen_chain(f01, gen_final(2)),
                    pre_pop=0, per_iter=1, prev_tail=tail)
        tail = attn(2, 1,
                    extras=gen_chain(gen_final(3), gen_final_a(4)),
                    pre_pop=0, per_iter=CFG["B21_PER"], prev_tail=tail)
        tail = attn(3, 1, act_copy_norm=True,
                    extras=gen_chain(gen_final_a(5), gen_final_a(6),
                                     gen_final_a(7)),
                    pre_pop=0, per_iter=CFG["B31_PER"], prev_tail=tail)
        tail()
        for m in range(AT, QSH // P):
            run_gen(gen_final_b(m))

    nc.finalize()
    return nc


def _get_nc():
    if "nc" not in _CACHE:
        _CACHE["nc"] = _build_nc()
    return _CACHE["nc"]


def make_in_maps(hidden_states, encoder_hidden_states, Wq, Wk, Wv, Wo, bo):
    f16 = np.float16
    hs = np.asarray(hidden_states, dtype=np.float32)
    enc = np.asarray(encoder_hidden_states, dtype=np.float32)
    wq = np.ascontiguousarray(np.asarray(Wq, dtype=np.float32) * WSCALE).astype(f16)
    wk = np.ascontiguousarray(np.asarray(Wk, dtype=np.float32) * WSCALE).astype(f16)
    wv = np.ascontiguousarray(np.asarray(Wv, dtype=np.float32)).astype(f16)
    wo = np.ascontiguousarray(np.asarray(Wo, dtype=np.float32)).astype(f16)
    bo_ = np.ascontiguousarray(np.asarray(bo, dtype=np.float32)).reshape(1, QD)
    encT = [np.ascontiguousarray(enc[b].T).astype(f16) for b in range(B)]
    in_maps = []
    for c in range(NCORES):
        b, s = divmod(c, 4)
        hsT = np.ascontiguousarray(hs[b, s * QSH:(s + 1) * QSH, :].T).astype(f16)
        in_maps.append(
            dict(hsT=hsT, encT=encT[b], wq=wq, wk=wk, wv=wv, wo=wo, bo=bo_)
        )
    return in_maps


def kernel(hidden_states, encoder_hidden_states, Wq, Wk, Wv, Wo, bo):
    global LAST_RESULTS
    from concourse.bass_utils import run_bass_kernel_spmd

    nc = _get_nc()
    in_maps = make_in_maps(
        hidden_states, encoder_hidden_states, Wq, Wk, Wv, Wo, bo
    )
    res = run_bass_kernel_spmd(nc, in_maps, core_ids=list(range(NCORES)))
    LAST_RESULTS = res
    out = np.empty((B, LQ, QD), dtype=np.float32)
    for c in range(NCORES):
        b, s = divmod(c, 4)
        out[b, s * QSH:(s + 1) * QSH, :] = res.results[c]["out"]
    return out



# revision 32
# speedup vs baseline: 1.0225x; 1.0225x over previous
"""CrossAttention kernel for 8 TRN2 NeuronCores.

Reference computation (B=2, Lq=4096, Lkv=1024, query_dim=512, cross_dim=768,
heads=8, dim_head=64, inner=512):
    q = hs @ Wq; k = enc @ Wk; v = enc @ Wv          (per batch)
    attn = softmax(q_h @ k_h^T * scale) per head
    out = concat_h(attn @ v_h) @ Wo + bo

Sharding: 8 cores = 2 batches x 4 query-slices of 1024 queries.  Each core
computes its full slice of the output (all heads), so outputs are disjoint
and no collective is needed.

Per-core dataflow (all matmuls f16 operands, fp32 PSUM accumulate):
  - host passes hs-slice and encoder transposed (hsT [512,1024], encT
    [768,1024]) already cast to f16, weights in f16
  - qT = Wq^T-weighted hsT   -> [inner=512, q=1024]  (heads along partitions)
  - kT likewise              -> [inner=512, kv=1024]
  - v natural                -> [kv=1024, slots]  slot h = 128 cols holding
        v_h (64) + a ones column + zero padding, arranged so the AV matmul
        output lands partition-aligned with head h's rows of outT and the
        softmax denominator (sum_kv exp) falls out of the same matmul.
  - scoresT_h = k_h qT_h     -> [kv, q] (kv on partitions; head pairs use
        base-partition row tiling of the 128x128 PE array)
  - expT = exp(scale * scoresT) on ScalarE, f16 out (no max-subtraction:
        |scaled scores| < ~3)
  - outT_unnorm_h = v_slot^T @ expT accumulated over kv chunks (PSUM),
        one row of which is the softmax denominator
  - normalize: reciprocal (DVE) + PE ones-column broadcast matmul + multiply
  - final = outT^T @ Wo + bo -> [1024, 512], DMA out per 128-row tile

Program order is pipelined for the Tile scheduler: k/v/q projections are
emitted ahead of the attention blocks that consume them, exp(t) is emitted
before AV(t-1) so the PE never waits in-order on ScalarE, and the final
projection m-tiles are interleaved between the last attention blocks.
"""

import sys

if "/opt/trn_rl_repo" not in sys.path:
    sys.path.insert(0, "/opt/trn_rl_repo")

import numpy as np

B, LQ, LKV = 2, 4096, 1024
QD, CD = 512, 768
H, DH = 8, 64
INNER = H * DH  # 512
SCALE = DH ** -0.5
NCORES = 8
WSCALE = 8.0    # host-side pre-scale on Wq (fp8 range centering)
WSCALE_K = 32.0  # host-side pre-scale on Wk (fp8 range centering)
QSH = LQ // 4  # 1024 queries per core
P = 128

_CACHE: dict = {}
LAST_RESULTS = None  # test harness introspection (exec_time_ns etc.)

# schedule-tuning knobs (sweepable from bench tooling)
CFG = {
    "W1": 12,       # warmup matmuls bridging the input-DMA head
    "B0_PRE": 8,    # block (0,0) pre-loop extra pops
    "B0_PER": 9,    # block (0,0) per-iter extra pops
    "BK_PRE": 2,    # later n=0 blocks pre-loop pops
    "BK_PER": 2,    # later n=0 blocks per-iter pops
    "B21_PER": 1,   # block (2,1) per-iter pops
    "B31_PER": 1,   # block (3,1) per-iter pops
}


def _build_nc():
    from contextlib import ExitStack

    import concourse.bass as bass
    import concourse.tile as tile
    from concourse import bacc, mybir

    f32 = mybir.dt.float32
    f16 = mybir.dt.float16
    f8 = mybir.dt.float8e4
    DR = mybir.MatmulPerfMode.DoubleRow
    Exp = mybir.ActivationFunctionType.Exp

    nc = bacc.Bacc(trn_type="TRN2")

    hsT_d = nc.declare_dram_parameter("hsT", [QD, QSH], f16, isOutput=False)
    encT_d = nc.declare_dram_parameter("encT", [CD, LKV], f16, isOutput=False)
    wq_d = nc.declare_dram_parameter("wq", [QD, INNER], f16, isOutput=False)
    wk_d = nc.declare_dram_parameter("wk", [CD, INNER], f16, isOutput=False)
    wv_d = nc.declare_dram_parameter("wv", [CD, INNER], f16, isOutput=False)
    wo_d = nc.declare_dram_parameter("wo", [INNER, QD], f16, isOutput=False)
    bo_d = nc.declare_dram_parameter("bo", [1, QD], f32, isOutput=False)
    wk8_d = nc.declare_dram_parameter("wk8", [CD, INNER], f8, isOutput=False)
    encT8_d = nc.declare_dram_parameter("encT8", [CD, 2, LKV], f8, isOutput=False)
    out_d = nc.declare_dram_parameter("out", [QSH, QD], f32, isOutput=True)

    KC_Q = QD // P   # 4 contraction chunks for q projection
    KC_KV = CD // P  # 6 for k/v projections
    AT = INNER // P  # 4 inner tiles (2 heads each)
    NT = LKV // P    # 8 kv chunks
    QN = QSH // 512  # 2 q slices of 512

    with ExitStack() as ctx:
        tc = ctx.enter_context(tile.TileContext(nc))
        const = ctx.enter_context(tc.tile_pool(name="const", bufs=1))
        acts = ctx.enter_context(tc.tile_pool(name="acts", bufs=1))
        expp = ctx.enter_context(tc.tile_pool(name="expp", bufs=4))
        outp = ctx.enter_context(tc.tile_pool(name="outp", bufs=4))
        small = ctx.enter_context(tc.tile_pool(name="small", bufs=6))
        psA = ctx.enter_context(tc.tile_pool(name="psA", bufs=4, space="PSUM"))
        psS = ctx.enter_context(tc.tile_pool(name="psS", bufs=2, space="PSUM"))
        drp = ctx.enter_context(tc.tile_pool(name="drp", bufs=4, space="DRAM"))

        # ---- input DMA, ordered by first use: the q projections (hsT+wq)
        # run during the PE warmup window, then kT (encT+wk), then v (wv);
        # the second encT half only gates scores t>=4 of the first block
        hsT_sb = acts.tile([P, KC_Q, QSH], f16)
        nc.sync.dma_start(hsT_sb[:], hsT_d.rearrange("(c p) n -> p c n", p=P))
        wq_sb = const.tile([P, KC_Q, INNER], f16)
        nc.sync.dma_start(wq_sb[:], wq_d.rearrange("(c p) n -> p c n", p=P))
        encT_sb = acts.tile([P, KC_KV, LKV], f16)
        encT_r = encT_d.rearrange("(c p) n -> p c n", p=P)
        nc.sync.dma_start(encT_sb[:, :, 0:512], encT_r[:, :, 0:512])
        wk_sb = const.tile([P, KC_KV, INNER], f16)
        nc.sync.dma_start(wk_sb[:], wk_d.rearrange("(c p) n -> p c n", p=P))
        nc.sync.dma_start(encT_sb[:, :, 512:1024], encT_r[:, :, 512:1024])
        wv_sb = const.tile([P, KC_KV, INNER], f16)
        nc.sync.dma_start(wv_sb[:], wv_d.rearrange("(c p) n -> p c n", p=P))
        # fp8 operands for the DoubleRow k projections (k heads 2..7): the
        # encoder comes as an fp8 (hi, lo) plane pair; wk8 rides both planes
        # of the stationary operand via a stride-0 broadcast
        encT8_sb = acts.tile([P, KC_KV, 2, LKV], f8)
        encT8_r = encT8_d.rearrange("(c p) j n -> p c j n", p=P)
        wk8_sb = const.tile([P, KC_KV, INNER], f8)
        nc.sync.dma_start(encT8_sb[:, :, 0, 0:512], encT8_r[:, :, 0, 0:512])
        nc.sync.dma_start(encT8_sb[:, :, 1, 0:512], encT8_r[:, :, 1, 0:512])
        nc.sync.dma_start(wk8_sb[:], wk8_d.rearrange("(c p) n -> p c n", p=P))
        nc.sync.dma_start(encT8_sb[:, :, 0, 512:1024], encT8_r[:, :, 0, 512:1024])
        nc.sync.dma_start(encT8_sb[:, :, 1, 512:1024], encT8_r[:, :, 1, 512:1024])
        wo_sb = const.tile([P, AT, QD], f16)
        nc.sync.dma_start(wo_sb[:], wo_d.rearrange("(c p) n -> p c n", p=P))
        bo_sb = const.tile([P, QD], f32)
        nc.sync.dma_start(bo_sb[:], bo_d.ap().to_broadcast((P, QD)))

        qT8 = acts.tile([P, AT, 2, QSH], f8)       # planes: (hi, lo)
        kT8 = acts.tile([P, AT, LKV], f8)          # single plane, broadcast in DR
        v_sb = acts.tile([P, NT, H * P], f16)
        outT_sb = acts.tile([P, AT, QSH], f16)
        vv4 = v_sb.rearrange("p t (s c) -> p t s c", c=P)

        # ---- PE warmup: dummy matmuls on zeroed scratch fill the DMA head
        # so the first real matmuls run at full clock (psD is never read)
        scratch = acts.tile([P, 512], f16)
        nc.gpsimd.memset(scratch[:], 0.0)

        # ones column for the PE-side partition broadcast in normalize
        ones_sb = const.tile([1, P], f16)
        nc.vector.memset(ones_sb[:], 1.0)

        def warmup(nmm):
            psD = psA.tile([P, 512], f32, tag="acc")
            for i in range(nmm):
                nc.tensor.matmul(
                    psD[:], scratch[:, 0:P], scratch[:],
                    start=(i == 0), stop=(i == nmm - 1),
                )

        # Generators yield once per emitted PE matmul so attention blocks can
        # interleave them into PE slack at a controlled rate (the per-engine
        # instruction streams execute strictly in program order).
        def gen_proj_k(a, nns=(0, 1)):
            # trailing copies are emitted BEFORE the final yield so that a
            # fully-popped generator has fully emitted its writes.
            # a=0 runs in fp16 off the early fp16 encoder DMA; a>=1 runs as
            # one-sided-fp8 DoubleRow (half PE cost; only the wk8 side
            # carries fp8 error, which softmax normalization absorbs).
            for nn in nns:
                ps = psA.tile([P, 512], f32, tag="acc")
                for c in range(KC_KV):
                    if a == 0:
                        nc.tensor.matmul(
                            ps[:],
                            wk_sb[:, c, a * P:(a + 1) * P],
                            encT_sb[:, c, nn * 512:(nn + 1) * 512],
                            start=(c == 0),
                            stop=(c == KC_KV - 1),
                        )
                    else:
                        nc.tensor.matmul(
                            ps[:],
                            wk8_sb[:, c, a * P:(a + 1) * P]
                            .unsqueeze(1).broadcast_to((P, 2, P)),
                            encT8_sb[:, c, :, nn * 512:(nn + 1) * 512],
                            start=(c == 0),
                            stop=(c == KC_KV - 1),
                            perf_mode=DR,
                        )
                    if c < KC_KV - 1:
                        yield
                with nc.allow_low_precision(reason="k stored fp8 for DR scores"):
                    nc.vector.tensor_copy(
                        kT8[:, a, nn * 512:(nn + 1) * 512], ps[:]
                    )
                yield

        def gen_proj_q(a, n):
            ps = psA.tile([P, 512], f32, tag="acc")
            for c in range(KC_Q):
                nc.tensor.matmul(
                    ps[:],
                    wq_sb[:, c, a * P:(a + 1) * P],
                    hsT_sb[:, c, n * 512:(n + 1) * 512],
                    start=(c == 0),
                    stop=(c == KC_Q - 1),
                )
                if c < KC_Q - 1:
                    yield
            sl = slice(n * 512, (n + 1) * 512)
            with nc.allow_low_precision(reason="q stored as fp8 hi/lo pair"):
                nc.vector.tensor_copy(qT8[:, a, 0, sl], ps[:])
                nc.vector.tensor_sub(qT8[:, a, 1, sl], ps[:], qT8[:, a, 0, sl])
            yield

        # v natural [kv, slots]: slot h (128 wide):
        #   h even: [v_h (0:64) | 1.0 at 64 | 0 at 65:128]   -> out rows 0:64, denom row 64
        #   h odd : [1.0 at 0 | 0 at 1:64 | v_h at 64:128]   -> out rows 64:128, denom row 0
        def v_memsets():
            nc.gpsimd.memset(vv4[:, :, 0::2, 64:65], 1.0)
            nc.gpsimd.memset(vv4[:, :, 1::2, 0:1], 1.0)
            nc.gpsimd.memset(vv4[:, :, 0::2, 65:P], 0.0)
            nc.gpsimd.memset(vv4[:, :, 1::2, 1:DH], 0.0)

        def gen_proj_v(t):
            ps = psA.tile([P, 512], f32, tag="acc")
            for c in range(KC_KV):
                nc.tensor.matmul(
                    ps[:],
                    encT_sb[:, c, t * P:(t + 1) * P],
                    wv_sb[:, c, :],
                    start=(c == 0),
                    stop=(c == KC_KV - 1),
                )
                if c < KC_KV - 1:
                    yield
            pv = ps.rearrange("p (s c) -> p s c", c=DH)
            nc.vector.tensor_copy(vv4[:, t, 0::2, 0:DH], pv[:, 0::2, :])
            nc.vector.tensor_copy(vv4[:, t, 1::2, DH:P], pv[:, 1::2, :])
            yield

        def gen_final(m):
            ps = psA.tile([P, 512], f32, tag="acc")
            for a in range(AT):
                nc.tensor.matmul(
                    ps[:],
                    outT_sb[:, a, m * P:(m + 1) * P],
                    wo_sb[:, a, :],
                    start=(a == 0),
                    stop=(a == AT - 1),
                )
                if a < AT - 1:
                    yield
            ob = outp.tile([P, QD], f32)
            nc.vector.tensor_add(ob[:], ps[:], bo_sb[:])
            nc.sync.dma_start(out_d[m * P:(m + 1) * P, :], ob[:])
            yield

        # final projection split for the tail m-tiles: partA (heads 0-1)
        # accumulates into an SBUF staging tile during earlier blocks; partB
        # (heads 2-3) only trails the last attention block
        facc = acts.tile([P, QSH // P, QD], f32)

        def gen_final_a(m):
            ps = psA.tile([P, 512], f32, tag="acc")
            for a in (0, 1):
                nc.tensor.matmul(
                    ps[:],
                    outT_sb[:, a, m * P:(m + 1) * P],
                    wo_sb[:, a, :],
                    start=(a == 0),
                    stop=(a == 1),
                )
                if a == 0:
                    yield
            nc.vector.tensor_add(facc[:, m, :], ps[:], bo_sb[:])
            yield

        def gen_final_b(m):
            ps = psA.tile([P, 512], f32, tag="acc")
            for a in (2, 3):
                nc.tensor.matmul(
                    ps[:],
                    outT_sb[:, a, m * P:(m + 1) * P],
                    wo_sb[:, a, :],
                    start=(a == 2),
                    stop=(a == 3),
                )
                if a == 2:
                    yield
            ob = outp.tile([P, QD], f32)
            nc.vector.tensor_add(ob[:], ps[:], facc[:, m, :])
            nc.sync.dma_start(out_d[m * P:(m + 1) * P, :], ob[:])
            yield

        def gen_chain(*gens):
            for g in gens:
                yield from g

        def run_gen(g):
            for _ in g:
                pass

        def attn(hp, n, extras=None, pre_pop=0, per_iter=0, prev_tail=None,
                 drain=True, act_copy_norm=False):
            """Emit one attention block.  Returns a closure that emits the
            block's last two AV matmuls + normalize; the caller passes it to
            the NEXT block so those trail instructions interleave with the
            next block's leading scores (removes the block-boundary bubble).
            """
            if extras is None:
                extras = iter(())

            def pop(k):
                for _ in range(k):
                    if next(extras, StopIteration) is StopIteration:
                        break

            av0 = psA.tile([P, 512], f32, tag="acc")
            av1 = psA.tile([P, 512], f32, tag="acc")
            av = (av0, av1)
            exs = []

            def s_(t):
                ss = psS.tile([P, 1024], f32)
                for i in range(2):
                    pr = slice(i * 64, (i + 1) * 64)
                    nc.tensor.matmul(
                        ss[:, i * 512:(i + 1) * 512],
                        kT8[pr, hp, t * P:(t + 1) * P]
                        .unsqueeze(1).broadcast_to((64, 2, P)),
                        qT8[pr, hp, :, n * 512:(n + 1) * 512],
                        start=True,
                        stop=True,
                        perf_mode=DR,
                    )
                ex = expp.tile([P, 1024], f16)
                nc.scalar.activation(
                    ex[:], ss[:], Exp, scale=SCALE / (WSCALE * WSCALE_K)
                )
                exs.append(ex)

            def A_(t):
                for i in range(2):
                    s = 2 * hp + i
                    nc.tensor.matmul(
                        av[i][:],
                        v_sb[:, t, s * P:(s + 1) * P],
                        exs[t][:, i * 512:(i + 1) * 512],
                        start=(t == 0),
                        stop=(t == NT - 1),
                    )

            s_(0)
            s_(1)
            pop(pre_pop)
            if prev_tail is not None:
                prev_tail()
            for t in range(2, NT):
                s_(t)
                A_(t - 2)
                pop(per_iter)
            if drain:  # drain leftovers so every generator completes
                for _ in extras:
                    pass

            def tail():
                A_(NT - 2)
                A_(NT - 1)
                # partition broadcast of 1/denom via a PE ones-column matmul
                # (GpSimd partition_broadcast proved flaky on HW; the DMA
                # round-trip costs ~4us per block).  The reciprocal lands on
                # partition 0 in f16, ones.T @ recip fills a PSUM tile,
                # which is copied to SBUF for the multiply (ScalarE for the
                # last block where it is idle, DVE elsewhere).
                for i in range(2):
                    drow = 64 if i == 0 else 0
                    dst = slice(0, 64) if i == 0 else slice(64, 128)
                    rc = small.tile([1, 512], f16, tag="rc")
                    with nc.allow_low_precision(
                        reason="softmax denom reciprocal, f16 suffices"
                    ):
                        nc.vector.reciprocal(
                            rc[0:1, :], av[i][drow:drow + 1, :]
                        )
                    rcps = psA.tile([P, 512], f32, tag="acc")
                    nc.tensor.matmul(
                        rcps[:], ones_sb[0:1, :], rc[0:1, :],
                        start=True, stop=True,
                    )
                    rcb = small.tile([P, 512], f32, tag="rcb")
                    if act_copy_norm:
                        nc.scalar.copy(rcb[:], rcps[:])
                    else:
                        nc.vector.tensor_copy(rcb[:], rcps[:])
                    nc.vector.tensor_mul(
                        outT_sb[dst, hp, n * 512:(n + 1) * 512],
                        av[i][dst, :],
                        rcb[dst, :],
                    )

            return tail

        # ---- emission = per-engine execution order.  Warmup dummies bridge
        # the DMA head up to qT(0,0); kT(0) kv-half 0 slots into the gap as
        # soon as its DMA lands; everything else (v, kT second half, later
        # k/q projections, finals) interleaves into attention-block PE slack.
        v_memsets()
        warmup(CFG["W1"])
        for a in range(AT):
            run_gen(gen_proj_q(a, 0))
        run_gen(gen_proj_q(0, 1))
        run_gen(gen_proj_k(0))
        tail = attn(
            0, 0,
            extras=gen_chain(
                *[gen_proj_v(t) for t in range(NT)],
                gen_proj_k(1),
            ),
            pre_pop=CFG["B0_PRE"], per_iter=CFG["B0_PER"],
        )
        tail = attn(1, 0, extras=gen_chain(gen_proj_k(2), gen_proj_q(1, 1)),
                    pre_pop=CFG["BK_PRE"], per_iter=CFG["BK_PER"],
                    prev_tail=tail)
        tail = attn(2, 0, extras=gen_chain(gen_proj_k(3), gen_proj_q(2, 1)),
                    pre_pop=CFG["BK_PRE"], per_iter=CFG["BK_PER"],
                    prev_tail=tail)
        tail = attn(3, 0, extras=gen_proj_q(3, 1), pre_pop=0, per_iter=1,
                    prev_tail=tail)
        f01 = gen_chain(gen_final(0), gen_final(1))
        tail = attn(0, 1, extras=f01, pre_pop=0, per_iter=1,
                    prev_tail=tail, drain=False)
        tail = attn(1, 1, extras=gen_chain(f01, gen_final(2)),
                    pre_pop=0, per_iter=1, prev_tail=tail)
        tail = attn(2, 1,
                    extras=gen_chain(gen_final(3), gen_final_a(4)),
                    pre_pop=0, per_iter=CFG["B21_PER"], prev_tail=tail)
        tail = attn(3, 1, act_copy_norm=True,
                    extras=gen_chain(gen_final_a(5), gen_final_a(6),
                                     gen_final_a(7)),
                    pre_pop=0, per_iter=CFG["B31_PER"], prev_tail=tail)
        tail()
        for m in range(AT, QSH // P):
            run_gen(gen_final_b(m))

    nc.finalize()
    return nc


def _get_nc():
    if "nc" not in _CACHE:
        _CACHE["nc"] = _build_nc()
    return _CACHE["nc"]


def make_in_maps(hidden_states, encoder_hidden_states, Wq, Wk, Wv, Wo, bo):
    f16 = np.float16
    hs = np.asarray(hidden_states, dtype=np.float32)
    enc = np.asarray(encoder_hidden_states, dtype=np.float32)
    import ml_dtypes

    f8 = ml_dtypes.float8_e4m3
    wq = np.ascontiguousarray(np.asarray(Wq, dtype=np.float32) * WSCALE).astype(f16)
    wk32 = np.asarray(Wk, dtype=np.float32) * WSCALE_K
    wk = np.ascontiguousarray(wk32).astype(f16)
    wk8 = np.ascontiguousarray(wk32).astype(f8)
    wv = np.ascontiguousarray(np.asarray(Wv, dtype=np.float32)).astype(f16)
    wo = np.ascontiguousarray(np.asarray(Wo, dtype=np.float32)).astype(f16)
    bo_ = np.ascontiguousarray(np.asarray(bo, dtype=np.float32)).reshape(1, QD)
    encT = [np.ascontiguousarray(enc[b].T).astype(f16) for b in range(B)]
    encT8 = []
    for b in range(B):
        e = enc[b].T.astype(np.float32)
        hi = e.astype(f8)
        lo = (e - hi.astype(np.float32)).astype(f8)
        encT8.append(np.ascontiguousarray(np.stack([hi, lo], axis=1)))
    in_maps = []
    for c in range(NCORES):
        b, s = divmod(c, 4)
        hsT = np.ascontiguousarray(hs[b, s * QSH:(s + 1) * QSH, :].T).astype(f16)
        in_maps.append(
            dict(hsT=hsT, encT=encT[b], wq=wq, wk=wk, wv=wv, wo=wo, bo=bo_,
                 wk8=wk8, encT8=encT8[b])
        )
    return in_maps


def kernel(hidden_states, encoder_hidden_states, Wq, Wk, Wv, Wo, bo):
    global LAST_RESULTS
    from concourse.bass_utils import run_bass_kernel_spmd

    nc = _get_nc()
    in_maps = make_in_maps(
        hidden_states, encoder_hidden_states, Wq, Wk, Wv, Wo, bo
    )
    res = run_bass_kernel_spmd(nc, in_maps, core_ids=list(range(NCORES)))
    LAST_RESULTS = res
    out = np.empty((B, LQ, QD), dtype=np.float32)
    for c in range(NCORES):
        b, s = divmod(c, 4)
        out[b, s * QSH:(s + 1) * QSH, :] = res.results[c]["out"]
    return out



# revision 37
# speedup vs baseline: 1.0276x; 1.0049x over previous
"""CrossAttention kernel for 8 TRN2 NeuronCores.

Reference computation (B=2, Lq=4096, Lkv=1024, query_dim=512, cross_dim=768,
heads=8, dim_head=64, inner=512):
    q = hs @ Wq; k = enc @ Wk; v = enc @ Wv          (per batch)
    attn = softmax(q_h @ k_h^T * scale) per head
    out = concat_h(attn @ v_h) @ Wo + bo

Sharding: 8 cores = 2 batches x 4 query-slices of 1024 queries.  Each core
computes its full slice of the output (all heads), so outputs are disjoint
and no collective is needed.

Per-core dataflow (all matmuls f16 operands, fp32 PSUM accumulate):
  - host passes hs-slice and encoder transposed (hsT [512,1024], encT
    [768,1024]) already cast to f16, weights in f16
  - qT = Wq^T-weighted hsT   -> [inner=512, q=1024]  (heads along partitions)
  - kT likewise              -> [inner=512, kv=1024]
  - v natural                -> [kv=1024, slots]  slot h = 128 cols holding
        v_h (64) + a ones column + zero padding, arranged so the AV matmul
        output lands partition-aligned with head h's rows of outT and the
        softmax denominator (sum_kv exp) falls out of the same matmul.
  - scoresT_h = k_h qT_h     -> [kv, q] (kv on partitions; head pairs use
        base-partition row tiling of the 128x128 PE array)
  - expT = exp(scale * scoresT) on ScalarE, f16 out (no max-subtraction:
        |scaled scores| < ~3)
  - outT_unnorm_h = v_slot^T @ expT accumulated over kv chunks (PSUM),
        one row of which is the softmax denominator
  - normalize: reciprocal (DVE) + PE ones-column broadcast matmul + multiply
  - final = outT^T @ Wo + bo -> [1024, 512], DMA out per 128-row tile

Program order is pipelined for the Tile scheduler: k/v/q projections are
emitted ahead of the attention blocks that consume them, exp(t) is emitted
before AV(t-1) so the PE never waits in-order on ScalarE, and the final
projection m-tiles are interleaved between the last attention blocks.
"""

import sys

if "/opt/trn_rl_repo" not in sys.path:
    sys.path.insert(0, "/opt/trn_rl_repo")

import numpy as np

B, LQ, LKV = 2, 4096, 1024
QD, CD = 512, 768
H, DH = 8, 64
INNER = H * DH  # 512
SCALE = DH ** -0.5
NCORES = 8
WSCALE = 8.0    # host-side pre-scale on Wq (fp8 range centering)
WSCALE_K = 32.0  # host-side pre-scale on Wk (fp8 range centering)
QSH = LQ // 4  # 1024 queries per core
P = 128

_CACHE: dict = {}
LAST_RESULTS = None  # test harness introspection (exec_time_ns etc.)

# schedule-tuning knobs (sweepable from bench tooling)
CFG = {
    "W1": 12,       # warmup matmuls bridging the input-DMA head
    "B0_PRE": 8,    # block (0,0) pre-loop extra pops
    "B0_PER": 7,    # block (0,0) per-iter extra pops
    "BK_PRE": 2,    # later n=0 blocks pre-loop pops
    "BK_PER": 2,    # later n=0 blocks per-iter pops
    "B21_PER": 1,   # block (2,1) per-iter pops
    "B31_PER": 1,   # block (3,1) per-iter pops
}


def _build_nc():
    from contextlib import ExitStack

    import concourse.bass as bass
    import concourse.tile as tile
    from concourse import bacc, mybir

    f32 = mybir.dt.float32
    f16 = mybir.dt.float16
    f8 = mybir.dt.float8e4
    DR = mybir.MatmulPerfMode.DoubleRow
    Exp = mybir.ActivationFunctionType.Exp

    nc = bacc.Bacc(trn_type="TRN2")

    hsT_d = nc.declare_dram_parameter("hsT", [QD, QSH], f16, isOutput=False)
    encT_d = nc.declare_dram_parameter("encT", [CD, LKV], f16, isOutput=False)
    wq_d = nc.declare_dram_parameter("wq", [QD, INNER], f16, isOutput=False)
    wk_d = nc.declare_dram_parameter("wk", [CD, P], f16, isOutput=False)
    wv_d = nc.declare_dram_parameter("wv", [CD, INNER], f16, isOutput=False)
    wo_d = nc.declare_dram_parameter("wo", [INNER, QD], f16, isOutput=False)
    bo_d = nc.declare_dram_parameter("bo", [1, QD], f32, isOutput=False)
    wk8_d = nc.declare_dram_parameter("wk8", [CD, INNER], f8, isOutput=False)
    encT8_d = nc.declare_dram_parameter("encT8", [CD, 2, LKV], f8, isOutput=False)
    out_d = nc.declare_dram_parameter("out", [QSH, QD], f32, isOutput=True)

    KC_Q = QD // P   # 4 contraction chunks for q projection
    KC_KV = CD // P  # 6 for k/v projections
    AT = INNER // P  # 4 inner tiles (2 heads each)
    NT = LKV // P    # 8 kv chunks
    QN = QSH // 512  # 2 q slices of 512

    with ExitStack() as ctx:
        tc = ctx.enter_context(tile.TileContext(nc))
        const = ctx.enter_context(tc.tile_pool(name="const", bufs=1))
        acts = ctx.enter_context(tc.tile_pool(name="acts", bufs=1))
        expp = ctx.enter_context(tc.tile_pool(name="expp", bufs=4))
        outp = ctx.enter_context(tc.tile_pool(name="outp", bufs=4))
        small = ctx.enter_context(tc.tile_pool(name="small", bufs=6))
        psA = ctx.enter_context(tc.tile_pool(name="psA", bufs=4, space="PSUM"))
        psS = ctx.enter_context(tc.tile_pool(name="psS", bufs=2, space="PSUM"))
        drp = ctx.enter_context(tc.tile_pool(name="drp", bufs=4, space="DRAM"))

        # ---- input DMA, ordered by first use: the q projections (hsT+wq)
        # run during the PE warmup window, then kT (encT+wk), then v (wv);
        # the second encT half only gates scores t>=4 of the first block
        hsT_sb = acts.tile([P, KC_Q, QSH], f16)
        hsT_r = hsT_d.rearrange("(c p) n -> p c n", p=P)
        nc.sync.dma_start(hsT_sb[:, :, 0:512], hsT_r[:, :, 0:512])
        wq_sb = const.tile([P, KC_Q, INNER], f16)
        nc.sync.dma_start(wq_sb[:], wq_d.rearrange("(c p) n -> p c n", p=P))
        encT_sb = acts.tile([P, KC_KV, LKV], f16)
        encT_r = encT_d.rearrange("(c p) n -> p c n", p=P)
        nc.sync.dma_start(encT_sb[:, :, 0:512], encT_r[:, :, 0:512])
        wk_sb = const.tile([P, KC_KV, P], f16)
        nc.sync.dma_start(wk_sb[:], wk_d.rearrange("(c p) n -> p c n", p=P))
        nc.sync.dma_start(encT_sb[:, :, 512:1024], encT_r[:, :, 512:1024])
        wv_sb = const.tile([P, KC_KV, INNER], f16)
        nc.sync.dma_start(wv_sb[:], wv_d.rearrange("(c p) n -> p c n", p=P))
        nc.sync.dma_start(hsT_sb[:, :, 512:1024], hsT_r[:, :, 512:1024])
        # fp8 operands for the DoubleRow k projections (k heads 2..7): the
        # encoder comes as an fp8 (hi, lo) plane pair; wk8 rides both planes
        # of the stationary operand via a stride-0 broadcast
        encT8_sb = acts.tile([P, KC_KV, 2, LKV], f8)
        encT8_r = encT8_d.rearrange("(c p) j n -> p c j n", p=P)
        wk8_sb = const.tile([P, KC_KV, INNER], f8)
        nc.sync.dma_start(encT8_sb[:, :, 0, 0:512], encT8_r[:, :, 0, 0:512])
        nc.sync.dma_start(encT8_sb[:, :, 1, 0:512], encT8_r[:, :, 1, 0:512])
        nc.sync.dma_start(wk8_sb[:], wk8_d.rearrange("(c p) n -> p c n", p=P))
        nc.sync.dma_start(encT8_sb[:, :, 0, 512:1024], encT8_r[:, :, 0, 512:1024])
        nc.sync.dma_start(encT8_sb[:, :, 1, 512:1024], encT8_r[:, :, 1, 512:1024])
        wo_sb = const.tile([P, AT, QD], f16)
        nc.sync.dma_start(wo_sb[:], wo_d.rearrange("(c p) n -> p c n", p=P))
        bo_sb = const.tile([P, QD], f32)
        nc.sync.dma_start(bo_sb[:], bo_d.ap().to_broadcast((P, QD)))

        qT8 = acts.tile([P, AT, 2, QSH], f8)       # planes: (hi, lo)
        kT8 = acts.tile([P, AT, LKV], f8)          # single plane, broadcast in DR
        v_sb = acts.tile([P, NT, H * P], f16)
        outT_sb = acts.tile([P, AT, QSH], f16)
        vv4 = v_sb.rearrange("p t (s c) -> p t s c", c=P)

        # ---- PE warmup: dummy matmuls on zeroed scratch fill the DMA head
        # so the first real matmuls run at full clock (psD is never read)
        scratch = acts.tile([P, 512], f16)
        nc.gpsimd.memset(scratch[:], 0.0)

        # ones column for the PE-side partition broadcast in normalize
        ones_sb = const.tile([1, P], f16)
        nc.vector.memset(ones_sb[:], 1.0)

        def warmup(nmm):
            psD = psA.tile([P, 512], f32, tag="acc")
            for i in range(nmm):
                nc.tensor.matmul(
                    psD[:], scratch[:, 0:P], scratch[:],
                    start=(i == 0), stop=(i == nmm - 1),
                )

        # Generators yield once per emitted PE matmul so attention blocks can
        # interleave them into PE slack at a controlled rate (the per-engine
        # instruction streams execute strictly in program order).
        def gen_proj_k(a, nns=(0, 1)):
            # trailing copies are emitted BEFORE the final yield so that a
            # fully-popped generator has fully emitted its writes.
            # a=0 runs in fp16 off the early fp16 encoder DMA; a>=1 runs as
            # one-sided-fp8 DoubleRow (half PE cost; only the wk8 side
            # carries fp8 error, which softmax normalization absorbs).
            for nn in nns:
                ps = psA.tile([P, 512], f32, tag="acc")
                for c in range(KC_KV):
                    if a == 0:
                        nc.tensor.matmul(
                            ps[:],
                            wk_sb[:, c, a * P:(a + 1) * P],
                            encT_sb[:, c, nn * 512:(nn + 1) * 512],
                            start=(c == 0),
                            stop=(c == KC_KV - 1),
                        )
                    else:
                        nc.tensor.matmul(
                            ps[:],
                            wk8_sb[:, c, a * P:(a + 1) * P]
                            .unsqueeze(1).broadcast_to((P, 2, P)),
                            encT8_sb[:, c, :, nn * 512:(nn + 1) * 512],
                            start=(c == 0),
                            stop=(c == KC_KV - 1),
                            perf_mode=DR,
                        )
                    if c < KC_KV - 1:
                        yield
                with nc.allow_low_precision(reason="k stored fp8 for DR scores"):
                    nc.vector.tensor_copy(
                        kT8[:, a, nn * 512:(nn + 1) * 512], ps[:]
                    )
                yield

        def gen_proj_q(a, n):
            ps = psA.tile([P, 512], f32, tag="acc")
            for c in range(KC_Q):
                nc.tensor.matmul(
                    ps[:],
                    wq_sb[:, c, a * P:(a + 1) * P],
                    hsT_sb[:, c, n * 512:(n + 1) * 512],
                    start=(c == 0),
                    stop=(c == KC_Q - 1),
                )
                if c < KC_Q - 1:
                    yield
            sl = slice(n * 512, (n + 1) * 512)
            with nc.allow_low_precision(reason="q stored as fp8 hi/lo pair"):
                nc.vector.tensor_copy(qT8[:, a, 0, sl], ps[:])
                nc.vector.tensor_sub(qT8[:, a, 1, sl], ps[:], qT8[:, a, 0, sl])
            yield

        # v natural [kv, slots]: slot h (128 wide):
        #   h even: [v_h (0:64) | 1.0 at 64 | 0 at 65:128]   -> out rows 0:64, denom row 64
        #   h odd : [1.0 at 0 | 0 at 1:64 | v_h at 64:128]   -> out rows 64:128, denom row 0
        def v_memsets():
            nc.gpsimd.memset(vv4[:, :, 0::2, 64:65], 1.0)
            nc.gpsimd.memset(vv4[:, :, 1::2, 0:1], 1.0)
            nc.gpsimd.memset(vv4[:, :, 0::2, 65:P], 0.0)
            nc.gpsimd.memset(vv4[:, :, 1::2, 1:DH], 0.0)

        def gen_proj_v(t):
            ps = psA.tile([P, 512], f32, tag="acc")
            for c in range(KC_KV):
                nc.tensor.matmul(
                    ps[:],
                    encT_sb[:, c, t * P:(t + 1) * P],
                    wv_sb[:, c, :],
                    start=(c == 0),
                    stop=(c == KC_KV - 1),
                )
                if c < KC_KV - 1:
                    yield
            pv = ps.rearrange("p (s c) -> p s c", c=DH)
            nc.vector.tensor_copy(vv4[:, t, 0::2, 0:DH], pv[:, 0::2, :])
            nc.vector.tensor_copy(vv4[:, t, 1::2, DH:P], pv[:, 1::2, :])
            yield

        def gen_final(m):
            ps = psA.tile([P, 512], f32, tag="acc")
            for a in range(AT):
                nc.tensor.matmul(
                    ps[:],
                    outT_sb[:, a, m * P:(m + 1) * P],
                    wo_sb[:, a, :],
                    start=(a == 0),
                    stop=(a == AT - 1),
                )
                if a < AT - 1:
                    yield
            ob = outp.tile([P, QD], f32)
            nc.vector.tensor_add(ob[:], ps[:], bo_sb[:])
            nc.sync.dma_start(out_d[m * P:(m + 1) * P, :], ob[:])
            yield

        # final projection split for the tail m-tiles: partA (heads 0-1)
        # accumulates into an SBUF staging tile during earlier blocks; partB
        # (heads 2-3) only trails the last attention block
        facc = acts.tile([P, QSH // P, QD], f32)

        def gen_final_a(m):
            ps = psA.tile([P, 512], f32, tag="acc")
            for a in (0, 1):
                nc.tensor.matmul(
                    ps[:],
                    outT_sb[:, a, m * P:(m + 1) * P],
                    wo_sb[:, a, :],
                    start=(a == 0),
                    stop=(a == 1),
                )
                if a == 0:
                    yield
            nc.vector.tensor_add(facc[:, m, :], ps[:], bo_sb[:])
            yield

        def gen_final_b(m):
            ps = psA.tile([P, 512], f32, tag="acc")
            for a in (2, 3):
                nc.tensor.matmul(
                    ps[:],
                    outT_sb[:, a, m * P:(m + 1) * P],
                    wo_sb[:, a, :],
                    start=(a == 2),
                    stop=(a == 3),
                )
                if a == 2:
                    yield
            ob = outp.tile([P, QD], f32)
            nc.vector.tensor_add(ob[:], ps[:], facc[:, m, :])
            nc.sync.dma_start(out_d[m * P:(m + 1) * P, :], ob[:])
            yield

        def gen_chain(*gens):
            for g in gens:
                yield from g

        def run_gen(g):
            for _ in g:
                pass

        def attn(hp, n, extras=None, pre_pop=0, per_iter=0, prev_tail=None,
                 drain=True, act_copy_norm=False):
            """Emit one attention block.  Returns a closure that emits the
            block's last two AV matmuls + normalize; the caller passes it to
            the NEXT block so those trail instructions interleave with the
            next block's leading scores (removes the block-boundary bubble).
            """
            if extras is None:
                extras = iter(())

            def pop(k):
                for _ in range(k):
                    if next(extras, StopIteration) is StopIteration:
                        break

            av0 = psA.tile([P, 512], f32, tag="acc")
            av1 = psA.tile([P, 512], f32, tag="acc")
            av = (av0, av1)
            exs = []

            def s_(t):
                ss = psS.tile([P, 1024], f32)
                for i in range(2):
                    pr = slice(i * 64, (i + 1) * 64)
                    nc.tensor.matmul(
                        ss[:, i * 512:(i + 1) * 512],
                        kT8[pr, hp, t * P:(t + 1) * P]
                        .unsqueeze(1).broadcast_to((64, 2, P)),
                        qT8[pr, hp, :, n * 512:(n + 1) * 512],
                        start=True,
                        stop=True,
                        perf_mode=DR,
                    )
                ex = expp.tile([P, 1024], f16)
                nc.scalar.activation(
                    ex[:], ss[:], Exp, scale=SCALE / (WSCALE * WSCALE_K)
                )
                exs.append(ex)

            def A_(t):
                for i in range(2):
                    s = 2 * hp + i
                    nc.tensor.matmul(
                        av[i][:],
                        v_sb[:, t, s * P:(s + 1) * P],
                        exs[t][:, i * 512:(i + 1) * 512],
                        start=(t == 0),
                        stop=(t == NT - 1),
                    )

            s_(0)
            s_(1)
            pop(pre_pop)
            if prev_tail is not None:
                prev_tail()
            for t in range(2, NT):
                s_(t)
                A_(t - 2)
                pop(per_iter)
            if drain:  # drain leftovers so every generator completes
                for _ in extras:
                    pass

            def tail():
                A_(NT - 2)
                A_(NT - 1)
                # partition broadcast of 1/denom via a PE ones-column matmul
                # (GpSimd partition_broadcast proved flaky on HW; the DMA
                # round-trip costs ~4us per block).  The reciprocal lands on
                # partition 0 in f16, ones.T @ recip fills a PSUM tile,
                # which is copied to SBUF for the multiply (ScalarE for the
                # last block where it is idle, DVE elsewhere).
                for i in range(2):
                    drow = 64 if i == 0 else 0
                    dst = slice(0, 64) if i == 0 else slice(64, 128)
                    rc = small.tile([1, 512], f16, tag="rc")
                    with nc.allow_low_precision(
                        reason="softmax denom reciprocal, f16 suffices"
                    ):
                        nc.vector.reciprocal(
                            rc[0:1, :], av[i][drow:drow + 1, :]
                        )
                    rcps = psA.tile([P, 512], f32, tag="acc")
                    nc.tensor.matmul(
                        rcps[:], ones_sb[0:1, :], rc[0:1, :],
                        start=True, stop=True,
                    )
                    rcb = small.tile([P, 512], f32, tag="rcb")
                    if act_copy_norm:
                        nc.scalar.copy(rcb[:], rcps[:])
                    else:
                        nc.vector.tensor_copy(rcb[:], rcps[:])
                    nc.vector.tensor_mul(
                        outT_sb[dst, hp, n * 512:(n + 1) * 512],
                        av[i][dst, :],
                        rcb[dst, :],
                    )

            return tail

        # ---- emission = per-engine execution order.  Warmup dummies bridge
        # the DMA head up to qT(0,0); kT(0) kv-half 0 slots into the gap as
        # soon as its DMA lands; everything else (v, kT second half, later
        # k/q projections, finals) interleaves into attention-block PE slack.
        v_memsets()
        warmup(CFG["W1"])
        for a in range(AT):
            run_gen(gen_proj_q(a, 0))
        run_gen(gen_proj_k(0))
        run_gen(gen_proj_q(0, 1))
        tail = attn(
            0, 0,
            extras=gen_chain(
                *[gen_proj_v(t) for t in range(NT)],
                gen_proj_k(1),
            ),
            pre_pop=CFG["B0_PRE"], per_iter=CFG["B0_PER"],
        )
        tail = attn(1, 0, extras=gen_chain(gen_proj_k(2), gen_proj_q(1, 1)),
                    pre_pop=CFG["BK_PRE"], per_iter=CFG["BK_PER"],
                    prev_tail=tail)
        tail = attn(2, 0, extras=gen_chain(gen_proj_k(3), gen_proj_q(2, 1)),
                    pre_pop=CFG["BK_PRE"], per_iter=CFG["BK_PER"],
                    prev_tail=tail)
        tail = attn(3, 0, extras=gen_proj_q(3, 1), pre_pop=0, per_iter=1,
                    prev_tail=tail)
        f01 = gen_chain(gen_final(0), gen_final(1))
        tail = attn(0, 1, extras=f01, pre_pop=0, per_iter=1,
                    prev_tail=tail, drain=False)
        tail = attn(1, 1, extras=gen_chain(f01, gen_final(2)),
                    pre_pop=0, per_iter=1, prev_tail=tail)
        tail = attn(2, 1,
                    extras=gen_chain(gen_final(3), gen_final_a(4)),
                    pre_pop=0, per_iter=CFG["B21_PER"], prev_tail=tail)
        tail = attn(3, 1, act_copy_norm=True,
                    extras=gen_chain(gen_final_a(5), gen_final_a(6),
                                     gen_final_a(7)),
                    pre_pop=0, per_iter=CFG["B31_PER"], prev_tail=tail)
        tail()
        for m in range(AT, QSH // P):
            run_gen(gen_final_b(m))

    nc.finalize()
    return nc


def _get_nc():
    if "nc" not in _CACHE:
        _CACHE["nc"] = _build_nc()
    return _CACHE["nc"]


def make_in_maps(hidden_states, encoder_hidden_states, Wq, Wk, Wv, Wo, bo):
    f16 = np.float16
    hs = np.asarray(hidden_states, dtype=np.float32)
    enc = np.asarray(encoder_hidden_states, dtype=np.float32)
    import ml_dtypes

    f8 = ml_dtypes.float8_e4m3
    wq = np.ascontiguousarray(np.asarray(Wq, dtype=np.float32) * WSCALE).astype(f16)
    wk32 = np.asarray(Wk, dtype=np.float32) * WSCALE_K
    wk = np.ascontiguousarray(wk32[:, 0:P]).astype(f16)
    wk8 = np.ascontiguousarray(wk32).astype(f8)
    wv = np.ascontiguousarray(np.asarray(Wv, dtype=np.float32)).astype(f16)
    wo = np.ascontiguousarray(np.asarray(Wo, dtype=np.float32)).astype(f16)
    bo_ = np.ascontiguousarray(np.asarray(bo, dtype=np.float32)).reshape(1, QD)
    encT = [np.ascontiguousarray(enc[b].T).astype(f16) for b in range(B)]
    encT8 = []
    for b in range(B):
        e = enc[b].T.astype(np.float32)
        hi = e.astype(f8)
        lo = (e - hi.astype(np.float32)).astype(f8)
        encT8.append(np.ascontiguousarray(np.stack([hi, lo], axis=1)))
    in_maps = []
    for c in range(NCORES):
        b, s = divmod(c, 4)
        hsT = np.ascontiguousarray(hs[b, s * QSH:(s + 1) * QSH, :].T).astype(f16)
        in_maps.append(
            dict(hsT=hsT, encT=encT[b], wq=wq, wk=wk, wv=wv, wo=wo, bo=bo_,
                 wk8=wk8, encT8=encT8[b])
        )
    return in_maps


def kernel(hidden_states, encoder_hidden_states, Wq, Wk, Wv, Wo, bo):
    global LAST_RESULTS
    from concourse.bass_utils import run_bass_kernel_spmd

    nc = _get_nc()
    in_maps = make_in_maps(
        hidden_states, encoder_hidden_states, Wq, Wk, Wv, Wo, bo
    )
    res = run_bass_kernel_spmd(nc, in_maps, core_ids=list(range(NCORES)))
    LAST_RESULTS = res
    out = np.empty((B, LQ, QD), dtype=np.float32)
    for c in range(NCORES):
        b, s = divmod(c, 4)
        out[b, s * QSH:(s + 1) * QSH, :] = res.results[c]["out"]
    return out



# revision 49
# speedup vs baseline: 1.0442x; 1.0161x over previous
"""CrossAttention kernel for 8 TRN2 NeuronCores.

Reference computation (B=2, Lq=4096, Lkv=1024, query_dim=512, cross_dim=768,
heads=8, dim_head=64, inner=512):
    q = hs @ Wq; k = enc @ Wk; v = enc @ Wv          (per batch)
    attn = softmax(q_h @ k_h^T * scale) per head
    out = concat_h(attn @ v_h) @ Wo + bo

Sharding: 8 cores = 2 batches x 4 query-slices of 1024 queries.  Each core
computes its full slice of the output (all heads), so outputs are disjoint
and no collective is needed.

Per-core dataflow (fp16 operands, fp32 PSUM accumulate, with the scores
pipeline in fp8 DoubleRow — the cost model charges DR matmuls 0.5
cycles/row, so these run at half the PE cost):
  - host passes hs-slice and encoder transposed (hsT [512,1024], encT
    [768,1024]) in fp16, plus fp8 copies for the DoubleRow paths: an
    encoder (hi, lo) fp8 plane pair and wk8 = fp8(Wk*32).  Wq is
    pre-scaled x8 and Wk x32 to center fp8 dynamic range; exp() absorbs
    the 1/256.
  - qT = Wq^T-weighted hsT -> [inner=512, q=1024] (heads along
    partitions), stored as an fp8 (hi, lo) pair (DVE copy + subtract)
  - kT likewise -> [inner=512, kv=1024], stored as single-plane fp8.
    Head group a=0 projects in fp16 off the early encoder DMA; groups
    a=1..3 project via one-sided-fp8 DoubleRow: stationary wk8 rides both
    planes through a stride-0 broadcast AP, the moving operand is the
    encoder (hi, lo) pair, so only the wk8 side carries fp8 error.
  - v natural -> [kv=1024, slots] fp16; slot h = 128 cols holding v_h
    (64) + a ones column + zero padding, so the AV output lands
    partition-aligned and the softmax denominator falls out of the same
    matmul.
  - scoresT_h = k_h qT_h via ONE DoubleRow matmul per head: stationary
    (k8, k8) via stride-0 broadcast, moving (q_hi, q_lo), giving
    sum_j k8*(q_hi+q_lo) = k8 * q at fp8-pair precision.  Only the
    k-side carries fp8 error, which softmax normalization largely
    absorbs (q-side fp8 measures ~2x worse; both-sides fails the gate).
  - expT = exp(scale/256 * scoresT) on ScalarE, fp16 out (no
    max-subtraction: |scaled scores| < ~3)
  - outT_unnorm_h = v_slot^T @ expT accumulated over kv chunks (PSUM),
    one row of which is the softmax denominator
  - normalize: reciprocal (DVE) + PE ones-column broadcast matmul +
    multiply
  - final = outT^T @ Wo + bo -> [1024, 512], DMA out per 128-row tile
Measured absmax/scale ~1.05e-2 against the fp64 reference (gate 2e-2).

Program order is pipelined for the Tile scheduler: k/v/q projections are
emitted ahead of the attention blocks that consume them, exp(t) is emitted
before AV(t-1) so the PE never waits in-order on ScalarE, and the final
projection m-tiles are interleaved between the last attention blocks.
"""

import sys

if "/opt/trn_rl_repo" not in sys.path:
    sys.path.insert(0, "/opt/trn_rl_repo")

import numpy as np

B, LQ, LKV = 2, 4096, 1024
QD, CD = 512, 768
H, DH = 8, 64
INNER = H * DH  # 512
SCALE = DH ** -0.5
NCORES = 8
WSCALE = 8.0    # host-side pre-scale on Wq (fp8 range centering)
WSCALE_K = 32.0  # host-side pre-scale on Wk (fp8 range centering)
QSH = LQ // 4  # 1024 queries per core
P = 128

_CACHE: dict = {}
LAST_RESULTS = None  # test harness introspection (exec_time_ns etc.)

# schedule-tuning knobs (sweepable from bench tooling)
CFG = {
    "W1": 9,       # warmup matmuls bridging the input-DMA head
    "B0_PRE": 8,    # block (0,0) pre-loop extra pops
    "B0_PER": 9,    # block (0,0) per-iter extra pops
    "BK_PRE": 2,    # later n=0 blocks pre-loop pops
    "BK_PER": 2,    # later n=0 blocks per-iter pops
    "B21_PER": 1,   # block (2,1) per-iter pops
    "B31_PER": 1,   # block (3,1) per-iter pops
}


def _build_nc():
    from contextlib import ExitStack

    import concourse.bass as bass
    import concourse.tile as tile
    from concourse import bacc, mybir

    f32 = mybir.dt.float32
    f16 = mybir.dt.float16
    f8 = mybir.dt.float8e4
    DR = mybir.MatmulPerfMode.DoubleRow
    Exp = mybir.ActivationFunctionType.Exp

    nc = bacc.Bacc(trn_type="TRN2")

    hsT_d = nc.declare_dram_parameter("hsT", [QD, QSH], f16, isOutput=False)
    encT_d = nc.declare_dram_parameter("encT", [CD, LKV], f16, isOutput=False)
    wq_d = nc.declare_dram_parameter("wq", [QD, INNER], f16, isOutput=False)
    wk_d = nc.declare_dram_parameter("wk", [CD, P], f16, isOutput=False)
    wv_d = nc.declare_dram_parameter("wv", [CD, INNER], f16, isOutput=False)
    wo_d = nc.declare_dram_parameter("wo", [INNER, QD], f16, isOutput=False)
    bo_d = nc.declare_dram_parameter("bo", [1, QD], f32, isOutput=False)
    wk8_d = nc.declare_dram_parameter("wk8", [CD, INNER], f8, isOutput=False)
    encT8_d = nc.declare_dram_parameter("encT8", [CD, 2, LKV], f8, isOutput=False)
    out_d = nc.declare_dram_parameter("out", [QSH, QD], f32, isOutput=True)

    KC_Q = QD // P   # 4 contraction chunks for q projection
    KC_KV = CD // P  # 6 for k/v projections
    AT = INNER // P  # 4 inner tiles (2 heads each)
    NT = LKV // P    # 8 kv chunks
    QN = QSH // 512  # 2 q slices of 512

    with ExitStack() as ctx:
        tc = ctx.enter_context(tile.TileContext(nc))
        const = ctx.enter_context(tc.tile_pool(name="const", bufs=1))
        acts = ctx.enter_context(tc.tile_pool(name="acts", bufs=1))
        expp = ctx.enter_context(tc.tile_pool(name="expp", bufs=8))
        outp = ctx.enter_context(tc.tile_pool(name="outp", bufs=6))
        small = ctx.enter_context(tc.tile_pool(name="small", bufs=8))
        psA = ctx.enter_context(tc.tile_pool(name="psA", bufs=4, space="PSUM"))
        psS = ctx.enter_context(tc.tile_pool(name="psS", bufs=2, space="PSUM"))
        drp = ctx.enter_context(tc.tile_pool(name="drp", bufs=4, space="DRAM"))

        # ---- input DMA, ordered by first use: the q projections (hsT+wq)
        # run during the PE warmup window, then kT (encT+wk), then v (wv);
        # the second encT half only gates scores t>=4 of the first block
        hsT_sb = acts.tile([P, KC_Q, QSH], f16)
        hsT_r = hsT_d.rearrange("(c p) n -> p c n", p=P)
        nc.sync.dma_start(hsT_sb[:, :, 0:512], hsT_r[:, :, 0:512])
        wq_sb = const.tile([P, KC_Q, INNER], f16)
        nc.sync.dma_start(wq_sb[:], wq_d.rearrange("(c p) n -> p c n", p=P))
        encT_sb = acts.tile([P, KC_KV, LKV], f16)
        encT_r = encT_d.rearrange("(c p) n -> p c n", p=P)
        wk_sb = const.tile([P, KC_KV, P], f16)
        nc.sync.dma_start(wk_sb[:], wk_d.rearrange("(c p) n -> p c n", p=P))
        nc.sync.dma_start(encT_sb[:, :, 0:512], encT_r[:, :, 0:512])
        wv_sb = const.tile([P, KC_KV, INNER], f16)
        nc.sync.dma_start(wv_sb[:], wv_d.rearrange("(c p) n -> p c n", p=P))
        nc.sync.dma_start(encT_sb[:, :, 512:1024], encT_r[:, :, 512:1024])
        nc.sync.dma_start(hsT_sb[:, :, 512:1024], hsT_r[:, :, 512:1024])
        # fp8 operands for the DoubleRow k projections (k heads 2..7): the
        # encoder comes as an fp8 (hi, lo) plane pair; wk8 rides both planes
        # of the stationary operand via a stride-0 broadcast
        encT8_sb = acts.tile([P, KC_KV, 2, LKV], f8)
        encT8_r = encT8_d.rearrange("(c p) j n -> p c j n", p=P)
        wk8_sb = const.tile([P, KC_KV, INNER], f8)
        nc.sync.dma_start(encT8_sb[:, :, 0, 0:512], encT8_r[:, :, 0, 0:512])
        nc.sync.dma_start(encT8_sb[:, :, 1, 0:512], encT8_r[:, :, 1, 0:512])
        nc.sync.dma_start(wk8_sb[:], wk8_d.rearrange("(c p) n -> p c n", p=P))
        nc.sync.dma_start(encT8_sb[:, :, 0, 512:1024], encT8_r[:, :, 0, 512:1024])
        nc.sync.dma_start(encT8_sb[:, :, 1, 512:1024], encT8_r[:, :, 1, 512:1024])
        wo_sb = const.tile([P, AT, QD], f16)
        nc.sync.dma_start(wo_sb[:], wo_d.rearrange("(c p) n -> p c n", p=P))
        bo_sb = const.tile([P, QD], f32)
        nc.sync.dma_start(bo_sb[:], bo_d.ap().to_broadcast((P, QD)))

        qT8 = acts.tile([P, AT, 2, QSH], f8)       # planes: (hi, lo)
        kT8 = acts.tile([P, AT, LKV], f8)          # single plane, broadcast in DR
        v_sb = acts.tile([P, NT, H * P], f16)
        outT_sb = acts.tile([P, AT, QSH], f16)
        vv4 = v_sb.rearrange("p t (s c) -> p t s c", c=P)

        # ---- PE warmup: dummy matmuls on zeroed scratch fill the DMA head
        # so the first real matmuls run at full clock (psD is never read)
        scratch = acts.tile([P, 512], f16)
        nc.gpsimd.memset(scratch[:], 0.0)

        # ones column for the PE-side partition broadcast in normalize
        ones_sb = const.tile([1, P], f16)
        nc.vector.memset(ones_sb[:], 1.0)

        def warmup(nmm):
            psD = psA.tile([P, 512], f32, tag="acc")
            for i in range(nmm):
                nc.tensor.matmul(
                    psD[:], scratch[:, 0:P], scratch[:],
                    start=(i == 0), stop=(i == nmm - 1),
                )

        # Generators yield once per emitted PE matmul so attention blocks can
        # interleave them into PE slack at a controlled rate (the per-engine
        # instruction streams execute strictly in program order).
        def gen_proj_k(a, nns=(0, 1)):
            # trailing copies are emitted BEFORE the final yield so that a
            # fully-popped generator has fully emitted its writes.
            # a=0 runs in fp16 off the early fp16 encoder DMA; a>=1 runs as
            # one-sided-fp8 DoubleRow (half PE cost; only the wk8 side
            # carries fp8 error, which softmax normalization absorbs).
            for nn in nns:
                ps = psA.tile([P, 512], f32, tag="acc")
                for c in range(KC_KV):
                    if a == 0:
                        nc.tensor.matmul(
                            ps[:],
                            wk_sb[:, c, a * P:(a + 1) * P],
                            encT_sb[:, c, nn * 512:(nn + 1) * 512],
                            start=(c == 0),
                            stop=(c == KC_KV - 1),
                        )
                    else:
                        nc.tensor.matmul(
                            ps[:],
                            wk8_sb[:, c, a * P:(a + 1) * P]
                            .unsqueeze(1).broadcast_to((P, 2, P)),
                            encT8_sb[:, c, :, nn * 512:(nn + 1) * 512],
                            start=(c == 0),
                            stop=(c == KC_KV - 1),
                            perf_mode=DR,
                        )
                    if c < KC_KV - 1:
                        yield
                with nc.allow_low_precision(reason="k stored fp8 for DR scores"):
                    nc.vector.tensor_copy(
                        kT8[:, a, nn * 512:(nn + 1) * 512], ps[:]
                    )
                yield

        def gen_proj_q(a, n):
            ps = psA.tile([P, 512], f32, tag="acc")
            for c in range(KC_Q):
                nc.tensor.matmul(
                    ps[:],
                    wq_sb[:, c, a * P:(a + 1) * P],
                    hsT_sb[:, c, n * 512:(n + 1) * 512],
                    start=(c == 0),
                    stop=(c == KC_Q - 1),
                )
                if c < KC_Q - 1:
                    yield
            sl = slice(n * 512, (n + 1) * 512)
            with nc.allow_low_precision(reason="q stored as fp8 hi/lo pair"):
                nc.vector.tensor_copy(qT8[:, a, 0, sl], ps[:])
                nc.vector.tensor_sub(qT8[:, a, 1, sl], ps[:], qT8[:, a, 0, sl])
            yield

        # v natural [kv, slots]: slot h (128 wide):
        #   h even: [v_h (0:64) | 1.0 at 64 | 0 at 65:128]   -> out rows 0:64, denom row 64
        #   h odd : [1.0 at 0 | 0 at 1:64 | v_h at 64:128]   -> out rows 64:128, denom row 0
        def v_memsets():
            nc.gpsimd.memset(vv4[:, :, 0::2, 64:65], 1.0)
            nc.gpsimd.memset(vv4[:, :, 1::2, 0:1], 1.0)
            nc.gpsimd.memset(vv4[:, :, 0::2, 65:P], 0.0)
            nc.gpsimd.memset(vv4[:, :, 1::2, 1:DH], 0.0)

        def gen_proj_v(t):
            ps = psA.tile([P, 512], f32, tag="acc")
            for c in range(KC_KV):
                nc.tensor.matmul(
                    ps[:],
                    encT_sb[:, c, t * P:(t + 1) * P],
                    wv_sb[:, c, :],
                    start=(c == 0),
                    stop=(c == KC_KV - 1),
                )
                if c < KC_KV - 1:
                    yield
            pv = ps.rearrange("p (s c) -> p s c", c=DH)
            nc.vector.tensor_copy(vv4[:, t, 0::2, 0:DH], pv[:, 0::2, :])
            nc.vector.tensor_copy(vv4[:, t, 1::2, DH:P], pv[:, 1::2, :])
            yield

        def gen_final(m):
            ps = psA.tile([P, 512], f32, tag="acc")
            for a in range(AT):
                nc.tensor.matmul(
                    ps[:],
                    outT_sb[:, a, m * P:(m + 1) * P],
                    wo_sb[:, a, :],
                    start=(a == 0),
                    stop=(a == AT - 1),
                )
                if a < AT - 1:
                    yield
            ob = outp.tile([P, QD], f32)
            nc.vector.tensor_add(ob[:], ps[:], bo_sb[:])
            nc.sync.dma_start(out_d[m * P:(m + 1) * P, :], ob[:])
            yield

        # final projection split for the tail m-tiles: partA (heads 0-1)
        # accumulates into an SBUF staging tile during earlier blocks; partB
        # (heads 2-3) only trails the last attention block
        facc = acts.tile([P, QSH // P, QD], f32)

        def gen_final_a(m):
            ps = psA.tile([P, 512], f32, tag="acc")
            for a in (0, 1):
                nc.tensor.matmul(
                    ps[:],
                    outT_sb[:, a, m * P:(m + 1) * P],
                    wo_sb[:, a, :],
                    start=(a == 0),
                    stop=(a == 1),
                )
                if a == 0:
                    yield
            nc.vector.tensor_add(facc[:, m, :], ps[:], bo_sb[:])
            yield

        def gen_final_b(m):
            ps = psA.tile([P, 512], f32, tag="acc")
            for a in (2, 3):
                nc.tensor.matmul(
                    ps[:],
                    outT_sb[:, a, m * P:(m + 1) * P],
                    wo_sb[:, a, :],
                    start=(a == 2),
                    stop=(a == 3),
                )
                if a == 2:
                    yield
            ob = outp.tile([P, QD], f32)
            nc.vector.tensor_add(ob[:], ps[:], facc[:, m, :])
            nc.sync.dma_start(out_d[m * P:(m + 1) * P, :], ob[:])
            yield

        # split final-b for the first two tail m-tiles: the a=2 matmul only
        # needs head group 2 (written by block (2,1)'s tail), so it runs as a
        # block-(3,1) extra; only the a=3 matmul + add + DMA trail the last
        # normalize.  At most TWO may be pre-opened: their live PSUM tiles
        # plus the two tail rcps tiles exactly fill the 4-slot acc rotation.
        fb_ps = {}

        def gen_fb_a2(m):
            ps = psA.tile([P, 512], f32, tag="acc", name="fbps")
            nc.tensor.matmul(
                ps[:], outT_sb[:, 2, m * P:(m + 1) * P], wo_sb[:, 2, :],
                start=True, stop=False,
            )
            fb_ps[m] = ps
            yield

        def fb_a3(m):
            ps = fb_ps[m]
            nc.tensor.matmul(
                ps[:], outT_sb[:, 3, m * P:(m + 1) * P], wo_sb[:, 3, :],
                start=False, stop=True,
            )
            ob = outp.tile([P, QD], f32)
            nc.vector.tensor_add(ob[:], ps[:], facc[:, m, :])
            nc.sync.dma_start(out_d[m * P:(m + 1) * P, :], ob[:])

        def gen_chain(*gens):
            for g in gens:
                yield from g

        def run_gen(g):
            for _ in g:
                pass

        def attn(hp, n, extras=None, pre_pop=0, per_iter=0, prev_tail=None,
                 drain=True, act_copy_norm=False, split_tail_exp=False):
            """Emit one attention block.  Returns a closure that emits the
            block's last two AV matmuls + normalize; the caller passes it to
            the NEXT block so those trail instructions interleave with the
            next block's leading scores (removes the block-boundary bubble).
            """
            if extras is None:
                extras = iter(())

            def pop(k):
                for _ in range(k):
                    if next(extras, StopIteration) is StopIteration:
                        break

            av0 = psA.tile([P, 512], f32, tag="acc")
            av1 = psA.tile([P, 512], f32, tag="acc")
            av = (av0, av1)
            exs = []

            def s_(t):
                ss = psS.tile([P, 1024], f32)
                for i in range(2):
                    pr = slice(i * 64, (i + 1) * 64)
                    nc.tensor.matmul(
                        ss[:, i * 512:(i + 1) * 512],
                        kT8[pr, hp, t * P:(t + 1) * P]
                        .unsqueeze(1).broadcast_to((64, 2, P)),
                        qT8[pr, hp, :, n * 512:(n + 1) * 512],
                        start=True,
                        stop=True,
                        perf_mode=DR,
                    )
                if split_tail_exp and t == NT - 1:
                    # last exp of the LAST block split per head: the final AV
                    # for head-even (and so the whole normalize/final-b tail
                    # chain) starts one half-exp earlier.
                    exa = expp.tile([P, 512], f16, tag="exh", name="exa")
                    exb = expp.tile([P, 512], f16, tag="exh", name="exb")
                    nc.scalar.activation(
                        exa[:], ss[:, 0:512], Exp,
                        scale=SCALE / (WSCALE * WSCALE_K),
                    )
                    nc.scalar.activation(
                        exb[:], ss[:, 512:1024], Exp,
                        scale=SCALE / (WSCALE * WSCALE_K),
                    )
                    exs.append((exa, exb))
                    return
                ex = expp.tile([P, 1024], f16)
                nc.scalar.activation(
                    ex[:], ss[:], Exp, scale=SCALE / (WSCALE * WSCALE_K)
                )
                exs.append(ex)

            def A_(t):
                for i in range(2):
                    s = 2 * hp + i
                    e = exs[t]
                    rhs = e[i][:, :] if isinstance(e, tuple) \
                        else e[:, i * 512:(i + 1) * 512]
                    nc.tensor.matmul(
                        av[i][:],
                        v_sb[:, t, s * P:(s + 1) * P],
                        rhs,
                        start=(t == 0),
                        stop=(t == NT - 1),
                    )

            s_(0)
            s_(1)
            pop(pre_pop)
            if prev_tail is not None:
                prev_tail()
            for t in range(2, NT):
                s_(t)
                A_(t - 2)
                pop(per_iter)
            if drain:  # drain leftovers so every generator completes
                for _ in extras:
                    pass

            def tail():
                A_(NT - 2)
                A_(NT - 1)
                # partition broadcast of 1/denom via a PE ones-column matmul
                # (GpSimd partition_broadcast proved flaky on HW; the DMA
                # round-trip costs ~4us per block).  The reciprocal lands on
                # partition 0 in f16, ones.T @ recip fills a PSUM tile,
                # which is copied to SBUF for the multiply (ScalarE for the
                # last block where it is idle, DVE elsewhere).
                for i in range(2):
                    drow = 64 if i == 0 else 0
                    dst = slice(0, 64) if i == 0 else slice(64, 128)
                    rc = small.tile([1, 512], f16, tag="rc")
                    with nc.allow_low_precision(
                        reason="softmax denom reciprocal, f16 suffices"
                    ):
                        nc.vector.reciprocal(
                            rc[0:1, :], av[i][drow:drow + 1, :]
                        )
                    rcps = psA.tile([P, 512], f32, tag="acc")
                    nc.tensor.matmul(
                        rcps[:], ones_sb[0:1, :], rc[0:1, :],
                        start=True, stop=True,
                    )
                    rcb = small.tile([P, 512], f32, tag="rcb")
                    if act_copy_norm:
                        nc.scalar.copy(rcb[:], rcps[:])
                    else:
                        nc.vector.tensor_copy(rcb[:], rcps[:])
                    nc.vector.tensor_mul(
                        outT_sb[dst, hp, n * 512:(n + 1) * 512],
                        av[i][dst, :],
                        rcb[dst, :],
                    )

            return tail

        # ---- emission = per-engine execution order.  Warmup dummies bridge
        # the DMA head up to qT(0,0); kT(0) kv-half 0 slots into the gap as
        # soon as its DMA lands; everything else (v, kT second half, later
        # k/q projections, finals) interleaves into attention-block PE slack.
        v_memsets()
        warmup(CFG["W1"])
        for a in range(AT):
            run_gen(gen_proj_q(a, 0))
        run_gen(gen_proj_k(0, nns=(0,)))
        tail = attn(
            0, 0,
            extras=gen_chain(
                gen_proj_v(0), gen_proj_v(1),
                gen_proj_k(0, nns=(1,)),
                *[gen_proj_v(t) for t in range(2, NT)],
                gen_proj_k(1),
                gen_proj_q(0, 1),
            ),
            pre_pop=CFG["B0_PRE"], per_iter=CFG["B0_PER"],
        )
        tail = attn(1, 0, extras=gen_chain(gen_proj_k(2), gen_proj_q(1, 1)),
                    pre_pop=CFG["BK_PRE"], per_iter=CFG["BK_PER"],
                    prev_tail=tail)
        tail = attn(2, 0, extras=gen_chain(gen_proj_k(3), gen_proj_q(2, 1)),
                    pre_pop=CFG["BK_PRE"], per_iter=CFG["BK_PER"],
                    prev_tail=tail)
        tail = attn(3, 0, extras=gen_proj_q(3, 1), pre_pop=0, per_iter=1,
                    prev_tail=tail)
        f01 = gen_chain(gen_final(0), gen_final(1))
        tail = attn(0, 1, extras=f01, pre_pop=0, per_iter=1,
                    prev_tail=tail, drain=False)
        tail = attn(1, 1, extras=gen_chain(f01, gen_final(2)),
                    pre_pop=0, per_iter=1, prev_tail=tail)
        tail = attn(2, 1,
                    extras=gen_chain(gen_final(3), gen_final_a(4)),
                    pre_pop=0, per_iter=CFG["B21_PER"], prev_tail=tail)
        tail = attn(3, 1, act_copy_norm=True, split_tail_exp=True,
                    extras=gen_chain(gen_final_a(5), gen_final_a(6),
                                     gen_final_a(7)),
                    pre_pop=0, per_iter=CFG["B31_PER"], prev_tail=tail)
        tail()
        for m in range(AT, QSH // P):
            run_gen(gen_final_b(m))

    nc.finalize()
    return nc


def _get_nc():
    if "nc" not in _CACHE:
        _CACHE["nc"] = _build_nc()
    return _CACHE["nc"]


def make_in_maps(hidden_states, encoder_hidden_states, Wq, Wk, Wv, Wo, bo):
    f16 = np.float16
    hs = np.asarray(hidden_states, dtype=np.float32)
    enc = np.asarray(encoder_hidden_states, dtype=np.float32)
    import ml_dtypes

    f8 = ml_dtypes.float8_e4m3
    wq = np.ascontiguousarray(np.asarray(Wq, dtype=np.float32) * WSCALE).astype(f16)
    wk32 = np.asarray(Wk, dtype=np.float32) * WSCALE_K
    wk = np.ascontiguousarray(wk32[:, 0:P]).astype(f16)
    wk8 = np.ascontiguousarray(wk32).astype(f8)
    wv = np.ascontiguousarray(np.asarray(Wv, dtype=np.float32)).astype(f16)
    wo = np.ascontiguousarray(np.asarray(Wo, dtype=np.float32)).astype(f16)
    bo_ = np.ascontiguousarray(np.asarray(bo, dtype=np.float32)).reshape(1, QD)
    encT = [np.ascontiguousarray(enc[b].T).astype(f16) for b in range(B)]
    encT8 = []
    for b in range(B):
        e = enc[b].T.astype(np.float32)
        hi = e.astype(f8)
        lo = (e - hi.astype(np.float32)).astype(f8)
        encT8.append(np.ascontiguousarray(np.stack([hi, lo], axis=1)))
    in_maps = []
    for c in range(NCORES):
        b, s = divmod(c, 4)
        hsT = np.ascontiguousarray(hs[b, s * QSH:(s + 1) * QSH, :].T).astype(f16)
        in_maps.append(
            dict(hsT=hsT, encT=encT[b], wq=wq, wk=wk, wv=wv, wo=wo, bo=bo_,
                 wk8=wk8, encT8=encT8[b])
        )
    return in_maps


def kernel(hidden_states, encoder_hidden_states, Wq, Wk, Wv, Wo, bo):
    global LAST_RESULTS
    from concourse.bass_utils import run_bass_kernel_spmd

    nc = _get_nc()
    in_maps = make_in_maps(
        hidden_states, encoder_hidden_states, Wq, Wk, Wv, Wo, bo
    )
    res = run_bass_kernel_spmd(nc, in_maps, core_ids=list(range(NCORES)))
    LAST_RESULTS = res
    out = np.empty((B, LQ, QD), dtype=np.float32)
    for c in range(NCORES):
        b, s = divmod(c, 4)
        out[b, s * QSH:(s + 1) * QSH, :] = res.results[c]["out"]
    return out



# revision 51
# speedup vs baseline: 1.0474x; 1.0031x over previous
"""CrossAttention kernel for 8 TRN2 NeuronCores.

Reference computation (B=2, Lq=4096, Lkv=1024, query_dim=512, cross_dim=768,
heads=8, dim_head=64, inner=512):
    q = hs @ Wq; k = enc @ Wk; v = enc @ Wv          (per batch)
    attn = softmax(q_h @ k_h^T * scale) per head
    out = concat_h(attn @ v_h) @ Wo + bo

Sharding: 8 cores = 2 batches x 4 query-slices of 1024 queries.  Each core
computes its full slice of the output (all heads), so outputs are disjoint
and no collective is needed.

Per-core dataflow (fp16 operands, fp32 PSUM accumulate, with the scores
pipeline in fp8 DoubleRow — the cost model charges DR matmuls 0.5
cycles/row, so these run at half the PE cost):
  - host passes hs-slice and encoder transposed (hsT [512,1024], encT
    [768,1024]) in fp16, plus fp8 copies for the DoubleRow paths: an
    encoder (hi, lo) fp8 plane pair and wk8 = fp8(Wk*32).  Wq is
    pre-scaled x8 and Wk x32 to center fp8 dynamic range; exp() absorbs
    the 1/256.
  - qT = Wq^T-weighted hsT -> [inner=512, q=1024] (heads along
    partitions), stored as an fp8 (hi, lo) pair (DVE copy + subtract)
  - kT likewise -> [inner=512, kv=1024], stored as single-plane fp8.
    Head group a=0 projects in fp16 off the early encoder DMA; groups
    a=1..3 project via one-sided-fp8 DoubleRow: stationary wk8 rides both
    planes through a stride-0 broadcast AP, the moving operand is the
    encoder (hi, lo) pair, so only the wk8 side carries fp8 error.
  - v natural -> [kv=1024, slots] fp16; slot h = 128 cols holding v_h
    (64) + a ones column + zero padding, so the AV output lands
    partition-aligned and the softmax denominator falls out of the same
    matmul.
  - scoresT_h = k_h qT_h via ONE DoubleRow matmul per head: stationary
    (k8, k8) via stride-0 broadcast, moving (q_hi, q_lo), giving
    sum_j k8*(q_hi+q_lo) = k8 * q at fp8-pair precision.  Only the
    k-side carries fp8 error, which softmax normalization largely
    absorbs (q-side fp8 measures ~2x worse; both-sides fails the gate).
  - expT = exp(scale/256 * scoresT) on ScalarE, fp16 out (no
    max-subtraction: |scaled scores| < ~3)
  - outT_unnorm_h = v_slot^T @ expT accumulated over kv chunks (PSUM),
    one row of which is the softmax denominator
  - normalize: reciprocal (DVE) + PE ones-column broadcast matmul +
    multiply
  - final = outT^T @ Wo + bo -> [1024, 512], DMA out per 128-row tile
Measured absmax/scale ~1.05e-2 against the fp64 reference (gate 2e-2).

Program order is pipelined for the Tile scheduler: k/v/q projections are
emitted ahead of the attention blocks that consume them, exp(t) is emitted
before AV(t-1) so the PE never waits in-order on ScalarE, and the final
projection m-tiles are interleaved between the last attention blocks.
"""

import sys

if "/opt/trn_rl_repo" not in sys.path:
    sys.path.insert(0, "/opt/trn_rl_repo")

import numpy as np

B, LQ, LKV = 2, 4096, 1024
QD, CD = 512, 768
H, DH = 8, 64
INNER = H * DH  # 512
SCALE = DH ** -0.5
NCORES = 8
WSCALE = 8.0    # host-side pre-scale on Wq (fp8 range centering)
WSCALE_K = 32.0  # host-side pre-scale on Wk (fp8 range centering)
QSH = LQ // 4  # 1024 queries per core
P = 128

_CACHE: dict = {}
LAST_RESULTS = None  # test harness introspection (exec_time_ns etc.)

# schedule-tuning knobs (sweepable from bench tooling)
CFG = {
    "W1": 9,       # warmup matmuls bridging the input-DMA head
    "B0_PRE": 8,    # block (0,0) pre-loop extra pops
    "B0_PER": 9,    # block (0,0) per-iter extra pops
    "BK_PRE": 2,    # later n=0 blocks pre-loop pops
    "BK_PER": 3,    # later n=0 blocks per-iter pops
    "B21_PER": 1,   # block (2,1) per-iter pops
    "B31_PER": 1,   # block (3,1) per-iter pops
}


def _build_nc():
    from contextlib import ExitStack

    import concourse.bass as bass
    import concourse.tile as tile
    from concourse import bacc, mybir

    f32 = mybir.dt.float32
    f16 = mybir.dt.float16
    f8 = mybir.dt.float8e4
    DR = mybir.MatmulPerfMode.DoubleRow
    Exp = mybir.ActivationFunctionType.Exp

    nc = bacc.Bacc(trn_type="TRN2")

    hsT_d = nc.declare_dram_parameter("hsT", [QD, QSH], f16, isOutput=False)
    encT_d = nc.declare_dram_parameter("encT", [CD, LKV], f16, isOutput=False)
    wq_d = nc.declare_dram_parameter("wq", [QD, INNER], f16, isOutput=False)
    wk_d = nc.declare_dram_parameter("wk", [CD, P], f16, isOutput=False)
    wv_d = nc.declare_dram_parameter("wv", [CD, INNER], f16, isOutput=False)
    wo_d = nc.declare_dram_parameter("wo", [INNER, QD], f16, isOutput=False)
    bo_d = nc.declare_dram_parameter("bo", [1, QD], f32, isOutput=False)
    wk8_d = nc.declare_dram_parameter("wk8", [CD, INNER], f8, isOutput=False)
    encT8_d = nc.declare_dram_parameter("encT8", [CD, 2, LKV], f8, isOutput=False)
    out_d = nc.declare_dram_parameter("out", [QSH, QD], f32, isOutput=True)

    KC_Q = QD // P   # 4 contraction chunks for q projection
    KC_KV = CD // P  # 6 for k/v projections
    AT = INNER // P  # 4 inner tiles (2 heads each)
    NT = LKV // P    # 8 kv chunks
    QN = QSH // 512  # 2 q slices of 512

    with ExitStack() as ctx:
        tc = ctx.enter_context(tile.TileContext(nc))
        const = ctx.enter_context(tc.tile_pool(name="const", bufs=1))
        acts = ctx.enter_context(tc.tile_pool(name="acts", bufs=1))
        expp = ctx.enter_context(tc.tile_pool(name="expp", bufs=8))
        outp = ctx.enter_context(tc.tile_pool(name="outp", bufs=6))
        small = ctx.enter_context(tc.tile_pool(name="small", bufs=8))
        psA = ctx.enter_context(tc.tile_pool(name="psA", bufs=4, space="PSUM"))
        psS = ctx.enter_context(tc.tile_pool(name="psS", bufs=2, space="PSUM"))
        drp = ctx.enter_context(tc.tile_pool(name="drp", bufs=4, space="DRAM"))

        # ---- input DMA, ordered by first use: the q projections (hsT+wq)
        # run during the PE warmup window, then kT (encT+wk), then v (wv);
        # the second encT half only gates scores t>=4 of the first block
        hsT_sb = acts.tile([P, KC_Q, QSH], f16)
        hsT_r = hsT_d.rearrange("(c p) n -> p c n", p=P)
        nc.sync.dma_start(hsT_sb[:, :, 0:512], hsT_r[:, :, 0:512])
        wq_sb = const.tile([P, KC_Q, INNER], f16)
        nc.sync.dma_start(wq_sb[:], wq_d.rearrange("(c p) n -> p c n", p=P))
        encT_sb = acts.tile([P, KC_KV, LKV], f16)
        encT_r = encT_d.rearrange("(c p) n -> p c n", p=P)
        wk_sb = const.tile([P, KC_KV, P], f16)
        nc.sync.dma_start(wk_sb[:], wk_d.rearrange("(c p) n -> p c n", p=P))
        nc.sync.dma_start(encT_sb[:, :, 0:512], encT_r[:, :, 0:512])
        wv_sb = const.tile([P, KC_KV, INNER], f16)
        nc.sync.dma_start(wv_sb[:], wv_d.rearrange("(c p) n -> p c n", p=P))
        nc.sync.dma_start(encT_sb[:, :, 512:1024], encT_r[:, :, 512:1024])
        nc.sync.dma_start(hsT_sb[:, :, 512:1024], hsT_r[:, :, 512:1024])
        # fp8 operands for the DoubleRow k projections (k heads 2..7): the
        # encoder comes as an fp8 (hi, lo) plane pair; wk8 rides both planes
        # of the stationary operand via a stride-0 broadcast
        encT8_sb = acts.tile([P, KC_KV, 2, LKV], f8)
        encT8_r = encT8_d.rearrange("(c p) j n -> p c j n", p=P)
        wk8_sb = const.tile([P, KC_KV, INNER], f8)
        nc.sync.dma_start(encT8_sb[:, :, 0, 0:512], encT8_r[:, :, 0, 0:512])
        nc.sync.dma_start(encT8_sb[:, :, 1, 0:512], encT8_r[:, :, 1, 0:512])
        nc.sync.dma_start(wk8_sb[:], wk8_d.rearrange("(c p) n -> p c n", p=P))
        nc.sync.dma_start(encT8_sb[:, :, 0, 512:1024], encT8_r[:, :, 0, 512:1024])
        nc.sync.dma_start(encT8_sb[:, :, 1, 512:1024], encT8_r[:, :, 1, 512:1024])
        wo_sb = const.tile([P, AT, QD], f16)
        nc.sync.dma_start(wo_sb[:], wo_d.rearrange("(c p) n -> p c n", p=P))
        bo_sb = const.tile([P, QD], f32)
        nc.sync.dma_start(bo_sb[:], bo_d.ap().to_broadcast((P, QD)))

        qT8 = acts.tile([P, AT, 2, QSH], f8)       # planes: (hi, lo)
        kT8 = acts.tile([P, AT, LKV], f8)          # single plane, broadcast in DR
        v_sb = acts.tile([P, NT, H * P], f16)
        outT_sb = acts.tile([P, AT, QSH], f16)
        vv4 = v_sb.rearrange("p t (s c) -> p t s c", c=P)

        # ---- PE warmup: dummy matmuls on zeroed scratch fill the DMA head
        # so the first real matmuls run at full clock (psD is never read)
        scratch = acts.tile([P, 512], f16)
        nc.gpsimd.memset(scratch[:], 0.0)

        # ones column for the PE-side partition broadcast in normalize
        ones_sb = const.tile([1, P], f16)
        nc.vector.memset(ones_sb[:], 1.0)

        def warmup(nmm):
            psD = psA.tile([P, 512], f32, tag="acc")
            for i in range(nmm):
                nc.tensor.matmul(
                    psD[:], scratch[:, 0:P], scratch[:],
                    start=(i == 0), stop=(i == nmm - 1),
                )

        # Generators yield once per emitted PE matmul so attention blocks can
        # interleave them into PE slack at a controlled rate (the per-engine
        # instruction streams execute strictly in program order).
        def gen_proj_k(a, nns=(0, 1)):
            # trailing copies are emitted BEFORE the final yield so that a
            # fully-popped generator has fully emitted its writes.
            # a=0 runs in fp16 off the early fp16 encoder DMA; a>=1 runs as
            # one-sided-fp8 DoubleRow (half PE cost; only the wk8 side
            # carries fp8 error, which softmax normalization absorbs).
            for nn in nns:
                ps = psA.tile([P, 512], f32, tag="acc")
                for c in range(KC_KV):
                    if a == 0:
                        nc.tensor.matmul(
                            ps[:],
                            wk_sb[:, c, a * P:(a + 1) * P],
                            encT_sb[:, c, nn * 512:(nn + 1) * 512],
                            start=(c == 0),
                            stop=(c == KC_KV - 1),
                        )
                    else:
                        nc.tensor.matmul(
                            ps[:],
                            wk8_sb[:, c, a * P:(a + 1) * P]
                            .unsqueeze(1).broadcast_to((P, 2, P)),
                            encT8_sb[:, c, :, nn * 512:(nn + 1) * 512],
                            start=(c == 0),
                            stop=(c == KC_KV - 1),
                            perf_mode=DR,
                        )
                    if c < KC_KV - 1:
                        yield
                with nc.allow_low_precision(reason="k stored fp8 for DR scores"):
                    nc.vector.tensor_copy(
                        kT8[:, a, nn * 512:(nn + 1) * 512], ps[:]
                    )
                yield

        def gen_proj_q(a, n):
            ps = psA.tile([P, 512], f32, tag="acc")
            for c in range(KC_Q):
                nc.tensor.matmul(
                    ps[:],
                    wq_sb[:, c, a * P:(a + 1) * P],
                    hsT_sb[:, c, n * 512:(n + 1) * 512],
                    start=(c == 0),
                    stop=(c == KC_Q - 1),
                )
                if c < KC_Q - 1:
                    yield
            sl = slice(n * 512, (n + 1) * 512)
            with nc.allow_low_precision(reason="q stored as fp8 hi/lo pair"):
                nc.vector.tensor_copy(qT8[:, a, 0, sl], ps[:])
                nc.vector.tensor_sub(qT8[:, a, 1, sl], ps[:], qT8[:, a, 0, sl])
            yield

        # v natural [kv, slots]: slot h (128 wide):
        #   h even: [v_h (0:64) | 1.0 at 64 | 0 at 65:128]   -> out rows 0:64, denom row 64
        #   h odd : [1.0 at 0 | 0 at 1:64 | v_h at 64:128]   -> out rows 64:128, denom row 0
        def v_memsets():
            nc.gpsimd.memset(vv4[:, :, 0::2, 64:65], 1.0)
            nc.gpsimd.memset(vv4[:, :, 1::2, 0:1], 1.0)
            nc.gpsimd.memset(vv4[:, :, 0::2, 65:P], 0.0)
            nc.gpsimd.memset(vv4[:, :, 1::2, 1:DH], 0.0)

        def gen_proj_v(t):
            ps = psA.tile([P, 512], f32, tag="acc")
            for c in range(KC_KV):
                nc.tensor.matmul(
                    ps[:],
                    encT_sb[:, c, t * P:(t + 1) * P],
                    wv_sb[:, c, :],
                    start=(c == 0),
                    stop=(c == KC_KV - 1),
                )
                if c < KC_KV - 1:
                    yield
            pv = ps.rearrange("p (s c) -> p s c", c=DH)
            nc.vector.tensor_copy(vv4[:, t, 0::2, 0:DH], pv[:, 0::2, :])
            nc.vector.tensor_copy(vv4[:, t, 1::2, DH:P], pv[:, 1::2, :])
            yield

        def gen_final(m):
            ps = psA.tile([P, 512], f32, tag="acc")
            for a in range(AT):
                nc.tensor.matmul(
                    ps[:],
                    outT_sb[:, a, m * P:(m + 1) * P],
                    wo_sb[:, a, :],
                    start=(a == 0),
                    stop=(a == AT - 1),
                )
                if a < AT - 1:
                    yield
            ob = outp.tile([P, QD], f32)
            nc.vector.tensor_add(ob[:], ps[:], bo_sb[:])
            nc.sync.dma_start(out_d[m * P:(m + 1) * P, :], ob[:])
            yield

        # final projection split for the tail m-tiles: partA (heads 0-1)
        # accumulates into an SBUF staging tile during earlier blocks; partB
        # (heads 2-3) only trails the last attention block
        facc = acts.tile([P, QSH // P, QD], f32)

        def gen_final_a(m):
            ps = psA.tile([P, 512], f32, tag="acc")
            for a in (0, 1):
                nc.tensor.matmul(
                    ps[:],
                    outT_sb[:, a, m * P:(m + 1) * P],
                    wo_sb[:, a, :],
                    start=(a == 0),
                    stop=(a == 1),
                )
                if a == 0:
                    yield
            nc.vector.tensor_add(facc[:, m, :], ps[:], bo_sb[:])
            yield

        def gen_final_b(m):
            ps = psA.tile([P, 512], f32, tag="acc")
            for a in (2, 3):
                nc.tensor.matmul(
                    ps[:],
                    outT_sb[:, a, m * P:(m + 1) * P],
                    wo_sb[:, a, :],
                    start=(a == 2),
                    stop=(a == 3),
                )
                if a == 2:
                    yield
            ob = outp.tile([P, QD], f32)
            nc.vector.tensor_add(ob[:], ps[:], facc[:, m, :])
            nc.sync.dma_start(out_d[m * P:(m + 1) * P, :], ob[:])
            yield

        # split final-b for the first two tail m-tiles: the a=2 matmul only
        # needs head group 2 (written by block (2,1)'s tail), so it runs as a
        # block-(3,1) extra; only the a=3 matmul + add + DMA trail the last
        # normalize.  At most TWO may be pre-opened: their live PSUM tiles
        # plus the two tail rcps tiles exactly fill the 4-slot acc rotation.
        fb_ps = {}

        def gen_fb_a2(m):
            ps = psA.tile([P, 512], f32, tag="acc", name="fbps")
            nc.tensor.matmul(
                ps[:], outT_sb[:, 2, m * P:(m + 1) * P], wo_sb[:, 2, :],
                start=True, stop=False,
            )
            fb_ps[m] = ps
            yield

        def fb_a3(m):
            ps = fb_ps[m]
            nc.tensor.matmul(
                ps[:], outT_sb[:, 3, m * P:(m + 1) * P], wo_sb[:, 3, :],
                start=False, stop=True,
            )
            ob = outp.tile([P, QD], f32)
            nc.vector.tensor_add(ob[:], ps[:], facc[:, m, :])
            nc.sync.dma_start(out_d[m * P:(m + 1) * P, :], ob[:])

        def gen_chain(*gens):
            for g in gens:
                yield from g

        def run_gen(g):
            for _ in g:
                pass

        def attn(hp, n, extras=None, pre_pop=0, per_iter=0, prev_tail=None,
                 drain=True, act_copy_norm=False, split_tail_exp=False):
            """Emit one attention block.  Returns a closure that emits the
            block's last two AV matmuls + normalize; the caller passes it to
            the NEXT block so those trail instructions interleave with the
            next block's leading scores (removes the block-boundary bubble).
            """
            if extras is None:
                extras = iter(())

            def pop(k):
                for _ in range(k):
                    if next(extras, StopIteration) is StopIteration:
                        break

            av0 = psA.tile([P, 512], f32, tag="acc")
            av1 = psA.tile([P, 512], f32, tag="acc")
            av = (av0, av1)
            exs = []

            def s_(t):
                ss = psS.tile([P, 1024], f32)
                for i in range(2):
                    pr = slice(i * 64, (i + 1) * 64)
                    nc.tensor.matmul(
                        ss[:, i * 512:(i + 1) * 512],
                        kT8[pr, hp, t * P:(t + 1) * P]
                        .unsqueeze(1).broadcast_to((64, 2, P)),
                        qT8[pr, hp, :, n * 512:(n + 1) * 512],
                        start=True,
                        stop=True,
                        perf_mode=DR,
                    )
                if split_tail_exp and t == NT - 1:
                    # last exp of the LAST block split per head: the final AV
                    # for head-even (and so the whole normalize/final-b tail
                    # chain) starts one half-exp earlier.
                    exa = expp.tile([P, 512], f16, tag="exh", name="exa")
                    exb = expp.tile([P, 512], f16, tag="exh", name="exb")
                    nc.scalar.activation(
                        exa[:], ss[:, 0:512], Exp,
                        scale=SCALE / (WSCALE * WSCALE_K),
                    )
                    nc.scalar.activation(
                        exb[:], ss[:, 512:1024], Exp,
                        scale=SCALE / (WSCALE * WSCALE_K),
                    )
                    exs.append((exa, exb))
                    return
                ex = expp.tile([P, 1024], f16)
                nc.scalar.activation(
                    ex[:], ss[:], Exp, scale=SCALE / (WSCALE * WSCALE_K)
                )
                exs.append(ex)

            def A_(t):
                for i in range(2):
                    s = 2 * hp + i
                    e = exs[t]
                    rhs = e[i][:, :] if isinstance(e, tuple) \
                        else e[:, i * 512:(i + 1) * 512]
                    nc.tensor.matmul(
                        av[i][:],
                        v_sb[:, t, s * P:(s + 1) * P],
                        rhs,
                        start=(t == 0),
                        stop=(t == NT - 1),
                    )

            s_(0)
            s_(1)
            pop(pre_pop)
            if prev_tail is not None:
                prev_tail()
            for t in range(2, NT):
                s_(t)
                A_(t - 2)
                pop(per_iter)
            if drain:  # drain leftovers so every generator completes
                for _ in extras:
                    pass

            def tail():
                A_(NT - 2)
                A_(NT - 1)
                # partition broadcast of 1/denom via a PE ones-column matmul
                # (GpSimd partition_broadcast proved flaky on HW; the DMA
                # round-trip costs ~4us per block).  The reciprocal lands on
                # partition 0 in f16, ones.T @ recip fills a PSUM tile,
                # which is copied to SBUF for the multiply (ScalarE for the
                # last block where it is idle, DVE elsewhere).
                for i in range(2):
                    drow = 64 if i == 0 else 0
                    dst = slice(0, 64) if i == 0 else slice(64, 128)
                    rc = small.tile([1, 512], f16, tag="rc")
                    with nc.allow_low_precision(
                        reason="softmax denom reciprocal, f16 suffices"
                    ):
                        nc.vector.reciprocal(
                            rc[0:1, :], av[i][drow:drow + 1, :]
                        )
                    rcps = psA.tile([P, 512], f32, tag="acc")
                    nc.tensor.matmul(
                        rcps[:], ones_sb[0:1, :], rc[0:1, :],
                        start=True, stop=True,
                    )
                    rcb = small.tile([P, 512], f32, tag="rcb")
                    if act_copy_norm:
                        nc.scalar.copy(rcb[:], rcps[:])
                    else:
                        nc.vector.tensor_copy(rcb[:], rcps[:])
                    nc.vector.tensor_mul(
                        outT_sb[dst, hp, n * 512:(n + 1) * 512],
                        av[i][dst, :],
                        rcb[dst, :],
                    )

            return tail

        # ---- emission = per-engine execution order.  Warmup dummies bridge
        # the DMA head up to qT(0,0); kT(0) kv-half 0 slots into the gap as
        # soon as its DMA lands; everything else (v, kT second half, later
        # k/q projections, finals) interleaves into attention-block PE slack.
        v_memsets()
        warmup(CFG["W1"])
        for a in range(AT):
            run_gen(gen_proj_q(a, 0))
        run_gen(gen_proj_k(0, nns=(0,)))
        tail = attn(
            0, 0,
            extras=gen_chain(
                gen_proj_v(0), gen_proj_v(1),
                gen_proj_k(0, nns=(1,)),
                *[gen_proj_v(t) for t in range(2, NT)],
                gen_proj_k(1, nns=(0,)),
            ),
            pre_pop=CFG["B0_PRE"], per_iter=CFG["B0_PER"],
        )
        tail = attn(1, 0, extras=gen_chain(gen_proj_k(1, nns=(1,)),
                                           gen_proj_q(0, 1),
                                           gen_proj_k(2), gen_proj_q(1, 1)),
                    pre_pop=CFG["BK_PRE"], per_iter=CFG["BK_PER"],
                    prev_tail=tail)
        tail = attn(2, 0, extras=gen_chain(gen_proj_k(3), gen_proj_q(2, 1)),
                    pre_pop=CFG["BK_PRE"], per_iter=CFG["BK_PER"],
                    prev_tail=tail)
        tail = attn(3, 0, extras=gen_proj_q(3, 1), pre_pop=0, per_iter=1,
                    prev_tail=tail)
        f01 = gen_chain(gen_final(0), gen_final(1))
        tail = attn(0, 1, extras=f01, pre_pop=0, per_iter=1,
                    prev_tail=tail, drain=False)
        tail = attn(1, 1, extras=gen_chain(f01, gen_final(2)),
                    pre_pop=0, per_iter=1, prev_tail=tail)
        tail = attn(2, 1,
                    extras=gen_chain(gen_final(3), gen_final_a(4)),
                    pre_pop=0, per_iter=CFG["B21_PER"], prev_tail=tail)
        tail = attn(3, 1, act_copy_norm=True, split_tail_exp=True,
                    extras=gen_chain(gen_final_a(5), gen_final_a(6),
                                     gen_final_a(7)),
                    pre_pop=0, per_iter=CFG["B31_PER"], prev_tail=tail)
        tail()
        for m in range(AT, QSH // P):
            run_gen(gen_final_b(m))

    nc.finalize()
    return nc


def _get_nc():
    if "nc" not in _CACHE:
        _CACHE["nc"] = _build_nc()
    return _CACHE["nc"]


def make_in_maps(hidden_states, encoder_hidden_states, Wq, Wk, Wv, Wo, bo):
    f16 = np.float16
    hs = np.asarray(hidden_states, dtype=np.float32)
    enc = np.asarray(encoder_hidden_states, dtype=np.float32)
    import ml_dtypes

    f8 = ml_dtypes.float8_e4m3
    wq = np.ascontiguousarray(np.asarray(Wq, dtype=np.float32) * WSCALE).astype(f16)
    wk32 = np.asarray(Wk, dtype=np.float32) * WSCALE_K
    wk = np.ascontiguousarray(wk32[:, 0:P]).astype(f16)
    wk8 = np.ascontiguousarray(wk32).astype(f8)
    wv = np.ascontiguousarray(np.asarray(Wv, dtype=np.float32)).astype(f16)
    wo = np.ascontiguousarray(np.asarray(Wo, dtype=np.float32)).astype(f16)
    bo_ = np.ascontiguousarray(np.asarray(bo, dtype=np.float32)).reshape(1, QD)
    encT = [np.ascontiguousarray(enc[b].T).astype(f16) for b in range(B)]
    encT8 = []
    for b in range(B):
        e = enc[b].T.astype(np.float32)
        hi = e.astype(f8)
        lo = (e - hi.astype(np.float32)).astype(f8)
        encT8.append(np.ascontiguousarray(np.stack([hi, lo], axis=1)))
    in_maps = []
    for c in range(NCORES):
        b, s = divmod(c, 4)
        hsT = np.ascontiguousarray(hs[b, s * QSH:(s + 1) * QSH, :].T).astype(f16)
        in_maps.append(
            dict(hsT=hsT, encT=encT[b], wq=wq, wk=wk, wv=wv, wo=wo, bo=bo_,
                 wk8=wk8, encT8=encT8[b])
        )
    return in_maps


def kernel(hidden_states, encoder_hidden_states, Wq, Wk, Wv, Wo, bo):
    global LAST_RESULTS
    from concourse.bass_utils import run_bass_kernel_spmd

    nc = _get_nc()
    in_maps = make_in_maps(
        hidden_states, encoder_hidden_states, Wq, Wk, Wv, Wo, bo
    )
    res = run_bass_kernel_spmd(nc, in_maps, core_ids=list(range(NCORES)))
    LAST_RESULTS = res
    out = np.empty((B, LQ, QD), dtype=np.float32)
    for c in range(NCORES):
        b, s = divmod(c, 4)
        out[b, s * QSH:(s + 1) * QSH, :] = res.results[c]["out"]
    return out



# revision 69
# speedup vs baseline: 1.0575x; 1.0097x over previous
"""CrossAttention kernel for 8 TRN2 NeuronCores.

Reference computation (B=2, Lq=4096, Lkv=1024, query_dim=512, cross_dim=768,
heads=8, dim_head=64, inner=512):
    q = hs @ Wq; k = enc @ Wk; v = enc @ Wv          (per batch)
    attn = softmax(q_h @ k_h^T * scale) per head
    out = concat_h(attn @ v_h) @ Wo + bo

Sharding: 8 cores = 2 batches x 4 query-slices of 1024 queries.  Each core
computes its full slice of the output (all heads), so outputs are disjoint
and no collective is needed.

Per-core dataflow (fp16 operands, fp32 PSUM accumulate, with the scores
pipeline in fp8 DoubleRow — the cost model charges DR matmuls 0.5
cycles/row, so these run at half the PE cost):
  - host passes hs-slice and encoder transposed (hsT [512,1024], encT
    [768,1024]) in fp16, plus fp8 copies for the DoubleRow paths: an
    encoder (hi, lo) fp8 plane pair and wk8 = fp8(Wk*32).  Wq is
    pre-scaled x8 and Wk x32 to center fp8 dynamic range; exp() absorbs
    the 1/256.
  - qT = Wq^T-weighted hsT -> [inner=512, q=1024] (heads along
    partitions), stored as an fp8 (hi, lo) pair (DVE copy + subtract)
  - kT likewise -> [inner=512, kv=1024], stored as single-plane fp8.
    Head group a=0 projects in fp16 off the early encoder DMA; groups
    a=1..3 project via one-sided-fp8 DoubleRow: stationary wk8 rides both
    planes through a stride-0 broadcast AP, the moving operand is the
    encoder (hi, lo) pair, so only the wk8 side carries fp8 error.
  - v natural -> [kv=1024, slots] fp16; slot h = 128 cols holding v_h
    (64) + a ones column + zero padding, so the AV output lands
    partition-aligned and the softmax denominator falls out of the same
    matmul.
  - scoresT_h = k_h qT_h via ONE DoubleRow matmul per head: stationary
    (k8, k8) via stride-0 broadcast, moving (q_hi, q_lo), giving
    sum_j k8*(q_hi+q_lo) = k8 * q at fp8-pair precision.  Only the
    k-side carries fp8 error, which softmax normalization largely
    absorbs (q-side fp8 measures ~2x worse; both-sides fails the gate).
  - expT = exp(scale/256 * scoresT) on ScalarE, fp16 out (no
    max-subtraction: |scaled scores| < ~3)
  - outT_unnorm_h = v_slot^T @ expT accumulated over kv chunks (PSUM),
    one row of which is the softmax denominator
  - normalize: reciprocal (DVE) + PE ones-column broadcast matmul +
    multiply
  - final = outT^T @ Wo + bo -> [1024, 512], DMA out per 128-row tile
Measured absmax/scale ~1.05e-2 against the fp64 reference (gate 2e-2).

Program order is pipelined for the Tile scheduler: k/v/q projections are
emitted ahead of the attention blocks that consume them, exp(t) is emitted
before AV(t-1) so the PE never waits in-order on ScalarE, and the final
projection m-tiles are interleaved between the last attention blocks.
"""

import sys

if "/opt/trn_rl_repo" not in sys.path:
    sys.path.insert(0, "/opt/trn_rl_repo")

import numpy as np

B, LQ, LKV = 2, 4096, 1024
QD, CD = 512, 768
H, DH = 8, 64
INNER = H * DH  # 512
SCALE = DH ** -0.5
NCORES = 8
WSCALE = 8.0    # host-side pre-scale on Wq (fp8 range centering)
WSCALE_K = 32.0  # host-side pre-scale on Wk (fp8 range centering)
QSH = LQ // 4  # 1024 queries per core
P = 128

_CACHE: dict = {}
LAST_RESULTS = None  # test harness introspection (exec_time_ns etc.)

# schedule-tuning knobs (sweepable from bench tooling)
CFG = {
    "W1": 9,       # warmup matmuls bridging the input-DMA head
    "W2": 4,        # second warmup burst bridging q(0,0) -> k(0) DMA gap
    "B0_PRE": 12,    # block (0,0) pre-loop extra pops
    "B0_PER": 9,    # block (0,0) per-iter extra pops
    "BK_PRE": 2,    # later n=0 blocks pre-loop pops
    "BK_PER": 3,    # later n=0 blocks per-iter pops
    "B21_PER": 1,   # block (2,1) per-iter pops
    "B31_PER": 1,   # block (3,1) per-iter pops
}


def _build_nc():
    from contextlib import ExitStack

    import concourse.bass as bass
    import concourse.tile as tile
    from concourse import bacc, mybir

    f32 = mybir.dt.float32
    f16 = mybir.dt.float16
    f8 = mybir.dt.float8e4
    DR = mybir.MatmulPerfMode.DoubleRow
    Exp = mybir.ActivationFunctionType.Exp

    nc = bacc.Bacc(trn_type="TRN2")

    hsT_d = nc.declare_dram_parameter("hsT", [QD, QSH], f16, isOutput=False)
    encT_d = nc.declare_dram_parameter("encT", [CD, LKV], f16, isOutput=False)
    wq_d = nc.declare_dram_parameter("wq", [QD, INNER], f16, isOutput=False)
    wk_d = nc.declare_dram_parameter("wk", [CD, P], f16, isOutput=False)
    wv_d = nc.declare_dram_parameter("wv", [CD, INNER], f16, isOutput=False)
    wo_d = nc.declare_dram_parameter("wo", [INNER, QD], f16, isOutput=False)
    bo_d = nc.declare_dram_parameter("bo", [1, QD], f32, isOutput=False)
    wk8_d = nc.declare_dram_parameter("wk8", [CD, INNER], f8, isOutput=False)
    encT8_d = nc.declare_dram_parameter("encT8", [CD, 2, LKV], f8, isOutput=False)
    out_d = nc.declare_dram_parameter("out", [QSH, QD], f32, isOutput=True)

    KC_Q = QD // P   # 4 contraction chunks for q projection
    KC_KV = CD // P  # 6 for k/v projections
    AT = INNER // P  # 4 inner tiles (2 heads each)
    NT = LKV // P    # 8 kv chunks
    QN = QSH // 512  # 2 q slices of 512

    with ExitStack() as ctx:
        tc = ctx.enter_context(tile.TileContext(nc))
        const = ctx.enter_context(tc.tile_pool(name="const", bufs=1))
        acts = ctx.enter_context(tc.tile_pool(name="acts", bufs=1))
        expp = ctx.enter_context(tc.tile_pool(name="expp", bufs=8))
        outp = ctx.enter_context(tc.tile_pool(name="outp", bufs=6))
        small = ctx.enter_context(tc.tile_pool(name="small", bufs=8))
        psA = ctx.enter_context(tc.tile_pool(name="psA", bufs=4, space="PSUM"))
        psS = ctx.enter_context(tc.tile_pool(name="psS", bufs=2, space="PSUM"))
        drp = ctx.enter_context(tc.tile_pool(name="drp", bufs=4, space="DRAM"))

        # ---- input DMA, ordered by first use: the q projections (hsT+wq)
        # run during the PE warmup window, then kT (encT+wk), then v (wv);
        # the second encT half only gates scores t>=4 of the first block
        hsT_sb = acts.tile([P, KC_Q, QSH], f16)
        hsT_r = hsT_d.rearrange("(c p) n -> p c n", p=P)
        nc.sync.dma_start(hsT_sb[:, :, 0:512], hsT_r[:, :, 0:512])
        wq_sb = const.tile([P, KC_Q, INNER], f16)
        nc.sync.dma_start(wq_sb[:], wq_d.rearrange("(c p) n -> p c n", p=P))
        encT_sb = acts.tile([P, KC_KV, LKV], f16)
        encT_r = encT_d.rearrange("(c p) n -> p c n", p=P)
        wk_sb = const.tile([P, KC_KV, P], f16)
        nc.sync.dma_start(wk_sb[:], wk_d.rearrange("(c p) n -> p c n", p=P))
        nc.sync.dma_start(encT_sb[:, 0:2, 0:512], encT_r[:, 0:2, 0:512])
        nc.sync.dma_start(encT_sb[:, 2:4, 0:512], encT_r[:, 2:4, 0:512])
        nc.sync.dma_start(encT_sb[:, 4:6, 0:512], encT_r[:, 4:6, 0:512])
        wv_sb = const.tile([P, KC_KV, INNER], f16)
        nc.sync.dma_start(wv_sb[:], wv_d.rearrange("(c p) n -> p c n", p=P))
        nc.sync.dma_start(encT_sb[:, :, 512:1024], encT_r[:, :, 512:1024])
        nc.sync.dma_start(hsT_sb[:, :, 512:1024], hsT_r[:, :, 512:1024])
        # fp8 operands for the DoubleRow k projections (k heads 2..7): the
        # encoder comes as an fp8 (hi, lo) plane pair; wk8 rides both planes
        # of the stationary operand via a stride-0 broadcast
        encT8_sb = acts.tile([P, KC_KV, 2, LKV], f8)
        encT8_r = encT8_d.rearrange("(c p) j n -> p c j n", p=P)
        wk8_sb = const.tile([P, KC_KV, INNER], f8)
        nc.sync.dma_start(encT8_sb[:, :, 0, 0:512], encT8_r[:, :, 0, 0:512])
        nc.sync.dma_start(encT8_sb[:, :, 1, 0:512], encT8_r[:, :, 1, 0:512])
        nc.sync.dma_start(wk8_sb[:], wk8_d.rearrange("(c p) n -> p c n", p=P))
        nc.sync.dma_start(encT8_sb[:, :, 0, 512:1024], encT8_r[:, :, 0, 512:1024])
        nc.sync.dma_start(encT8_sb[:, :, 1, 512:1024], encT8_r[:, :, 1, 512:1024])
        wo_sb = const.tile([P, AT, QD], f16)
        nc.sync.dma_start(wo_sb[:], wo_d.rearrange("(c p) n -> p c n", p=P))
        bo_sb = const.tile([P, QD], f32)
        nc.sync.dma_start(bo_sb[:], bo_d.ap().to_broadcast((P, QD)))

        qT8 = acts.tile([P, AT, 2, QSH], f8)       # planes: (hi, lo)
        kT8 = acts.tile([P, AT, LKV], f8)          # single plane, broadcast in DR
        v_sb = acts.tile([P, NT, H * P], f16)
        outT_sb = acts.tile([P, AT, QSH], f16)
        vv4 = v_sb.rearrange("p t (s c) -> p t s c", c=P)

        # ---- PE warmup: dummy matmuls on zeroed scratch fill the DMA head
        # so the first real matmuls run at full clock (psD is never read)
        scratch = acts.tile([P, 512], f16)
        nc.gpsimd.memset(scratch[:], 0.0)

        # ones column for the PE-side partition broadcast in normalize
        ones_sb = const.tile([1, P], f16)
        nc.vector.memset(ones_sb[:], 1.0)

        def warmup(nmm):
            psD = psA.tile([P, 512], f32, tag="acc")
            for i in range(nmm):
                nc.tensor.matmul(
                    psD[:], scratch[:, 0:P], scratch[:],
                    start=(i == 0), stop=(i == nmm - 1),
                )

        # Generators yield once per emitted PE matmul so attention blocks can
        # interleave them into PE slack at a controlled rate (the per-engine
        # instruction streams execute strictly in program order).
        def gen_proj_k(a, nns=(0, 1)):
            # trailing copies are emitted BEFORE the final yield so that a
            # fully-popped generator has fully emitted its writes.
            # a=0 runs in fp16 off the early fp16 encoder DMA; a>=1 runs as
            # one-sided-fp8 DoubleRow (half PE cost; only the wk8 side
            # carries fp8 error, which softmax normalization absorbs).
            for nn in nns:
                ps = psA.tile([P, 512], f32, tag="acc")
                for c in range(KC_KV):
                    if a == 0:
                        nc.tensor.matmul(
                            ps[:],
                            wk_sb[:, c, a * P:(a + 1) * P],
                            encT_sb[:, c, nn * 512:(nn + 1) * 512],
                            start=(c == 0),
                            stop=(c == KC_KV - 1),
                        )
                    else:
                        nc.tensor.matmul(
                            ps[:],
                            wk8_sb[:, c, a * P:(a + 1) * P]
                            .unsqueeze(1).broadcast_to((P, 2, P)),
                            encT8_sb[:, c, :, nn * 512:(nn + 1) * 512],
                            start=(c == 0),
                            stop=(c == KC_KV - 1),
                            perf_mode=DR,
                        )
                    if c < KC_KV - 1:
                        yield
                with nc.allow_low_precision(reason="k stored fp8 for DR scores"):
                    if a == 0 and nn == 0:
                        # split so the first two scores matmuls (kv chunks
                        # 0-1) are gated by a half-copy, not the full 512
                        nc.vector.tensor_copy(kT8[:, 0, 0:256], ps[:, 0:256])
                        nc.vector.tensor_copy(
                            kT8[:, 0, 256:512], ps[:, 256:512]
                        )
                    else:
                        nc.vector.tensor_copy(
                            kT8[:, a, nn * 512:(nn + 1) * 512], ps[:]
                        )
                yield

        def gen_proj_q(a, n):
            ps = psA.tile([P, 512], f32, tag="acc")
            for c in range(KC_Q):
                nc.tensor.matmul(
                    ps[:],
                    wq_sb[:, c, a * P:(a + 1) * P],
                    hsT_sb[:, c, n * 512:(n + 1) * 512],
                    start=(c == 0),
                    stop=(c == KC_Q - 1),
                )
                if c < KC_Q - 1:
                    yield
            sl = slice(n * 512, (n + 1) * 512)
            with nc.allow_low_precision(reason="q stored as fp8 hi/lo pair"):
                nc.vector.tensor_copy(qT8[:, a, 0, sl], ps[:])
                nc.vector.tensor_sub(qT8[:, a, 1, sl], ps[:], qT8[:, a, 0, sl])
            yield

        # v natural [kv, slots]: slot h (128 wide):
        #   h even: [v_h (0:64) | 1.0 at 64 | 0 at 65:128]   -> out rows 0:64, denom row 64
        #   h odd : [1.0 at 0 | 0 at 1:64 | v_h at 64:128]   -> out rows 64:128, denom row 0
        def v_memsets():
            nc.gpsimd.memset(vv4[:, :, 0::2, 64:65], 1.0)
            nc.gpsimd.memset(vv4[:, :, 1::2, 0:1], 1.0)
            nc.gpsimd.memset(vv4[:, :, 0::2, 65:P], 0.0)
            nc.gpsimd.memset(vv4[:, :, 1::2, 1:DH], 0.0)

        def gen_proj_v(t):
            ps = psA.tile([P, 512], f32, tag="acc")
            for c in range(KC_KV):
                nc.tensor.matmul(
                    ps[:],
                    encT_sb[:, c, t * P:(t + 1) * P],
                    wv_sb[:, c, :],
                    start=(c == 0),
                    stop=(c == KC_KV - 1),
                )
                if c < KC_KV - 1:
                    yield
            pv = ps.rearrange("p (s c) -> p s c", c=DH)
            nc.vector.tensor_copy(vv4[:, t, 0::2, 0:DH], pv[:, 0::2, :])
            nc.vector.tensor_copy(vv4[:, t, 1::2, DH:P], pv[:, 1::2, :])
            yield

        def gen_final(m):
            ps = psA.tile([P, 512], f32, tag="acc")
            for a in range(AT):
                nc.tensor.matmul(
                    ps[:],
                    outT_sb[:, a, m * P:(m + 1) * P],
                    wo_sb[:, a, :],
                    start=(a == 0),
                    stop=(a == AT - 1),
                )
                if a < AT - 1:
                    yield
            ob = outp.tile([P, QD], f32)
            nc.vector.tensor_add(ob[:], ps[:], bo_sb[:])
            nc.sync.dma_start(out_d[m * P:(m + 1) * P, :], ob[:])
            yield

        # final projection split for the tail m-tiles: partA (heads 0-1)
        # accumulates into an SBUF staging tile during earlier blocks; partB
        # (heads 2-3) only trails the last attention block
        facc = acts.tile([P, QSH // P, QD], f32)

        def gen_final_a(m):
            ps = psA.tile([P, 512], f32, tag="acc")
            for a in (0, 1):
                nc.tensor.matmul(
                    ps[:],
                    outT_sb[:, a, m * P:(m + 1) * P],
                    wo_sb[:, a, :],
                    start=(a == 0),
                    stop=(a == 1),
                )
                if a == 0:
                    yield
            nc.vector.tensor_add(facc[:, m, :], ps[:], bo_sb[:])
            yield

        def gen_final_b(m):
            ps = psA.tile([P, 512], f32, tag="acc")
            for a in (2, 3):
                nc.tensor.matmul(
                    ps[:],
                    outT_sb[:, a, m * P:(m + 1) * P],
                    wo_sb[:, a, :],
                    start=(a == 2),
                    stop=(a == 3),
                )
                if a == 2:
                    yield
            ob = outp.tile([P, QD], f32)
            nc.vector.tensor_add(ob[:], ps[:], facc[:, m, :])
            nc.sync.dma_start(out_d[m * P:(m + 1) * P, :], ob[:])
            yield

        # split final-b for the first two tail m-tiles: the a=2 matmul only
        # needs head group 2 (written by block (2,1)'s tail), so it runs as a
        # block-(3,1) extra; only the a=3 matmul + add + DMA trail the last
        # normalize.  At most TWO may be pre-opened: their live PSUM tiles
        # plus the two tail rcps tiles exactly fill the 4-slot acc rotation.
        fb_ps = {}

        def gen_fb_a2(m):
            ps = psA.tile([P, 512], f32, tag="acc", name="fbps")
            nc.tensor.matmul(
                ps[:], outT_sb[:, 2, m * P:(m + 1) * P], wo_sb[:, 2, :],
                start=True, stop=False,
            )
            fb_ps[m] = ps
            yield

        def fb_a3(m):
            ps = fb_ps[m]
            nc.tensor.matmul(
                ps[:], outT_sb[:, 3, m * P:(m + 1) * P], wo_sb[:, 3, :],
                start=False, stop=True,
            )
            ob = outp.tile([P, QD], f32)
            nc.vector.tensor_add(ob[:], ps[:], facc[:, m, :])
            nc.sync.dma_start(out_d[m * P:(m + 1) * P, :], ob[:])

        def gen_chain(*gens):
            for g in gens:
                yield from g

        def run_gen(g):
            for _ in g:
                pass

        def attn(hp, n, extras=None, pre_pop=0, per_iter=0, prev_tail=None,
                 drain=True, act_copy_norm=False, split_tail_exp=False,
                 tail_mid=None):
            """Emit one attention block.  Returns a closure that emits the
            block's last two AV matmuls + normalize; the caller passes it to
            the NEXT block so those trail instructions interleave with the
            next block's leading scores (removes the block-boundary bubble).
            """
            if extras is None:
                extras = iter(())

            def pop(k):
                for _ in range(k):
                    if next(extras, StopIteration) is StopIteration:
                        break

            av0 = psA.tile([P, 512], f32, tag="acc")
            av1 = psA.tile([P, 512], f32, tag="acc")
            av = (av0, av1)
            exs = []

            def s_(t):
                ss = psS.tile([P, 1024], f32)
                for i in range(2):
                    pr = slice(i * 64, (i + 1) * 64)
                    nc.tensor.matmul(
                        ss[:, i * 512:(i + 1) * 512],
                        kT8[pr, hp, t * P:(t + 1) * P]
                        .unsqueeze(1).broadcast_to((64, 2, P)),
                        qT8[pr, hp, :, n * 512:(n + 1) * 512],
                        start=True,
                        stop=True,
                        perf_mode=DR,
                    )
                if split_tail_exp and t == NT - 1:
                    # last exp of the LAST block split per head: the final AV
                    # for head-even (and so the whole normalize/final-b tail
                    # chain) starts one half-exp earlier.
                    exa = expp.tile([P, 512], f16, tag="exh", name="exa")
                    exb = expp.tile([P, 512], f16, tag="exh", name="exb")
                    nc.scalar.activation(
                        exa[:], ss[:, 0:512], Exp,
                        scale=SCALE / (WSCALE * WSCALE_K),
                    )
                    nc.scalar.activation(
                        exb[:], ss[:, 512:1024], Exp,
                        scale=SCALE / (WSCALE * WSCALE_K),
                    )
                    exs.append((exa, exb))
                    return
                ex = expp.tile([P, 1024], f16)
                nc.scalar.activation(
                    ex[:], ss[:], Exp, scale=SCALE / (WSCALE * WSCALE_K)
                )
                exs.append(ex)

            def A_(t):
                for i in range(2):
                    s = 2 * hp + i
                    e = exs[t]
                    rhs = e[i][:, :] if isinstance(e, tuple) \
                        else e[:, i * 512:(i + 1) * 512]
                    nc.tensor.matmul(
                        av[i][:],
                        v_sb[:, t, s * P:(s + 1) * P],
                        rhs,
                        start=(t == 0),
                        stop=(t == NT - 1),
                    )

            s_(0)
            s_(1)
            pop(pre_pop)
            if prev_tail is not None:
                prev_tail()
            for t in range(2, NT):
                s_(t)
                A_(t - 2)
                pop(per_iter)
            if drain:  # drain leftovers so every generator completes
                for _ in extras:
                    pass

            def tail():
                A_(NT - 2)
                A_(NT - 1)
                # partition broadcast of 1/denom via a PE ones-column matmul
                # (GpSimd partition_broadcast proved flaky on HW; the DMA
                # round-trip costs ~4us per block).  The reciprocal lands on
                # partition 0 in f16, ones.T @ recip fills a PSUM tile,
                # which is copied to SBUF for the multiply (ScalarE for the
                # last block where it is idle, DVE elsewhere).
                rcbs = []
                for i in range(2):
                    drow = 64 if i == 0 else 0
                    rc = small.tile([1, 512], f16, tag="rc")
                    with nc.allow_low_precision(
                        reason="softmax denom reciprocal, f16 suffices"
                    ):
                        nc.vector.reciprocal(
                            rc[0:1, :], av[i][drow:drow + 1, :]
                        )
                    rcps = psA.tile([P, 512], f32, tag="acc")
                    nc.tensor.matmul(
                        rcps[:], ones_sb[0:1, :], rc[0:1, :],
                        start=True, stop=True,
                    )
                    rcb = small.tile([P, 512], f32, tag="rcb")
                    if act_copy_norm:
                        # last block: ScalarE staging copies in column halves
                        # so the first multiplies (and the final-b m-tiles
                        # they gate) start after half the copy latency
                        nc.scalar.copy(rcb[:, 0:256], rcps[:, 0:256])
                        nc.scalar.copy(rcb[:, 256:512], rcps[:, 256:512])
                    else:
                        nc.vector.tensor_copy(rcb[:], rcps[:])
                    rcbs.append(rcb)
                dsts = (slice(0, 64), slice(64, 128))
                if act_copy_norm:
                    # column-half multiplies with the first two final-b
                    # m-tiles emitted in between: their matmuls/adds/DMAs
                    # only need outT cols 0:256, so the serial out-DMA chain
                    # starts while the second halves still compute
                    for cols in (slice(0, 256), slice(256, 512)):
                        for i in range(2):
                            nc.vector.tensor_mul(
                                outT_sb[dsts[i], hp,
                                        n * 512 + cols.start:
                                        n * 512 + cols.stop],
                                av[i][dsts[i], cols],
                                rcbs[i][dsts[i], cols],
                            )
                        if cols.start == 0 and tail_mid is not None:
                            tail_mid()
                else:
                    for i in range(2):
                        nc.vector.tensor_mul(
                            outT_sb[dsts[i], hp, n * 512:(n + 1) * 512],
                            av[i][dsts[i], :],
                            rcbs[i][dsts[i], :],
                        )

            return tail

        # ---- emission = per-engine execution order.  Warmup dummies bridge
        # the DMA head up to qT(0,0); kT(0) kv-half 0 slots into the gap as
        # soon as its DMA lands; everything else (v, kT second half, later
        # k/q projections, finals) interleaves into attention-block PE slack.
        v_memsets()
        warmup(CFG["W1"])
        run_gen(gen_proj_q(0, 0))
        run_gen(gen_proj_q(1, 0))
        run_gen(gen_proj_k(0, nns=(0,)))
        run_gen(gen_proj_q(2, 0))
        run_gen(gen_proj_q(3, 0))
        tail = attn(
            0, 0,
            extras=gen_chain(
                gen_proj_v(0), gen_proj_v(1),
                gen_proj_k(0, nns=(1,)),
                *[gen_proj_v(t) for t in range(2, NT)],
                gen_proj_k(1, nns=(0,)),
            ),
            pre_pop=CFG["B0_PRE"], per_iter=CFG["B0_PER"],
        )
        tail = attn(1, 0, extras=gen_chain(gen_proj_k(1, nns=(1,)),
                                           gen_proj_q(0, 1),
                                           gen_proj_k(2), gen_proj_q(1, 1)),
                    pre_pop=CFG["BK_PRE"], per_iter=CFG["BK_PER"],
                    prev_tail=tail)
        tail = attn(2, 0, extras=gen_chain(gen_proj_k(3), gen_proj_q(2, 1)),
                    pre_pop=CFG["BK_PRE"], per_iter=CFG["BK_PER"],
                    prev_tail=tail)
        tail = attn(3, 0, extras=gen_proj_q(3, 1), pre_pop=0, per_iter=1,
                    prev_tail=tail)
        f01 = gen_chain(gen_final(0), gen_final(1))
        tail = attn(0, 1, extras=f01, pre_pop=0, per_iter=1,
                    prev_tail=tail, drain=False)
        tail = attn(1, 1, extras=gen_chain(f01, gen_final(2)),
                    pre_pop=0, per_iter=1, prev_tail=tail)
        tail = attn(2, 1,
                    extras=gen_chain(gen_final(3), gen_final_a(4)),
                    pre_pop=0, per_iter=CFG["B21_PER"], prev_tail=tail)
        tail = attn(3, 1, act_copy_norm=True, split_tail_exp=True,
                    tail_mid=lambda: (run_gen(gen_final_b(4)),
                                      run_gen(gen_final_b(5))),
                    extras=gen_chain(gen_final_a(5), gen_final_a(6),
                                     gen_final_a(7)),
                    pre_pop=0, per_iter=CFG["B31_PER"], prev_tail=tail)
        tail()
        for m in (6, 7):
            run_gen(gen_final_b(m))

    nc.finalize()
    return nc


def _get_nc():
    if "nc" not in _CACHE:
        _CACHE["nc"] = _build_nc()
    return _CACHE["nc"]


def make_in_maps(hidden_states, encoder_hidden_states, Wq, Wk, Wv, Wo, bo):
    f16 = np.float16
    hs = np.asarray(hidden_states, dtype=np.float32)
    enc = np.asarray(encoder_hidden_states, dtype=np.float32)
    import ml_dtypes

    f8 = ml_dtypes.float8_e4m3
    wq = np.ascontiguousarray(np.asarray(Wq, dtype=np.float32) * WSCALE).astype(f16)
    wk32 = np.asarray(Wk, dtype=np.float32) * WSCALE_K
    wk = np.ascontiguousarray(wk32[:, 0:P]).astype(f16)
    wk8 = np.ascontiguousarray(wk32).astype(f8)
    wv = np.ascontiguousarray(np.asarray(Wv, dtype=np.float32)).astype(f16)
    wo = np.ascontiguousarray(np.asarray(Wo, dtype=np.float32)).astype(f16)
    bo_ = np.ascontiguousarray(np.asarray(bo, dtype=np.float32)).reshape(1, QD)
    encT = [np.ascontiguousarray(enc[b].T).astype(f16) for b in range(B)]
    encT8 = []
    for b in range(B):
        e = enc[b].T.astype(np.float32)
        hi = e.astype(f8)
        lo = (e - hi.astype(np.float32)).astype(f8)
        encT8.append(np.ascontiguousarray(np.stack([hi, lo], axis=1)))
    in_maps = []
    for c in range(NCORES):
        b, s = divmod(c, 4)
        hsT = np.ascontiguousarray(hs[b, s * QSH:(s + 1) * QSH, :].T).astype(f16)
        in_maps.append(
            dict(hsT=hsT, encT=encT[b], wq=wq, wk=wk, wv=wv, wo=wo, bo=bo_,
                 wk8=wk8, encT8=encT8[b])
        )
    return in_maps


def kernel(hidden_states, encoder_hidden_states, Wq, Wk, Wv, Wo, bo):
    global LAST_RESULTS
    from concourse.bass_utils import run_bass_kernel_spmd

    nc = _get_nc()
    in_maps = make_in_maps(
        hidden_states, encoder_hidden_states, Wq, Wk, Wv, Wo, bo
    )
    res = run_bass_kernel_spmd(nc, in_maps, core_ids=list(range(NCORES)))
    LAST_RESULTS = res
    out = np.empty((B, LQ, QD), dtype=np.float32)
    for c in range(NCORES):
        b, s = divmod(c, 4)
        out[b, s * QSH:(s + 1) * QSH, :] = res.results[c]["out"]
    return out



# revision 70
# speedup vs baseline: 1.0603x; 1.0026x over previous
"""CrossAttention kernel for 8 TRN2 NeuronCores.

Reference computation (B=2, Lq=4096, Lkv=1024, query_dim=512, cross_dim=768,
heads=8, dim_head=64, inner=512):
    q = hs @ Wq; k = enc @ Wk; v = enc @ Wv          (per batch)
    attn = softmax(q_h @ k_h^T * scale) per head
    out = concat_h(attn @ v_h) @ Wo + bo

Sharding: 8 cores = 2 batches x 4 query-slices of 1024 queries.  Each core
computes its full slice of the output (all heads), so outputs are disjoint
and no collective is needed.

Per-core dataflow (fp16 operands, fp32 PSUM accumulate, with the scores
pipeline in fp8 DoubleRow — the cost model charges DR matmuls 0.5
cycles/row, so these run at half the PE cost):
  - host passes hs-slice and encoder transposed (hsT [512,1024], encT
    [768,1024]) in fp16, plus fp8 copies for the DoubleRow paths: an
    encoder (hi, lo) fp8 plane pair and wk8 = fp8(Wk*32).  Wq is
    pre-scaled x8 and Wk x32 to center fp8 dynamic range; exp() absorbs
    the 1/256.
  - qT = Wq^T-weighted hsT -> [inner=512, q=1024] (heads along
    partitions), stored as an fp8 (hi, lo) pair (DVE copy + subtract)
  - kT likewise -> [inner=512, kv=1024], stored as single-plane fp8.
    Head group a=0 projects in fp16 off the early encoder DMA; groups
    a=1..3 project via one-sided-fp8 DoubleRow: stationary wk8 rides both
    planes through a stride-0 broadcast AP, the moving operand is the
    encoder (hi, lo) pair, so only the wk8 side carries fp8 error.
  - v natural -> [kv=1024, slots] fp16; slot h = 128 cols holding v_h
    (64) + a ones column + zero padding, so the AV output lands
    partition-aligned and the softmax denominator falls out of the same
    matmul.
  - scoresT_h = k_h qT_h via ONE DoubleRow matmul per head: stationary
    (k8, k8) via stride-0 broadcast, moving (q_hi, q_lo), giving
    sum_j k8*(q_hi+q_lo) = k8 * q at fp8-pair precision.  Only the
    k-side carries fp8 error, which softmax normalization largely
    absorbs (q-side fp8 measures ~2x worse; both-sides fails the gate).
  - expT = exp(scale/256 * scoresT) on ScalarE, fp16 out (no
    max-subtraction: |scaled scores| < ~3)
  - outT_unnorm_h = v_slot^T @ expT accumulated over kv chunks (PSUM),
    one row of which is the softmax denominator
  - normalize: reciprocal (DVE) + PE ones-column broadcast matmul +
    multiply
  - final = outT^T @ Wo + bo -> [1024, 512], DMA out per 128-row tile
Measured absmax/scale ~1.05e-2 against the fp64 reference (gate 2e-2).

Program order is pipelined for the Tile scheduler: k/v/q projections are
emitted ahead of the attention blocks that consume them, exp(t) is emitted
before AV(t-1) so the PE never waits in-order on ScalarE, and the final
projection m-tiles are interleaved between the last attention blocks.
"""

import sys

if "/opt/trn_rl_repo" not in sys.path:
    sys.path.insert(0, "/opt/trn_rl_repo")

import numpy as np

B, LQ, LKV = 2, 4096, 1024
QD, CD = 512, 768
H, DH = 8, 64
INNER = H * DH  # 512
SCALE = DH ** -0.5
NCORES = 8
WSCALE = 8.0    # host-side pre-scale on Wq (fp8 range centering)
WSCALE_K = 32.0  # host-side pre-scale on Wk (fp8 range centering)
QSH = LQ // 4  # 1024 queries per core
P = 128

_CACHE: dict = {}
LAST_RESULTS = None  # test harness introspection (exec_time_ns etc.)

# schedule-tuning knobs (sweepable from bench tooling)
CFG = {
    "W1": 9,       # warmup matmuls bridging the input-DMA head
    "W2": 4,        # second warmup burst bridging q(0,0) -> k(0) DMA gap
    "B0_PRE": 12,    # block (0,0) pre-loop extra pops
    "B0_PER": 9,    # block (0,0) per-iter extra pops
    "BK_PRE": 2,    # later n=0 blocks pre-loop pops
    "BK_PER": 3,    # later n=0 blocks per-iter pops
    "B21_PER": 0,   # block (2,1) per-iter pops
    "B31_PER": 1,   # block (3,1) per-iter pops
}


def _build_nc():
    from contextlib import ExitStack

    import concourse.bass as bass
    import concourse.tile as tile
    from concourse import bacc, mybir

    f32 = mybir.dt.float32
    f16 = mybir.dt.float16
    f8 = mybir.dt.float8e4
    DR = mybir.MatmulPerfMode.DoubleRow
    Exp = mybir.ActivationFunctionType.Exp

    nc = bacc.Bacc(trn_type="TRN2")

    hsT_d = nc.declare_dram_parameter("hsT", [QD, QSH], f16, isOutput=False)
    encT_d = nc.declare_dram_parameter("encT", [CD, LKV], f16, isOutput=False)
    wq_d = nc.declare_dram_parameter("wq", [QD, INNER], f16, isOutput=False)
    wk_d = nc.declare_dram_parameter("wk", [CD, P], f16, isOutput=False)
    wv_d = nc.declare_dram_parameter("wv", [CD, INNER], f16, isOutput=False)
    wo_d = nc.declare_dram_parameter("wo", [INNER, QD], f16, isOutput=False)
    bo_d = nc.declare_dram_parameter("bo", [1, QD], f32, isOutput=False)
    wk8_d = nc.declare_dram_parameter("wk8", [CD, INNER], f8, isOutput=False)
    encT8_d = nc.declare_dram_parameter("encT8", [CD, 2, LKV], f8, isOutput=False)
    out_d = nc.declare_dram_parameter("out", [QSH, QD], f32, isOutput=True)

    KC_Q = QD // P   # 4 contraction chunks for q projection
    KC_KV = CD // P  # 6 for k/v projections
    AT = INNER // P  # 4 inner tiles (2 heads each)
    NT = LKV // P    # 8 kv chunks
    QN = QSH // 512  # 2 q slices of 512

    with ExitStack() as ctx:
        tc = ctx.enter_context(tile.TileContext(nc))
        const = ctx.enter_context(tc.tile_pool(name="const", bufs=1))
        acts = ctx.enter_context(tc.tile_pool(name="acts", bufs=1))
        expp = ctx.enter_context(tc.tile_pool(name="expp", bufs=8))
        outp = ctx.enter_context(tc.tile_pool(name="outp", bufs=6))
        small = ctx.enter_context(tc.tile_pool(name="small", bufs=8))
        psA = ctx.enter_context(tc.tile_pool(name="psA", bufs=4, space="PSUM"))
        psS = ctx.enter_context(tc.tile_pool(name="psS", bufs=2, space="PSUM"))
        drp = ctx.enter_context(tc.tile_pool(name="drp", bufs=4, space="DRAM"))

        # ---- input DMA, ordered by first use: the q projections (hsT+wq)
        # run during the PE warmup window, then kT (encT+wk), then v (wv);
        # the second encT half only gates scores t>=4 of the first block
        hsT_sb = acts.tile([P, KC_Q, QSH], f16)
        hsT_r = hsT_d.rearrange("(c p) n -> p c n", p=P)
        nc.sync.dma_start(hsT_sb[:, :, 0:512], hsT_r[:, :, 0:512])
        wq_sb = const.tile([P, KC_Q, INNER], f16)
        nc.sync.dma_start(wq_sb[:], wq_d.rearrange("(c p) n -> p c n", p=P))
        encT_sb = acts.tile([P, KC_KV, LKV], f16)
        encT_r = encT_d.rearrange("(c p) n -> p c n", p=P)
        wk_sb = const.tile([P, KC_KV, P], f16)
        nc.sync.dma_start(wk_sb[:], wk_d.rearrange("(c p) n -> p c n", p=P))
        nc.sync.dma_start(encT_sb[:, 0:2, 0:512], encT_r[:, 0:2, 0:512])
        nc.sync.dma_start(encT_sb[:, 2:4, 0:512], encT_r[:, 2:4, 0:512])
        nc.sync.dma_start(encT_sb[:, 4:6, 0:512], encT_r[:, 4:6, 0:512])
        wv_sb = const.tile([P, KC_KV, INNER], f16)
        nc.sync.dma_start(wv_sb[:], wv_d.rearrange("(c p) n -> p c n", p=P))
        nc.sync.dma_start(encT_sb[:, :, 512:1024], encT_r[:, :, 512:1024])
        nc.sync.dma_start(hsT_sb[:, :, 512:1024], hsT_r[:, :, 512:1024])
        # fp8 operands for the DoubleRow k projections (k heads 2..7): the
        # encoder comes as an fp8 (hi, lo) plane pair; wk8 rides both planes
        # of the stationary operand via a stride-0 broadcast
        encT8_sb = acts.tile([P, KC_KV, 2, LKV], f8)
        encT8_r = encT8_d.rearrange("(c p) j n -> p c j n", p=P)
        wk8_sb = const.tile([P, KC_KV, INNER], f8)
        nc.sync.dma_start(encT8_sb[:, :, 0, 0:512], encT8_r[:, :, 0, 0:512])
        nc.sync.dma_start(encT8_sb[:, :, 1, 0:512], encT8_r[:, :, 1, 0:512])
        nc.sync.dma_start(wk8_sb[:], wk8_d.rearrange("(c p) n -> p c n", p=P))
        nc.sync.dma_start(encT8_sb[:, :, 0, 512:1024], encT8_r[:, :, 0, 512:1024])
        nc.sync.dma_start(encT8_sb[:, :, 1, 512:1024], encT8_r[:, :, 1, 512:1024])
        wo_sb = const.tile([P, AT, QD], f16)
        nc.sync.dma_start(wo_sb[:], wo_d.rearrange("(c p) n -> p c n", p=P))
        bo_sb = const.tile([P, QD], f32)
        nc.sync.dma_start(bo_sb[:], bo_d.ap().to_broadcast((P, QD)))

        qT8 = acts.tile([P, AT, 2, QSH], f8)       # planes: (hi, lo)
        kT8 = acts.tile([P, AT, LKV], f8)          # single plane, broadcast in DR
        v_sb = acts.tile([P, NT, H * P], f16)
        outT_sb = acts.tile([P, AT, QSH], f16)
        vv4 = v_sb.rearrange("p t (s c) -> p t s c", c=P)

        # ---- PE warmup: dummy matmuls on zeroed scratch fill the DMA head
        # so the first real matmuls run at full clock (psD is never read)
        scratch = acts.tile([P, 512], f16)
        nc.gpsimd.memset(scratch[:], 0.0)

        # ones column for the PE-side partition broadcast in normalize
        ones_sb = const.tile([1, P], f16)
        nc.vector.memset(ones_sb[:], 1.0)

        def warmup(nmm):
            psD = psA.tile([P, 512], f32, tag="acc")
            for i in range(nmm):
                nc.tensor.matmul(
                    psD[:], scratch[:, 0:P], scratch[:],
                    start=(i == 0), stop=(i == nmm - 1),
                )

        # Generators yield once per emitted PE matmul so attention blocks can
        # interleave them into PE slack at a controlled rate (the per-engine
        # instruction streams execute strictly in program order).
        def gen_proj_k(a, nns=(0, 1)):
            # trailing copies are emitted BEFORE the final yield so that a
            # fully-popped generator has fully emitted its writes.
            # a=0 runs in fp16 off the early fp16 encoder DMA; a>=1 runs as
            # one-sided-fp8 DoubleRow (half PE cost; only the wk8 side
            # carries fp8 error, which softmax normalization absorbs).
            for nn in nns:
                ps = psA.tile([P, 512], f32, tag="acc")
                for c in range(KC_KV):
                    if a == 0:
                        nc.tensor.matmul(
                            ps[:],
                            wk_sb[:, c, a * P:(a + 1) * P],
                            encT_sb[:, c, nn * 512:(nn + 1) * 512],
                            start=(c == 0),
                            stop=(c == KC_KV - 1),
                        )
                    else:
                        nc.tensor.matmul(
                            ps[:],
                            wk8_sb[:, c, a * P:(a + 1) * P]
                            .unsqueeze(1).broadcast_to((P, 2, P)),
                            encT8_sb[:, c, :, nn * 512:(nn + 1) * 512],
                            start=(c == 0),
                            stop=(c == KC_KV - 1),
                            perf_mode=DR,
                        )
                    if c < KC_KV - 1:
                        yield
                with nc.allow_low_precision(reason="k stored fp8 for DR scores"):
                    if a == 0 and nn == 0:
                        # split so the first two scores matmuls (kv chunks
                        # 0-1) are gated by a half-copy, not the full 512
                        nc.vector.tensor_copy(kT8[:, 0, 0:256], ps[:, 0:256])
                        nc.vector.tensor_copy(
                            kT8[:, 0, 256:512], ps[:, 256:512]
                        )
                    else:
                        nc.vector.tensor_copy(
                            kT8[:, a, nn * 512:(nn + 1) * 512], ps[:]
                        )
                yield

        def gen_proj_q(a, n):
            ps = psA.tile([P, 512], f32, tag="acc")
            for c in range(KC_Q):
                nc.tensor.matmul(
                    ps[:],
                    wq_sb[:, c, a * P:(a + 1) * P],
                    hsT_sb[:, c, n * 512:(n + 1) * 512],
                    start=(c == 0),
                    stop=(c == KC_Q - 1),
                )
                if c < KC_Q - 1:
                    yield
            sl = slice(n * 512, (n + 1) * 512)
            with nc.allow_low_precision(reason="q stored as fp8 hi/lo pair"):
                nc.vector.tensor_copy(qT8[:, a, 0, sl], ps[:])
                nc.vector.tensor_sub(qT8[:, a, 1, sl], ps[:], qT8[:, a, 0, sl])
            yield

        # v natural [kv, slots]: slot h (128 wide):
        #   h even: [v_h (0:64) | 1.0 at 64 | 0 at 65:128]   -> out rows 0:64, denom row 64
        #   h odd : [1.0 at 0 | 0 at 1:64 | v_h at 64:128]   -> out rows 64:128, denom row 0
        def v_memsets():
            nc.gpsimd.memset(vv4[:, :, 0::2, 64:65], 1.0)
            nc.gpsimd.memset(vv4[:, :, 1::2, 0:1], 1.0)
            nc.gpsimd.memset(vv4[:, :, 0::2, 65:P], 0.0)
            nc.gpsimd.memset(vv4[:, :, 1::2, 1:DH], 0.0)

        def gen_proj_v(t):
            ps = psA.tile([P, 512], f32, tag="acc")
            for c in range(KC_KV):
                nc.tensor.matmul(
                    ps[:],
                    encT_sb[:, c, t * P:(t + 1) * P],
                    wv_sb[:, c, :],
                    start=(c == 0),
                    stop=(c == KC_KV - 1),
                )
                if c < KC_KV - 1:
                    yield
            pv = ps.rearrange("p (s c) -> p s c", c=DH)
            nc.vector.tensor_copy(vv4[:, t, 0::2, 0:DH], pv[:, 0::2, :])
            nc.vector.tensor_copy(vv4[:, t, 1::2, DH:P], pv[:, 1::2, :])
            yield

        def gen_final(m):
            ps = psA.tile([P, 512], f32, tag="acc")
            for a in range(AT):
                nc.tensor.matmul(
                    ps[:],
                    outT_sb[:, a, m * P:(m + 1) * P],
                    wo_sb[:, a, :],
                    start=(a == 0),
                    stop=(a == AT - 1),
                )
                if a < AT - 1:
                    yield
            ob = outp.tile([P, QD], f32)
            nc.vector.tensor_add(ob[:], ps[:], bo_sb[:])
            nc.sync.dma_start(out_d[m * P:(m + 1) * P, :], ob[:])
            yield

        # final projection split for the tail m-tiles: partA (heads 0-1)
        # accumulates into an SBUF staging tile during earlier blocks; partB
        # (heads 2-3) only trails the last attention block
        facc = acts.tile([P, QSH // P, QD], f32)

        def gen_final_a(m):
            ps = psA.tile([P, 512], f32, tag="acc")
            for a in (0, 1):
                nc.tensor.matmul(
                    ps[:],
                    outT_sb[:, a, m * P:(m + 1) * P],
                    wo_sb[:, a, :],
                    start=(a == 0),
                    stop=(a == 1),
                )
                if a == 0:
                    yield
            nc.vector.tensor_add(facc[:, m, :], ps[:], bo_sb[:])
            yield

        def gen_final_b(m):
            ps = psA.tile([P, 512], f32, tag="acc")
            for a in (2, 3):
                nc.tensor.matmul(
                    ps[:],
                    outT_sb[:, a, m * P:(m + 1) * P],
                    wo_sb[:, a, :],
                    start=(a == 2),
                    stop=(a == 3),
                )
                if a == 2:
                    yield
            ob = outp.tile([P, QD], f32)
            nc.vector.tensor_add(ob[:], ps[:], facc[:, m, :])
            nc.sync.dma_start(out_d[m * P:(m + 1) * P, :], ob[:])
            yield

        # split final-b for the first two tail m-tiles: the a=2 matmul only
        # needs head group 2 (written by block (2,1)'s tail), so it runs as a
        # block-(3,1) extra; only the a=3 matmul + add + DMA trail the last
        # normalize.  At most TWO may be pre-opened: their live PSUM tiles
        # plus the two tail rcps tiles exactly fill the 4-slot acc rotation.
        fb_ps = {}

        def gen_fb_a2(m):
            ps = psA.tile([P, 512], f32, tag="acc", name="fbps")
            nc.tensor.matmul(
                ps[:], outT_sb[:, 2, m * P:(m + 1) * P], wo_sb[:, 2, :],
                start=True, stop=False,
            )
            fb_ps[m] = ps
            yield

        def fb_a3(m):
            ps = fb_ps[m]
            nc.tensor.matmul(
                ps[:], outT_sb[:, 3, m * P:(m + 1) * P], wo_sb[:, 3, :],
                start=False, stop=True,
            )
            ob = outp.tile([P, QD], f32)
            nc.vector.tensor_add(ob[:], ps[:], facc[:, m, :])
            nc.sync.dma_start(out_d[m * P:(m + 1) * P, :], ob[:])

        def gen_chain(*gens):
            for g in gens:
                yield from g

        def run_gen(g):
            for _ in g:
                pass

        def attn(hp, n, extras=None, pre_pop=0, per_iter=0, prev_tail=None,
                 drain=True, act_copy_norm=False, split_tail_exp=False,
                 tail_mid=None):
            """Emit one attention block.  Returns a closure that emits the
            block's last two AV matmuls + normalize; the caller passes it to
            the NEXT block so those trail instructions interleave with the
            next block's leading scores (removes the block-boundary bubble).
            """
            if extras is None:
                extras = iter(())

            def pop(k):
                for _ in range(k):
                    if next(extras, StopIteration) is StopIteration:
                        break

            av0 = psA.tile([P, 512], f32, tag="acc")
            av1 = psA.tile([P, 512], f32, tag="acc")
            av = (av0, av1)
            exs = []

            def s_(t):
                ss = psS.tile([P, 1024], f32)
                for i in range(2):
                    pr = slice(i * 64, (i + 1) * 64)
                    nc.tensor.matmul(
                        ss[:, i * 512:(i + 1) * 512],
                        kT8[pr, hp, t * P:(t + 1) * P]
                        .unsqueeze(1).broadcast_to((64, 2, P)),
                        qT8[pr, hp, :, n * 512:(n + 1) * 512],
                        start=True,
                        stop=True,
                        perf_mode=DR,
                    )
                if split_tail_exp and t == NT - 1:
                    # last exp of the LAST block split per head: the final AV
                    # for head-even (and so the whole normalize/final-b tail
                    # chain) starts one half-exp earlier.
                    exa = expp.tile([P, 512], f16, tag="exh", name="exa")
                    exb = expp.tile([P, 512], f16, tag="exh", name="exb")
                    nc.scalar.activation(
                        exa[:], ss[:, 0:512], Exp,
                        scale=SCALE / (WSCALE * WSCALE_K),
                    )
                    nc.scalar.activation(
                        exb[:], ss[:, 512:1024], Exp,
                        scale=SCALE / (WSCALE * WSCALE_K),
                    )
                    exs.append((exa, exb))
                    return
                ex = expp.tile([P, 1024], f16)
                nc.scalar.activation(
                    ex[:], ss[:], Exp, scale=SCALE / (WSCALE * WSCALE_K)
                )
                exs.append(ex)

            def A_(t):
                for i in range(2):
                    s = 2 * hp + i
                    e = exs[t]
                    rhs = e[i][:, :] if isinstance(e, tuple) \
                        else e[:, i * 512:(i + 1) * 512]
                    nc.tensor.matmul(
                        av[i][:],
                        v_sb[:, t, s * P:(s + 1) * P],
                        rhs,
                        start=(t == 0),
                        stop=(t == NT - 1),
                    )

            s_(0)
            s_(1)
            pop(pre_pop)
            if prev_tail is not None:
                prev_tail()
            for t in range(2, NT):
                s_(t)
                A_(t - 2)
                pop(per_iter)
            if drain:  # drain leftovers so every generator completes
                for _ in extras:
                    pass

            def tail():
                A_(NT - 2)
                A_(NT - 1)
                # partition broadcast of 1/denom via a PE ones-column matmul
                # (GpSimd partition_broadcast proved flaky on HW; the DMA
                # round-trip costs ~4us per block).  The reciprocal lands on
                # partition 0 in f16, ones.T @ recip fills a PSUM tile,
                # which is copied to SBUF for the multiply (ScalarE for the
                # last block where it is idle, DVE elsewhere).
                rcbs = []
                for i in range(2):
                    drow = 64 if i == 0 else 0
                    rc = small.tile([1, 512], f16, tag="rc")
                    with nc.allow_low_precision(
                        reason="softmax denom reciprocal, f16 suffices"
                    ):
                        nc.vector.reciprocal(
                            rc[0:1, :], av[i][drow:drow + 1, :]
                        )
                    rcps = psA.tile([P, 512], f32, tag="acc")
                    nc.tensor.matmul(
                        rcps[:], ones_sb[0:1, :], rc[0:1, :],
                        start=True, stop=True,
                    )
                    rcb = small.tile([P, 512], f32, tag="rcb")
                    if act_copy_norm:
                        # last block: ScalarE staging copies in column halves
                        # so the first multiplies (and the final-b m-tiles
                        # they gate) start after half the copy latency
                        nc.scalar.copy(rcb[:, 0:256], rcps[:, 0:256])
                        nc.scalar.copy(rcb[:, 256:512], rcps[:, 256:512])
                    else:
                        nc.vector.tensor_copy(rcb[:], rcps[:])
                    rcbs.append(rcb)
                dsts = (slice(0, 64), slice(64, 128))
                if act_copy_norm:
                    # column-half multiplies with the first two final-b
                    # m-tiles emitted in between: their matmuls/adds/DMAs
                    # only need outT cols 0:256, so the serial out-DMA chain
                    # starts while the second halves still compute
                    for cols in (slice(0, 256), slice(256, 512)):
                        for i in range(2):
                            nc.vector.tensor_mul(
                                outT_sb[dsts[i], hp,
                                        n * 512 + cols.start:
                                        n * 512 + cols.stop],
                                av[i][dsts[i], cols],
                                rcbs[i][dsts[i], cols],
                            )
                        if cols.start == 0 and tail_mid is not None:
                            tail_mid()
                else:
                    for i in range(2):
                        nc.vector.tensor_mul(
                            outT_sb[dsts[i], hp, n * 512:(n + 1) * 512],
                            av[i][dsts[i], :],
                            rcbs[i][dsts[i], :],
                        )

            return tail

        # ---- emission = per-engine execution order.  Warmup dummies bridge
        # the DMA head up to qT(0,0); kT(0) kv-half 0 slots into the gap as
        # soon as its DMA lands; everything else (v, kT second half, later
        # k/q projections, finals) interleaves into attention-block PE slack.
        v_memsets()
        warmup(CFG["W1"])
        run_gen(gen_proj_q(0, 0))
        run_gen(gen_proj_q(1, 0))
        run_gen(gen_proj_k(0, nns=(0,)))
        run_gen(gen_proj_q(2, 0))
        run_gen(gen_proj_q(3, 0))
        tail = attn(
            0, 0,
            extras=gen_chain(
                gen_proj_v(0), gen_proj_v(1),
                gen_proj_k(0, nns=(1,)),
                *[gen_proj_v(t) for t in range(2, NT)],
                gen_proj_k(1, nns=(0,)),
            ),
            pre_pop=CFG["B0_PRE"], per_iter=CFG["B0_PER"],
        )
        tail = attn(1, 0, extras=gen_chain(gen_proj_k(1, nns=(1,)),
                                           gen_proj_q(0, 1),
                                           gen_proj_k(2), gen_proj_q(1, 1)),
                    pre_pop=CFG["BK_PRE"], per_iter=CFG["BK_PER"],
                    prev_tail=tail)
        tail = attn(2, 0, extras=gen_chain(gen_proj_k(3), gen_proj_q(2, 1)),
                    pre_pop=CFG["BK_PRE"], per_iter=CFG["BK_PER"],
                    prev_tail=tail)
        tail = attn(3, 0, extras=gen_proj_q(3, 1), pre_pop=0, per_iter=1,
                    prev_tail=tail)
        f01 = gen_chain(gen_final(0), gen_final(1))
        tail = attn(0, 1, extras=f01, pre_pop=0, per_iter=1,
                    prev_tail=tail, drain=False)
        tail = attn(1, 1, extras=gen_chain(f01, gen_final(2)),
                    pre_pop=0, per_iter=1, prev_tail=tail)
        tail = attn(2, 1,
                    extras=gen_chain(gen_final(3), gen_final_a(4)),
                    pre_pop=0, per_iter=CFG["B21_PER"], prev_tail=tail)
        tail = attn(3, 1, act_copy_norm=True, split_tail_exp=True,
                    tail_mid=lambda: (run_gen(gen_final_b(4)),
                                      run_gen(gen_final_b(5))),
                    extras=gen_chain(gen_final_a(5), gen_final_a(6),
                                     gen_final_a(7)),
                    pre_pop=0, per_iter=CFG["B31_PER"], prev_tail=tail)
        tail()
        for m in (6, 7):
            run_gen(gen_final_b(m))

    nc.finalize()
    return nc


def _get_nc():
    if "nc" not in _CACHE:
        _CACHE["nc"] = _build_nc()
    return _CACHE["nc"]


def make_in_maps(hidden_states, encoder_hidden_states, Wq, Wk, Wv, Wo, bo):
    f16 = np.float16
    hs = np.asarray(hidden_states, dtype=np.float32)
    enc = np.asarray(encoder_hidden_states, dtype=np.float32)
    import ml_dtypes

    f8 = ml_dtypes.float8_e4m3
    wq = np.ascontiguousarray(np.asarray(Wq, dtype=np.float32) * WSCALE).astype(f16)
    wk32 = np.asarray(Wk, dtype=np.float32) * WSCALE_K
    wk = np.ascontiguousarray(wk32[:, 0:P]).astype(f16)
    wk8 = np.ascontiguousarray(wk32).astype(f8)
    wv = np.ascontiguousarray(np.asarray(Wv, dtype=np.float32)).astype(f16)
    wo = np.ascontiguousarray(np.asarray(Wo, dtype=np.float32)).astype(f16)
    bo_ = np.ascontiguousarray(np.asarray(bo, dtype=np.float32)).reshape(1, QD)
    encT = [np.ascontiguousarray(enc[b].T).astype(f16) for b in range(B)]
    encT8 = []
    for b in range(B):
        e = enc[b].T.astype(np.float32)
        hi = e.astype(f8)
        lo = (e - hi.astype(np.float32)).astype(f8)
        encT8.append(np.ascontiguousarray(np.stack([hi, lo], axis=1)))
    in_maps = []
    for c in range(NCORES):
        b, s = divmod(c, 4)
        hsT = np.ascontiguousarray(hs[b, s * QSH:(s + 1) * QSH, :].T).astype(f16)
        in_maps.append(
            dict(hsT=hsT, encT=encT[b], wq=wq, wk=wk, wv=wv, wo=wo, bo=bo_,
                 wk8=wk8, encT8=encT8[b])
        )
    return in_maps


def kernel(hidden_states, encoder_hidden_states, Wq, Wk, Wv, Wo, bo):
    global LAST_RESULTS
    from concourse.bass_utils import run_bass_kernel_spmd

    nc = _get_nc()
    in_maps = make_in_maps(
        hidden_states, encoder_hidden_states, Wq, Wk, Wv, Wo, bo
    )
    res = run_bass_kernel_spmd(nc, in_maps, core_ids=list(range(NCORES)))
    LAST_RESULTS = res
    out = np.empty((B, LQ, QD), dtype=np.float32)
    for c in range(NCORES):
        b, s = divmod(c, 4)
        out[b, s * QSH:(s + 1) * QSH, :] = res.results[c]["out"]
    return out



# revision 72
# speedup vs baseline: 1.0637x; 1.0032x over previous
"""CrossAttention kernel for 8 TRN2 NeuronCores.

Reference computation (B=2, Lq=4096, Lkv=1024, query_dim=512, cross_dim=768,
heads=8, dim_head=64, inner=512):
    q = hs @ Wq; k = enc @ Wk; v = enc @ Wv          (per batch)
    attn = softmax(q_h @ k_h^T * scale) per head
    out = concat_h(attn @ v_h) @ Wo + bo

Sharding: 8 cores = 2 batches x 4 query-slices of 1024 queries.  Each core
computes its full slice of the output (all heads), so outputs are disjoint
and no collective is needed.

Per-core dataflow (fp16 operands, fp32 PSUM accumulate, with the scores
pipeline in fp8 DoubleRow — the cost model charges DR matmuls 0.5
cycles/row, so these run at half the PE cost):
  - host passes hs-slice and encoder transposed (hsT [512,1024], encT
    [768,1024]) in fp16, plus fp8 copies for the DoubleRow paths: an
    encoder (hi, lo) fp8 plane pair and wk8 = fp8(Wk*32).  Wq is
    pre-scaled x8 and Wk x32 to center fp8 dynamic range; exp() absorbs
    the 1/256.
  - qT = Wq^T-weighted hsT -> [inner=512, q=1024] (heads along
    partitions), stored as an fp8 (hi, lo) pair (DVE copy + subtract)
  - kT likewise -> [inner=512, kv=1024], stored as single-plane fp8.
    Head group a=0 projects in fp16 off the early encoder DMA; groups
    a=1..3 project via one-sided-fp8 DoubleRow: stationary wk8 rides both
    planes through a stride-0 broadcast AP, the moving operand is the
    encoder (hi, lo) pair, so only the wk8 side carries fp8 error.
  - v natural -> [kv=1024, slots] fp16; slot h = 128 cols holding v_h
    (64) + a ones column + zero padding, so the AV output lands
    partition-aligned and the softmax denominator falls out of the same
    matmul.
  - scoresT_h = k_h qT_h via ONE DoubleRow matmul per head: stationary
    (k8, k8) via stride-0 broadcast, moving (q_hi, q_lo), giving
    sum_j k8*(q_hi+q_lo) = k8 * q at fp8-pair precision.  Only the
    k-side carries fp8 error, which softmax normalization largely
    absorbs (q-side fp8 measures ~2x worse; both-sides fails the gate).
  - expT = exp(scale/256 * scoresT) on ScalarE, fp16 out (no
    max-subtraction: |scaled scores| < ~3)
  - outT_unnorm_h = v_slot^T @ expT accumulated over kv chunks (PSUM),
    one row of which is the softmax denominator
  - normalize: reciprocal (DVE) + PE ones-column broadcast matmul +
    multiply
  - final = outT^T @ Wo + bo -> [1024, 512], DMA out per 128-row tile
Measured absmax/scale ~1.05e-2 against the fp64 reference (gate 2e-2).

Program order is pipelined for the Tile scheduler: k/v/q projections are
emitted ahead of the attention blocks that consume them, exp(t) is emitted
before AV(t-1) so the PE never waits in-order on ScalarE, and the final
projection m-tiles are interleaved between the last attention blocks.
"""

import sys

if "/opt/trn_rl_repo" not in sys.path:
    sys.path.insert(0, "/opt/trn_rl_repo")

import numpy as np

B, LQ, LKV = 2, 4096, 1024
QD, CD = 512, 768
H, DH = 8, 64
INNER = H * DH  # 512
SCALE = DH ** -0.5
NCORES = 8
WSCALE = 8.0    # host-side pre-scale on Wq (fp8 range centering)
WSCALE_K = 32.0  # host-side pre-scale on Wk (fp8 range centering)
QSH = LQ // 4  # 1024 queries per core
P = 128

_CACHE: dict = {}
LAST_RESULTS = None  # test harness introspection (exec_time_ns etc.)

# schedule-tuning knobs (sweepable from bench tooling)
CFG = {
    "W1": 9,       # warmup matmuls bridging the input-DMA head
    "W2": 4,        # second warmup burst bridging q(0,0) -> k(0) DMA gap
    "B0_PRE": 12,    # block (0,0) pre-loop extra pops
    "B0_PER": 9,    # block (0,0) per-iter extra pops
    "BK_PRE": 2,    # later n=0 blocks pre-loop pops
    "BK_PER": 3,    # later n=0 blocks per-iter pops
    "B21_PER": 0,   # block (2,1) per-iter pops
    "B31_PER": 0,   # block (3,1) per-iter pops
}


def _build_nc():
    from contextlib import ExitStack

    import concourse.bass as bass
    import concourse.tile as tile
    from concourse import bacc, mybir

    f32 = mybir.dt.float32
    f16 = mybir.dt.float16
    f8 = mybir.dt.float8e4
    DR = mybir.MatmulPerfMode.DoubleRow
    Exp = mybir.ActivationFunctionType.Exp

    nc = bacc.Bacc(trn_type="TRN2")

    hsT_d = nc.declare_dram_parameter("hsT", [QD, QSH], f16, isOutput=False)
    encT_d = nc.declare_dram_parameter("encT", [CD, LKV], f16, isOutput=False)
    wq_d = nc.declare_dram_parameter("wq", [QD, INNER], f16, isOutput=False)
    wk_d = nc.declare_dram_parameter("wk", [CD, P], f16, isOutput=False)
    wv_d = nc.declare_dram_parameter("wv", [CD, INNER], f16, isOutput=False)
    wo_d = nc.declare_dram_parameter("wo", [INNER, QD], f16, isOutput=False)
    bo_d = nc.declare_dram_parameter("bo", [1, QD], f32, isOutput=False)
    wk8_d = nc.declare_dram_parameter("wk8", [CD, INNER], f8, isOutput=False)
    encT8_d = nc.declare_dram_parameter("encT8", [CD, 2, LKV], f8, isOutput=False)
    out_d = nc.declare_dram_parameter("out", [QSH, QD], f32, isOutput=True)

    KC_Q = QD // P   # 4 contraction chunks for q projection
    KC_KV = CD // P  # 6 for k/v projections
    AT = INNER // P  # 4 inner tiles (2 heads each)
    NT = LKV // P    # 8 kv chunks
    QN = QSH // 512  # 2 q slices of 512

    with ExitStack() as ctx:
        tc = ctx.enter_context(tile.TileContext(nc))
        const = ctx.enter_context(tc.tile_pool(name="const", bufs=1))
        acts = ctx.enter_context(tc.tile_pool(name="acts", bufs=1))
        expp = ctx.enter_context(tc.tile_pool(name="expp", bufs=8))
        outp = ctx.enter_context(tc.tile_pool(name="outp", bufs=6))
        small = ctx.enter_context(tc.tile_pool(name="small", bufs=8))
        psA = ctx.enter_context(tc.tile_pool(name="psA", bufs=4, space="PSUM"))
        psS = ctx.enter_context(tc.tile_pool(name="psS", bufs=2, space="PSUM"))
        drp = ctx.enter_context(tc.tile_pool(name="drp", bufs=4, space="DRAM"))

        # ---- input DMA, ordered by first use: the q projections (hsT+wq)
        # run during the PE warmup window, then kT (encT+wk), then v (wv);
        # the second encT half only gates scores t>=4 of the first block
        hsT_sb = acts.tile([P, KC_Q, QSH], f16)
        hsT_r = hsT_d.rearrange("(c p) n -> p c n", p=P)
        nc.sync.dma_start(hsT_sb[:, :, 0:512], hsT_r[:, :, 0:512])
        wq_sb = const.tile([P, KC_Q, INNER], f16)
        nc.sync.dma_start(wq_sb[:], wq_d.rearrange("(c p) n -> p c n", p=P))
        encT_sb = acts.tile([P, KC_KV, LKV], f16)
        encT_r = encT_d.rearrange("(c p) n -> p c n", p=P)
        wk_sb = const.tile([P, KC_KV, P], f16)
        nc.sync.dma_start(wk_sb[:], wk_d.rearrange("(c p) n -> p c n", p=P))
        nc.sync.dma_start(encT_sb[:, 0:2, 0:512], encT_r[:, 0:2, 0:512])
        nc.sync.dma_start(encT_sb[:, 2:4, 0:512], encT_r[:, 2:4, 0:512])
        nc.sync.dma_start(encT_sb[:, 4:6, 0:512], encT_r[:, 4:6, 0:512])
        wv_sb = const.tile([P, KC_KV, INNER], f16)
        nc.sync.dma_start(wv_sb[:], wv_d.rearrange("(c p) n -> p c n", p=P))
        nc.sync.dma_start(encT_sb[:, :, 512:1024], encT_r[:, :, 512:1024])
        nc.sync.dma_start(hsT_sb[:, :, 512:1024], hsT_r[:, :, 512:1024])
        # fp8 operands for the DoubleRow k projections (k heads 2..7): the
        # encoder comes as an fp8 (hi, lo) plane pair; wk8 rides both planes
        # of the stationary operand via a stride-0 broadcast
        encT8_sb = acts.tile([P, KC_KV, 2, LKV], f8)
        encT8_r = encT8_d.rearrange("(c p) j n -> p c j n", p=P)
        wk8_sb = const.tile([P, KC_KV, INNER], f8)
        nc.sync.dma_start(encT8_sb[:, :, 0, 0:512], encT8_r[:, :, 0, 0:512])
        nc.sync.dma_start(encT8_sb[:, :, 1, 0:512], encT8_r[:, :, 1, 0:512])
        nc.sync.dma_start(wk8_sb[:], wk8_d.rearrange("(c p) n -> p c n", p=P))
        nc.sync.dma_start(encT8_sb[:, :, 0, 512:1024], encT8_r[:, :, 0, 512:1024])
        nc.sync.dma_start(encT8_sb[:, :, 1, 512:1024], encT8_r[:, :, 1, 512:1024])
        wo_sb = const.tile([P, AT, QD], f16)
        nc.sync.dma_start(wo_sb[:], wo_d.rearrange("(c p) n -> p c n", p=P))
        bo_sb = const.tile([P, QD], f32)
        nc.sync.dma_start(bo_sb[:], bo_d.ap().to_broadcast((P, QD)))

        qT8 = acts.tile([P, AT, 2, QSH], f8)       # planes: (hi, lo)
        kT8 = acts.tile([P, AT, LKV], f8)          # single plane, broadcast in DR
        v_sb = acts.tile([P, NT, H * P], f16)
        outT_sb = acts.tile([P, AT, QSH], f16)
        vv4 = v_sb.rearrange("p t (s c) -> p t s c", c=P)

        # ---- PE warmup: dummy matmuls on zeroed scratch fill the DMA head
        # so the first real matmuls run at full clock (psD is never read)
        scratch = acts.tile([P, 512], f16)
        nc.gpsimd.memset(scratch[:], 0.0)

        # ones column for the PE-side partition broadcast in normalize
        ones_sb = const.tile([1, P], f16)
        nc.vector.memset(ones_sb[:], 1.0)

        def warmup(nmm):
            psD = psA.tile([P, 512], f32, tag="acc")
            for i in range(nmm):
                nc.tensor.matmul(
                    psD[:], scratch[:, 0:P], scratch[:],
                    start=(i == 0), stop=(i == nmm - 1),
                )

        # Generators yield once per emitted PE matmul so attention blocks can
        # interleave them into PE slack at a controlled rate (the per-engine
        # instruction streams execute strictly in program order).
        def gen_proj_k(a, nns=(0, 1)):
            # trailing copies are emitted BEFORE the final yield so that a
            # fully-popped generator has fully emitted its writes.
            # a=0 runs in fp16 off the early fp16 encoder DMA; a>=1 runs as
            # one-sided-fp8 DoubleRow (half PE cost; only the wk8 side
            # carries fp8 error, which softmax normalization absorbs).
            for nn in nns:
                ps = psA.tile([P, 512], f32, tag="acc")
                for c in range(KC_KV):
                    if a == 0:
                        nc.tensor.matmul(
                            ps[:],
                            wk_sb[:, c, a * P:(a + 1) * P],
                            encT_sb[:, c, nn * 512:(nn + 1) * 512],
                            start=(c == 0),
                            stop=(c == KC_KV - 1),
                        )
                    else:
                        nc.tensor.matmul(
                            ps[:],
                            wk8_sb[:, c, a * P:(a + 1) * P]
                            .unsqueeze(1).broadcast_to((P, 2, P)),
                            encT8_sb[:, c, :, nn * 512:(nn + 1) * 512],
                            start=(c == 0),
                            stop=(c == KC_KV - 1),
                            perf_mode=DR,
                        )
                    if c < KC_KV - 1:
                        yield
                with nc.allow_low_precision(reason="k stored fp8 for DR scores"):
                    if a == 0 and nn == 0:
                        # split so the first two scores matmuls (kv chunks
                        # 0-1) are gated by a half-copy, not the full 512
                        nc.vector.tensor_copy(kT8[:, 0, 0:256], ps[:, 0:256])
                        nc.vector.tensor_copy(
                            kT8[:, 0, 256:512], ps[:, 256:512]
                        )
                    else:
                        nc.vector.tensor_copy(
                            kT8[:, a, nn * 512:(nn + 1) * 512], ps[:]
                        )
                yield

        def gen_proj_q(a, n):
            ps = psA.tile([P, 512], f32, tag="acc")
            for c in range(KC_Q):
                nc.tensor.matmul(
                    ps[:],
                    wq_sb[:, c, a * P:(a + 1) * P],
                    hsT_sb[:, c, n * 512:(n + 1) * 512],
                    start=(c == 0),
                    stop=(c == KC_Q - 1),
                )
                if c < KC_Q - 1:
                    yield
            sl = slice(n * 512, (n + 1) * 512)
            with nc.allow_low_precision(reason="q stored as fp8 hi/lo pair"):
                nc.vector.tensor_copy(qT8[:, a, 0, sl], ps[:])
                nc.vector.tensor_sub(qT8[:, a, 1, sl], ps[:], qT8[:, a, 0, sl])
            yield

        # v natural [kv, slots]: slot h (128 wide):
        #   h even: [v_h (0:64) | 1.0 at 64 | 0 at 65:128]   -> out rows 0:64, denom row 64
        #   h odd : [1.0 at 0 | 0 at 1:64 | v_h at 64:128]   -> out rows 64:128, denom row 0
        def v_memsets():
            nc.gpsimd.memset(vv4[:, :, 0::2, 64:65], 1.0)
            nc.gpsimd.memset(vv4[:, :, 1::2, 0:1], 1.0)
            nc.gpsimd.memset(vv4[:, :, 0::2, 65:P], 0.0)
            nc.gpsimd.memset(vv4[:, :, 1::2, 1:DH], 0.0)

        def gen_proj_v(t):
            ps = psA.tile([P, 512], f32, tag="acc")
            for c in range(KC_KV):
                nc.tensor.matmul(
                    ps[:],
                    encT_sb[:, c, t * P:(t + 1) * P],
                    wv_sb[:, c, :],
                    start=(c == 0),
                    stop=(c == KC_KV - 1),
                )
                if c < KC_KV - 1:
                    yield
            pv = ps.rearrange("p (s c) -> p s c", c=DH)
            nc.vector.tensor_copy(vv4[:, t, 0::2, 0:DH], pv[:, 0::2, :])
            nc.vector.tensor_copy(vv4[:, t, 1::2, DH:P], pv[:, 1::2, :])
            yield

        def gen_final(m):
            ps = psA.tile([P, 512], f32, tag="acc")
            for a in range(AT):
                nc.tensor.matmul(
                    ps[:],
                    outT_sb[:, a, m * P:(m + 1) * P],
                    wo_sb[:, a, :],
                    start=(a == 0),
                    stop=(a == AT - 1),
                )
                if a < AT - 1:
                    yield
            ob = outp.tile([P, QD], f32)
            nc.vector.tensor_add(ob[:], ps[:], bo_sb[:])
            nc.sync.dma_start(out_d[m * P:(m + 1) * P, :], ob[:])
            yield

        # final projection split for the tail m-tiles: partA (heads 0-1)
        # accumulates into an SBUF staging tile during earlier blocks; partB
        # (heads 2-3) only trails the last attention block
        facc = acts.tile([P, QSH // P, QD], f32)

        def gen_final_a(m):
            ps = psA.tile([P, 512], f32, tag="acc")
            for a in (0, 1):
                nc.tensor.matmul(
                    ps[:],
                    outT_sb[:, a, m * P:(m + 1) * P],
                    wo_sb[:, a, :],
                    start=(a == 0),
                    stop=(a == 1),
                )
                if a == 0:
                    yield
            nc.vector.tensor_add(facc[:, m, :], ps[:], bo_sb[:])
            yield

        def gen_final_b(m):
            ps = psA.tile([P, 512], f32, tag="acc")
            for a in (2, 3):
                nc.tensor.matmul(
                    ps[:],
                    outT_sb[:, a, m * P:(m + 1) * P],
                    wo_sb[:, a, :],
                    start=(a == 2),
                    stop=(a == 3),
                )
                if a == 2:
                    yield
            ob = outp.tile([P, QD], f32)
            nc.vector.tensor_add(ob[:], ps[:], facc[:, m, :])
            nc.sync.dma_start(out_d[m * P:(m + 1) * P, :], ob[:])
            yield

        # split final-b for the first two tail m-tiles: the a=2 matmul only
        # needs head group 2 (written by block (2,1)'s tail), so it runs as a
        # block-(3,1) extra; only the a=3 matmul + add + DMA trail the last
        # normalize.  At most TWO may be pre-opened: their live PSUM tiles
        # plus the two tail rcps tiles exactly fill the 4-slot acc rotation.
        fb_ps = {}

        def gen_fb_a2(m):
            ps = psA.tile([P, 512], f32, tag="acc", name="fbps")
            nc.tensor.matmul(
                ps[:], outT_sb[:, 2, m * P:(m + 1) * P], wo_sb[:, 2, :],
                start=True, stop=False,
            )
            fb_ps[m] = ps
            yield

        def fb_a3(m):
            ps = fb_ps[m]
            nc.tensor.matmul(
                ps[:], outT_sb[:, 3, m * P:(m + 1) * P], wo_sb[:, 3, :],
                start=False, stop=True,
            )
            ob = outp.tile([P, QD], f32)
            nc.vector.tensor_add(ob[:], ps[:], facc[:, m, :])
            nc.sync.dma_start(out_d[m * P:(m + 1) * P, :], ob[:])

        def gen_chain(*gens):
            for g in gens:
                yield from g

        def run_gen(g):
            for _ in g:
                pass

        def attn(hp, n, extras=None, pre_pop=0, per_iter=0, prev_tail=None,
                 drain=True, act_copy_norm=False, split_exp_ts=(),
                 tail_mid=None):
            """Emit one attention block.  Returns a closure that emits the
            block's last two AV matmuls + normalize; the caller passes it to
            the NEXT block so those trail instructions interleave with the
            next block's leading scores (removes the block-boundary bubble).
            """
            if extras is None:
                extras = iter(())

            def pop(k):
                for _ in range(k):
                    if next(extras, StopIteration) is StopIteration:
                        break

            av0 = psA.tile([P, 512], f32, tag="acc")
            av1 = psA.tile([P, 512], f32, tag="acc")
            av = (av0, av1)
            exs = []

            def s_(t):
                ss = psS.tile([P, 1024], f32)
                for i in range(2):
                    pr = slice(i * 64, (i + 1) * 64)
                    nc.tensor.matmul(
                        ss[:, i * 512:(i + 1) * 512],
                        kT8[pr, hp, t * P:(t + 1) * P]
                        .unsqueeze(1).broadcast_to((64, 2, P)),
                        qT8[pr, hp, :, n * 512:(n + 1) * 512],
                        start=True,
                        stop=True,
                        perf_mode=DR,
                    )
                if t in split_exp_ts:
                    # last exp of the LAST block split per head: the final AV
                    # for head-even (and so the whole normalize/final-b tail
                    # chain) starts one half-exp earlier.
                    exa = expp.tile([P, 512], f16, tag="exh", name="exa")
                    exb = expp.tile([P, 512], f16, tag="exh", name="exb")
                    nc.scalar.activation(
                        exa[:], ss[:, 0:512], Exp,
                        scale=SCALE / (WSCALE * WSCALE_K),
                    )
                    nc.scalar.activation(
                        exb[:], ss[:, 512:1024], Exp,
                        scale=SCALE / (WSCALE * WSCALE_K),
                    )
                    exs.append((exa, exb))
                    return
                ex = expp.tile([P, 1024], f16)
                nc.scalar.activation(
                    ex[:], ss[:], Exp, scale=SCALE / (WSCALE * WSCALE_K)
                )
                exs.append(ex)

            def A_(t):
                for i in range(2):
                    s = 2 * hp + i
                    e = exs[t]
                    rhs = e[i][:, :] if isinstance(e, tuple) \
                        else e[:, i * 512:(i + 1) * 512]
                    nc.tensor.matmul(
                        av[i][:],
                        v_sb[:, t, s * P:(s + 1) * P],
                        rhs,
                        start=(t == 0),
                        stop=(t == NT - 1),
                    )

            s_(0)
            s_(1)
            pop(pre_pop)
            if prev_tail is not None:
                prev_tail()
            for t in range(2, NT):
                s_(t)
                A_(t - 2)
                pop(per_iter)
            if drain:  # drain leftovers so every generator completes
                for _ in extras:
                    pass

            def tail():
                A_(NT - 2)
                A_(NT - 1)
                # partition broadcast of 1/denom via a PE ones-column matmul
                # (GpSimd partition_broadcast proved flaky on HW; the DMA
                # round-trip costs ~4us per block).  The reciprocal lands on
                # partition 0 in f16, ones.T @ recip fills a PSUM tile,
                # which is copied to SBUF for the multiply (ScalarE for the
                # last block where it is idle, DVE elsewhere).
                rcbs = []
                for i in range(2):
                    drow = 64 if i == 0 else 0
                    rc = small.tile([1, 512], f16, tag="rc")
                    with nc.allow_low_precision(
                        reason="softmax denom reciprocal, f16 suffices"
                    ):
                        nc.vector.reciprocal(
                            rc[0:1, :], av[i][drow:drow + 1, :]
                        )
                    rcps = psA.tile([P, 512], f32, tag="acc")
                    nc.tensor.matmul(
                        rcps[:], ones_sb[0:1, :], rc[0:1, :],
                        start=True, stop=True,
                    )
                    rcb = small.tile([P, 512], f32, tag="rcb")
                    if act_copy_norm:
                        # last block: ScalarE staging copies in column halves
                        # so the first multiplies (and the final-b m-tiles
                        # they gate) start after half the copy latency
                        nc.scalar.copy(rcb[:, 0:256], rcps[:, 0:256])
                        nc.scalar.copy(rcb[:, 256:512], rcps[:, 256:512])
                    else:
                        nc.vector.tensor_copy(rcb[:], rcps[:])
                    rcbs.append(rcb)
                dsts = (slice(0, 64), slice(64, 128))
                if act_copy_norm:
                    # column-half multiplies with the first two final-b
                    # m-tiles emitted in between: their matmuls/adds/DMAs
                    # only need outT cols 0:256, so the serial out-DMA chain
                    # starts while the second halves still compute
                    for cols in (slice(0, 256), slice(256, 512)):
                        for i in range(2):
                            nc.vector.tensor_mul(
                                outT_sb[dsts[i], hp,
                                        n * 512 + cols.start:
                                        n * 512 + cols.stop],
                                av[i][dsts[i], cols],
                                rcbs[i][dsts[i], cols],
                            )
                        if cols.start == 0 and tail_mid is not None:
                            tail_mid()
                else:
                    for i in range(2):
                        nc.vector.tensor_mul(
                            outT_sb[dsts[i], hp, n * 512:(n + 1) * 512],
                            av[i][dsts[i], :],
                            rcbs[i][dsts[i], :],
                        )

            return tail

        # ---- emission = per-engine execution order.  Warmup dummies bridge
        # the DMA head up to qT(0,0); kT(0) kv-half 0 slots into the gap as
        # soon as its DMA lands; everything else (v, kT second half, later
        # k/q projections, finals) interleaves into attention-block PE slack.
        v_memsets()
        warmup(CFG["W1"])
        run_gen(gen_proj_q(0, 0))
        run_gen(gen_proj_q(1, 0))
        run_gen(gen_proj_k(0, nns=(0,)))
        run_gen(gen_proj_q(2, 0))
        run_gen(gen_proj_q(3, 0))
        tail = attn(
            0, 0, split_exp_ts=(0,),
            extras=gen_chain(
                gen_proj_v(0), gen_proj_v(1),
                gen_proj_k(0, nns=(1,)),
                *[gen_proj_v(t) for t in range(2, NT)],
                gen_proj_k(1, nns=(0,)),
            ),
            pre_pop=CFG["B0_PRE"], per_iter=CFG["B0_PER"],
        )
        tail = attn(1, 0, extras=gen_chain(gen_proj_k(1, nns=(1,)),
                                           gen_proj_q(0, 1),
                                           gen_proj_k(2), gen_proj_q(1, 1)),
                    pre_pop=CFG["BK_PRE"], per_iter=CFG["BK_PER"],
                    prev_tail=tail)
        tail = attn(2, 0, extras=gen_chain(gen_proj_k(3), gen_proj_q(2, 1)),
                    pre_pop=CFG["BK_PRE"], per_iter=CFG["BK_PER"],
                    prev_tail=tail)
        tail = attn(3, 0, extras=gen_proj_q(3, 1), pre_pop=0, per_iter=1,
                    prev_tail=tail)
        f01 = gen_chain(gen_final(0), gen_final(1))
        tail = attn(0, 1, extras=f01, pre_pop=0, per_iter=1,
                    prev_tail=tail, drain=False)
        tail = attn(1, 1, extras=gen_chain(f01, gen_final(2)),
                    pre_pop=0, per_iter=1, prev_tail=tail)
        tail = attn(2, 1,
                    extras=gen_chain(gen_final(3), gen_final_a(4)),
                    pre_pop=0, per_iter=CFG["B21_PER"], prev_tail=tail)
        tail = attn(3, 1, act_copy_norm=True, split_exp_ts=(NT - 1,),
                    tail_mid=lambda: (run_gen(gen_final_b(4)),
                                      run_gen(gen_final_b(5))),
                    extras=gen_chain(gen_final_a(5), gen_final_a(6),
                                     gen_final_a(7)),
                    pre_pop=0, per_iter=CFG["B31_PER"], prev_tail=tail)
        tail()
        for m in (6, 7):
            run_gen(gen_final_b(m))

    nc.finalize()
    return nc


def _get_nc():
    if "nc" not in _CACHE:
        _CACHE["nc"] = _build_nc()
    return _CACHE["nc"]


def make_in_maps(hidden_states, encoder_hidden_states, Wq, Wk, Wv, Wo, bo):
    f16 = np.float16
    hs = np.asarray(hidden_states, dtype=np.float32)
    enc = np.asarray(encoder_hidden_states, dtype=np.float32)
    import ml_dtypes

    f8 = ml_dtypes.float8_e4m3
    wq = np.ascontiguousarray(np.asarray(Wq, dtype=np.float32) * WSCALE).astype(f16)
    wk32 = np.asarray(Wk, dtype=np.float32) * WSCALE_K
    wk = np.ascontiguousarray(wk32[:, 0:P]).astype(f16)
    wk8 = np.ascontiguousarray(wk32).astype(f8)
    wv = np.ascontiguousarray(np.asarray(Wv, dtype=np.float32)).astype(f16)
    wo = np.ascontiguousarray(np.asarray(Wo, dtype=np.float32)).astype(f16)
    bo_ = np.ascontiguousarray(np.asarray(bo, dtype=np.float32)).reshape(1, QD)
    encT = [np.ascontiguousarray(enc[b].T).astype(f16) for b in range(B)]
    encT8 = []
    for b in range(B):
        e = enc[b].T.astype(np.float32)
        hi = e.astype(f8)
        lo = (e - hi.astype(np.float32)).astype(f8)
        encT8.append(np.ascontiguousarray(np.stack([hi, lo], axis=1)))
    in_maps = []
    for c in range(NCORES):
        b, s = divmod(c, 4)
        hsT = np.ascontiguousarray(hs[b, s * QSH:(s + 1) * QSH, :].T).astype(f16)
        in_maps.append(
            dict(hsT=hsT, encT=encT[b], wq=wq, wk=wk, wv=wv, wo=wo, bo=bo_,
                 wk8=wk8, encT8=encT8[b])
        )
    return in_maps


def kernel(hidden_states, encoder_hidden_states, Wq, Wk, Wv, Wo, bo):
    global LAST_RESULTS
    from concourse.bass_utils import run_bass_kernel_spmd

    nc = _get_nc()
    in_maps = make_in_maps(
        hidden_states, encoder_hidden_states, Wq, Wk, Wv, Wo, bo
    )
    res = run_bass_kernel_spmd(nc, in_maps, core_ids=list(range(NCORES)))
    LAST_RESULTS = res
    out = np.empty((B, LQ, QD), dtype=np.float32)
    for c in range(NCORES):
        b, s = divmod(c, 4)
        out[b, s * QSH:(s + 1) * QSH, :] = res.results[c]["out"]
    return out



# revision 82
# speedup vs baseline: 1.0659x; 1.0021x over previous
"""CrossAttention kernel for 8 TRN2 NeuronCores.

Reference computation (B=2, Lq=4096, Lkv=1024, query_dim=512, cross_dim=768,
heads=8, dim_head=64, inner=512):
    q = hs @ Wq; k = enc @ Wk; v = enc @ Wv          (per batch)
    attn = softmax(q_h @ k_h^T * scale) per head
    out = concat_h(attn @ v_h) @ Wo + bo

Sharding: 8 cores = 2 batches x 4 query-slices of 1024 queries.  Each core
computes its full slice of the output (all heads), so outputs are disjoint
and no collective is needed.

Per-core dataflow (fp16 operands, fp32 PSUM accumulate, with the scores
pipeline in fp8 DoubleRow — the cost model charges DR matmuls 0.5
cycles/row, so these run at half the PE cost):
  - host passes hs-slice and encoder transposed (hsT [512,1024], encT
    [768,1024]) in fp16, plus fp8 copies for the DoubleRow paths: an
    encoder (hi, lo) fp8 plane pair and wk8 = fp8(Wk*32).  Wq is
    pre-scaled x8 and Wk x32 to center fp8 dynamic range; exp() absorbs
    the 1/256.
  - qT = Wq^T-weighted hsT -> [inner=512, q=1024] (heads along
    partitions), stored as an fp8 (hi, lo) pair (DVE copy + subtract)
  - kT likewise -> [inner=512, kv=1024], stored as single-plane fp8.
    Head group a=0 projects in fp16 off the early encoder DMA; groups
    a=1..3 project via one-sided-fp8 DoubleRow: stationary wk8 rides both
    planes through a stride-0 broadcast AP, the moving operand is the
    encoder (hi, lo) pair, so only the wk8 side carries fp8 error.
  - v natural -> [kv=1024, slots] fp16; slot h = 128 cols holding v_h
    (64) + a ones column + zero padding, so the AV output lands
    partition-aligned and the softmax denominator falls out of the same
    matmul.
  - scoresT_h = k_h qT_h via ONE DoubleRow matmul per head: stationary
    (k8, k8) via stride-0 broadcast, moving (q_hi, q_lo), giving
    sum_j k8*(q_hi+q_lo) = k8 * q at fp8-pair precision.  Only the
    k-side carries fp8 error, which softmax normalization largely
    absorbs (q-side fp8 measures ~2x worse; both-sides fails the gate).
  - expT = exp(scale/256 * scoresT) on ScalarE, fp16 out (no
    max-subtraction: |scaled scores| < ~3)
  - outT_unnorm_h = v_slot^T @ expT accumulated over kv chunks (PSUM),
    one row of which is the softmax denominator
  - normalize: reciprocal (DVE) + PE ones-column broadcast matmul +
    multiply
  - final = outT^T @ Wo + bo -> [1024, 512], DMA out per 128-row tile
Measured absmax/scale ~1.05e-2 against the fp64 reference (gate 2e-2).

Program order is pipelined for the Tile scheduler: k/v/q projections are
emitted ahead of the attention blocks that consume them, exp(t) is emitted
before AV(t-1) so the PE never waits in-order on ScalarE, and the final
projection m-tiles are interleaved between the last attention blocks.
"""

import sys

if "/opt/trn_rl_repo" not in sys.path:
    sys.path.insert(0, "/opt/trn_rl_repo")

import numpy as np

B, LQ, LKV = 2, 4096, 1024
QD, CD = 512, 768
H, DH = 8, 64
INNER = H * DH  # 512
SCALE = DH ** -0.5
NCORES = 8
WSCALE = 8.0    # host-side pre-scale on Wq (fp8 range centering)
WSCALE_K = 32.0  # host-side pre-scale on Wk (fp8 range centering)
QSH = LQ // 4  # 1024 queries per core
P = 128

_CACHE: dict = {}
LAST_RESULTS = None  # test harness introspection (exec_time_ns etc.)

# schedule-tuning knobs (sweepable from bench tooling)
CFG = {
    "W1": 9,       # warmup matmuls bridging the input-DMA head
    "W2": 4,        # second warmup burst bridging q(0,0) -> k(0) DMA gap
    "B0_PRE": 14,    # block (0,0) pre-loop extra pops
    "B0_PER": 8,    # block (0,0) per-iter extra pops
    "BK_PRE": 1,    # block (1,0) pre-loop pops
    "BK_PER": 3,    # block (1,0) per-iter pops
    "B2_PRE": 4,    # block (2,0) pre-loop pops
    "B2_PER": 2,    # block (2,0) per-iter pops
    "B21_PER": 0,   # block (2,1) per-iter pops
    "B31_PER": 0,   # block (3,1) per-iter pops
}


def _build_nc():
    from contextlib import ExitStack

    import concourse.bass as bass
    import concourse.tile as tile
    from concourse import bacc, mybir

    f32 = mybir.dt.float32
    f16 = mybir.dt.float16
    f8 = mybir.dt.float8e4
    DR = mybir.MatmulPerfMode.DoubleRow
    Exp = mybir.ActivationFunctionType.Exp

    nc = bacc.Bacc(trn_type="TRN2")

    hsT_d = nc.declare_dram_parameter("hsT", [QD, QSH], f16, isOutput=False)
    encT_d = nc.declare_dram_parameter("encT", [CD, LKV], f16, isOutput=False)
    wq_d = nc.declare_dram_parameter("wq", [QD, INNER], f16, isOutput=False)
    wk_d = nc.declare_dram_parameter("wk", [CD, P], f16, isOutput=False)
    wv_d = nc.declare_dram_parameter("wv", [CD, INNER], f16, isOutput=False)
    wo_d = nc.declare_dram_parameter("wo", [INNER, QD], f16, isOutput=False)
    bo_d = nc.declare_dram_parameter("bo", [1, QD], f32, isOutput=False)
    wk8_d = nc.declare_dram_parameter("wk8", [CD, INNER], f8, isOutput=False)
    encT8_d = nc.declare_dram_parameter("encT8", [CD, 2, LKV], f8, isOutput=False)
    out_d = nc.declare_dram_parameter("out", [QSH, QD], f32, isOutput=True)

    KC_Q = QD // P   # 4 contraction chunks for q projection
    KC_KV = CD // P  # 6 for k/v projections
    AT = INNER // P  # 4 inner tiles (2 heads each)
    NT = LKV // P    # 8 kv chunks
    QN = QSH // 512  # 2 q slices of 512

    with ExitStack() as ctx:
        tc = ctx.enter_context(tile.TileContext(nc))
        const = ctx.enter_context(tc.tile_pool(name="const", bufs=1))
        acts = ctx.enter_context(tc.tile_pool(name="acts", bufs=1))
        expp = ctx.enter_context(tc.tile_pool(name="expp", bufs=8))
        outp = ctx.enter_context(tc.tile_pool(name="outp", bufs=6))
        small = ctx.enter_context(tc.tile_pool(name="small", bufs=8))
        psA = ctx.enter_context(tc.tile_pool(name="psA", bufs=4, space="PSUM"))
        psS = ctx.enter_context(tc.tile_pool(name="psS", bufs=2, space="PSUM"))
        drp = ctx.enter_context(tc.tile_pool(name="drp", bufs=4, space="DRAM"))

        # ---- input DMA, ordered by first use: the q projections (hsT+wq)
        # run during the PE warmup window, then kT (encT+wk), then v (wv);
        # the second encT half only gates scores t>=4 of the first block
        hsT_sb = acts.tile([P, KC_Q, QSH], f16)
        hsT_r = hsT_d.rearrange("(c p) n -> p c n", p=P)
        nc.sync.dma_start(hsT_sb[:, :, 0:512], hsT_r[:, :, 0:512])
        wq_sb = const.tile([P, KC_Q, INNER], f16)
        nc.sync.dma_start(wq_sb[:], wq_d.rearrange("(c p) n -> p c n", p=P))
        encT_sb = acts.tile([P, KC_KV, LKV], f16)
        encT_r = encT_d.rearrange("(c p) n -> p c n", p=P)
        wk_sb = const.tile([P, KC_KV, P], f16)
        nc.sync.dma_start(wk_sb[:], wk_d.rearrange("(c p) n -> p c n", p=P))
        nc.sync.dma_start(encT_sb[:, 0:2, 0:512], encT_r[:, 0:2, 0:512])
        nc.sync.dma_start(encT_sb[:, 2:4, 0:512], encT_r[:, 2:4, 0:512])
        nc.sync.dma_start(encT_sb[:, 4:6, 0:512], encT_r[:, 4:6, 0:512])
        wv_sb = const.tile([P, KC_KV, INNER], f16)
        nc.sync.dma_start(wv_sb[:], wv_d.rearrange("(c p) n -> p c n", p=P))
        nc.sync.dma_start(encT_sb[:, :, 512:1024], encT_r[:, :, 512:1024])
        nc.sync.dma_start(hsT_sb[:, :, 512:1024], hsT_r[:, :, 512:1024])
        # fp8 operands for the DoubleRow k projections (k heads 2..7): the
        # encoder comes as an fp8 (hi, lo) plane pair; wk8 rides both planes
        # of the stationary operand via a stride-0 broadcast
        encT8_sb = acts.tile([P, KC_KV, 2, LKV], f8)
        encT8_r = encT8_d.rearrange("(c p) j n -> p c j n", p=P)
        wk8_sb = const.tile([P, KC_KV, INNER], f8)
        nc.sync.dma_start(encT8_sb[:, :, 0, 0:512], encT8_r[:, :, 0, 0:512])
        nc.sync.dma_start(encT8_sb[:, :, 1, 0:512], encT8_r[:, :, 1, 0:512])
        nc.sync.dma_start(wk8_sb[:], wk8_d.rearrange("(c p) n -> p c n", p=P))
        nc.sync.dma_start(encT8_sb[:, :, 0, 512:1024], encT8_r[:, :, 0, 512:1024])
        nc.sync.dma_start(encT8_sb[:, :, 1, 512:1024], encT8_r[:, :, 1, 512:1024])
        wo_sb = const.tile([P, AT, QD], f16)
        nc.sync.dma_start(wo_sb[:], wo_d.rearrange("(c p) n -> p c n", p=P))
        bo_sb = const.tile([P, QD], f32)
        nc.sync.dma_start(bo_sb[:], bo_d.ap().to_broadcast((P, QD)))

        qT8 = acts.tile([P, AT, 2, QSH], f8)       # planes: (hi, lo)
        kT8 = acts.tile([P, AT, LKV], f8)          # single plane, broadcast in DR
        v_sb = acts.tile([P, NT, H * P], f16)
        outT_sb = acts.tile([P, AT, QSH], f16)
        vv4 = v_sb.rearrange("p t (s c) -> p t s c", c=P)

        # ---- PE warmup: dummy matmuls on zeroed scratch fill the DMA head
        # so the first real matmuls run at full clock (psD is never read)
        scratch = acts.tile([P, 512], f16)
        nc.gpsimd.memset(scratch[:], 0.0)

        # ones column for the PE-side partition broadcast in normalize
        ones_sb = const.tile([1, P], f16)
        nc.vector.memset(ones_sb[:], 1.0)

        def warmup(nmm):
            psD = psA.tile([P, 512], f32, tag="acc")
            for i in range(nmm):
                nc.tensor.matmul(
                    psD[:], scratch[:, 0:P], scratch[:],
                    start=(i == 0), stop=(i == nmm - 1),
                )

        # Generators yield once per emitted PE matmul so attention blocks can
        # interleave them into PE slack at a controlled rate (the per-engine
        # instruction streams execute strictly in program order).
        def gen_proj_k(a, nns=(0, 1)):
            # trailing copies are emitted BEFORE the final yield so that a
            # fully-popped generator has fully emitted its writes.
            # a=0 runs in fp16 off the early fp16 encoder DMA; a>=1 runs as
            # one-sided-fp8 DoubleRow (half PE cost; only the wk8 side
            # carries fp8 error, which softmax normalization absorbs).
            for nn in nns:
                ps = psA.tile([P, 512], f32, tag="acc")
                for c in range(KC_KV):
                    if a == 0:
                        nc.tensor.matmul(
                            ps[:],
                            wk_sb[:, c, a * P:(a + 1) * P],
                            encT_sb[:, c, nn * 512:(nn + 1) * 512],
                            start=(c == 0),
                            stop=(c == KC_KV - 1),
                        )
                    else:
                        nc.tensor.matmul(
                            ps[:],
                            wk8_sb[:, c, a * P:(a + 1) * P]
                            .unsqueeze(1).broadcast_to((P, 2, P)),
                            encT8_sb[:, c, :, nn * 512:(nn + 1) * 512],
                            start=(c == 0),
                            stop=(c == KC_KV - 1),
                            perf_mode=DR,
                        )
                    if c < KC_KV - 1:
                        yield
                with nc.allow_low_precision(reason="k stored fp8 for DR scores"):
                    if a == 0 and nn == 0:
                        # split so the first two scores matmuls (kv chunks
                        # 0-1) are gated by a half-copy, not the full 512
                        nc.vector.tensor_copy(kT8[:, 0, 0:256], ps[:, 0:256])
                        nc.vector.tensor_copy(
                            kT8[:, 0, 256:512], ps[:, 256:512]
                        )
                    else:
                        nc.vector.tensor_copy(
                            kT8[:, a, nn * 512:(nn + 1) * 512], ps[:]
                        )
                yield

        def gen_proj_q(a, n):
            ps = psA.tile([P, 512], f32, tag="acc")
            for c in range(KC_Q):
                nc.tensor.matmul(
                    ps[:],
                    wq_sb[:, c, a * P:(a + 1) * P],
                    hsT_sb[:, c, n * 512:(n + 1) * 512],
                    start=(c == 0),
                    stop=(c == KC_Q - 1),
                )
                if c < KC_Q - 1:
                    yield
            sl = slice(n * 512, (n + 1) * 512)
            with nc.allow_low_precision(reason="q stored as fp8 hi/lo pair"):
                nc.vector.tensor_copy(qT8[:, a, 0, sl], ps[:])
                nc.vector.tensor_sub(qT8[:, a, 1, sl], ps[:], qT8[:, a, 0, sl])
            yield

        # v natural [kv, slots]: slot h (128 wide):
        #   h even: [v_h (0:64) | 1.0 at 64 | 0 at 65:128]   -> out rows 0:64, denom row 64
        #   h odd : [1.0 at 0 | 0 at 1:64 | v_h at 64:128]   -> out rows 64:128, denom row 0
        def v_memsets():
            nc.gpsimd.memset(vv4[:, :, 0::2, 64:65], 1.0)
            nc.gpsimd.memset(vv4[:, :, 1::2, 0:1], 1.0)
            nc.gpsimd.memset(vv4[:, :, 0::2, 65:P], 0.0)
            nc.gpsimd.memset(vv4[:, :, 1::2, 1:DH], 0.0)

        def gen_proj_v(t):
            ps = psA.tile([P, 512], f32, tag="acc")
            for c in range(KC_KV):
                nc.tensor.matmul(
                    ps[:],
                    encT_sb[:, c, t * P:(t + 1) * P],
                    wv_sb[:, c, :],
                    start=(c == 0),
                    stop=(c == KC_KV - 1),
                )
                if c < KC_KV - 1:
                    yield
            pv = ps.rearrange("p (s c) -> p s c", c=DH)
            nc.vector.tensor_copy(vv4[:, t, 0::2, 0:DH], pv[:, 0::2, :])
            nc.vector.tensor_copy(vv4[:, t, 1::2, DH:P], pv[:, 1::2, :])
            yield

        def gen_final(m):
            ps = psA.tile([P, 512], f32, tag="acc")
            for a in range(AT):
                nc.tensor.matmul(
                    ps[:],
                    outT_sb[:, a, m * P:(m + 1) * P],
                    wo_sb[:, a, :],
                    start=(a == 0),
                    stop=(a == AT - 1),
                )
                if a < AT - 1:
                    yield
            ob = outp.tile([P, QD], f32)
            nc.vector.tensor_add(ob[:], ps[:], bo_sb[:])
            nc.sync.dma_start(out_d[m * P:(m + 1) * P, :], ob[:])
            yield

        # final projection split for the tail m-tiles: partA (heads 0-1)
        # accumulates into an SBUF staging tile during earlier blocks; partB
        # (heads 2-3) only trails the last attention block
        facc = acts.tile([P, QSH // P, QD], f32)

        def gen_final_a(m):
            ps = psA.tile([P, 512], f32, tag="acc")
            for a in (0, 1):
                nc.tensor.matmul(
                    ps[:],
                    outT_sb[:, a, m * P:(m + 1) * P],
                    wo_sb[:, a, :],
                    start=(a == 0),
                    stop=(a == 1),
                )
                if a == 0:
                    yield
            nc.vector.tensor_add(facc[:, m, :], ps[:], bo_sb[:])
            yield

        def gen_final_b(m):
            ps = psA.tile([P, 512], f32, tag="acc")
            for a in (2, 3):
                nc.tensor.matmul(
                    ps[:],
                    outT_sb[:, a, m * P:(m + 1) * P],
                    wo_sb[:, a, :],
                    start=(a == 2),
                    stop=(a == 3),
                )
                if a == 2:
                    yield
            ob = outp.tile([P, QD], f32)
            nc.vector.tensor_add(ob[:], ps[:], facc[:, m, :])
            nc.sync.dma_start(out_d[m * P:(m + 1) * P, :], ob[:])
            yield

        # split final-b for the first two tail m-tiles: the a=2 matmul only
        # needs head group 2 (written by block (2,1)'s tail), so it runs as a
        # block-(3,1) extra; only the a=3 matmul + add + DMA trail the last
        # normalize.  At most TWO may be pre-opened: their live PSUM tiles
        # plus the two tail rcps tiles exactly fill the 4-slot acc rotation.
        fb_ps = {}

        def gen_fb_a2(m):
            ps = psA.tile([P, 512], f32, tag="acc", name="fbps")
            nc.tensor.matmul(
                ps[:], outT_sb[:, 2, m * P:(m + 1) * P], wo_sb[:, 2, :],
                start=True, stop=False,
            )
            fb_ps[m] = ps
            yield

        def fb_a3(m):
            ps = fb_ps[m]
            nc.tensor.matmul(
                ps[:], outT_sb[:, 3, m * P:(m + 1) * P], wo_sb[:, 3, :],
                start=False, stop=True,
            )
            ob = outp.tile([P, QD], f32)
            nc.vector.tensor_add(ob[:], ps[:], facc[:, m, :])
            nc.sync.dma_start(out_d[m * P:(m + 1) * P, :], ob[:])

        def gen_chain(*gens):
            for g in gens:
                yield from g

        def run_gen(g):
            for _ in g:
                pass

        def attn(hp, n, extras=None, pre_pop=0, per_iter=0, prev_tail=None,
                 drain=True, act_copy_norm=False, split_exp_ts=(),
                 tail_mid=None):
            """Emit one attention block.  Returns a closure that emits the
            block's last two AV matmuls + normalize; the caller passes it to
            the NEXT block so those trail instructions interleave with the
            next block's leading scores (removes the block-boundary bubble).
            """
            if extras is None:
                extras = iter(())

            def pop(k):
                for _ in range(k):
                    if next(extras, StopIteration) is StopIteration:
                        break

            av0 = psA.tile([P, 512], f32, tag="acc")
            av1 = psA.tile([P, 512], f32, tag="acc")
            av = (av0, av1)
            exs = []

            def s_(t):
                ss = psS.tile([P, 1024], f32)
                for i in range(2):
                    pr = slice(i * 64, (i + 1) * 64)
                    nc.tensor.matmul(
                        ss[:, i * 512:(i + 1) * 512],
                        kT8[pr, hp, t * P:(t + 1) * P]
                        .unsqueeze(1).broadcast_to((64, 2, P)),
                        qT8[pr, hp, :, n * 512:(n + 1) * 512],
                        start=True,
                        stop=True,
                        perf_mode=DR,
                    )
                if t in split_exp_ts:
                    # last exp of the LAST block split per head: the final AV
                    # for head-even (and so the whole normalize/final-b tail
                    # chain) starts one half-exp earlier.
                    exa = expp.tile([P, 512], f16, tag="exh", name="exa")
                    exb = expp.tile([P, 512], f16, tag="exh", name="exb")
                    nc.scalar.activation(
                        exa[:], ss[:, 0:512], Exp,
                        scale=SCALE / (WSCALE * WSCALE_K),
                    )
                    nc.scalar.activation(
                        exb[:], ss[:, 512:1024], Exp,
                        scale=SCALE / (WSCALE * WSCALE_K),
                    )
                    exs.append((exa, exb))
                    return
                ex = expp.tile([P, 1024], f16)
                nc.scalar.activation(
                    ex[:], ss[:], Exp, scale=SCALE / (WSCALE * WSCALE_K)
                )
                exs.append(ex)

            def A_(t):
                for i in range(2):
                    s = 2 * hp + i
                    e = exs[t]
                    rhs = e[i][:, :] if isinstance(e, tuple) \
                        else e[:, i * 512:(i + 1) * 512]
                    nc.tensor.matmul(
                        av[i][:],
                        v_sb[:, t, s * P:(s + 1) * P],
                        rhs,
                        start=(t == 0),
                        stop=(t == NT - 1),
                    )

            s_(0)
            s_(1)
            pop(pre_pop)
            if prev_tail is not None:
                prev_tail()
            for t in range(2, NT):
                s_(t)
                A_(t - 2)
                pop(per_iter)
            if drain:  # drain leftovers so every generator completes
                for _ in extras:
                    pass

            def tail():
                A_(NT - 2)
                A_(NT - 1)
                # partition broadcast of 1/denom via a PE ones-column matmul
                # (GpSimd partition_broadcast proved flaky on HW; the DMA
                # round-trip costs ~4us per block).  The reciprocal lands on
                # partition 0 in f16, ones.T @ recip fills a PSUM tile,
                # which is copied to SBUF for the multiply (ScalarE for the
                # last block where it is idle, DVE elsewhere).
                rcbs = []
                for i in range(2):
                    drow = 64 if i == 0 else 0
                    rc = small.tile([1, 512], f16, tag="rc")
                    with nc.allow_low_precision(
                        reason="softmax denom reciprocal, f16 suffices"
                    ):
                        nc.vector.reciprocal(
                            rc[0:1, :], av[i][drow:drow + 1, :]
                        )
                    rcps = psA.tile([P, 512], f32, tag="acc")
                    nc.tensor.matmul(
                        rcps[:], ones_sb[0:1, :], rc[0:1, :],
                        start=True, stop=True,
                    )
                    rcb = small.tile([P, 512], f32, tag="rcb")
                    if act_copy_norm:
                        # last block: ScalarE staging copies in column halves
                        # so the first multiplies (and the final-b m-tiles
                        # they gate) start after half the copy latency
                        nc.scalar.copy(rcb[:, 0:256], rcps[:, 0:256])
                        nc.scalar.copy(rcb[:, 256:512], rcps[:, 256:512])
                    else:
                        nc.vector.tensor_copy(rcb[:], rcps[:])
                    rcbs.append(rcb)
                dsts = (slice(0, 64), slice(64, 128))
                if act_copy_norm:
                    # column-half multiplies with the first two final-b
                    # m-tiles emitted in between: their matmuls/adds/DMAs
                    # only need outT cols 0:256, so the serial out-DMA chain
                    # starts while the second halves still compute
                    for cols in (slice(0, 256), slice(256, 512)):
                        for i in range(2):
                            nc.vector.tensor_mul(
                                outT_sb[dsts[i], hp,
                                        n * 512 + cols.start:
                                        n * 512 + cols.stop],
                                av[i][dsts[i], cols],
                                rcbs[i][dsts[i], cols],
                            )
                        if cols.start == 0 and tail_mid is not None:
                            tail_mid()
                else:
                    for i in range(2):
                        nc.vector.tensor_mul(
                            outT_sb[dsts[i], hp, n * 512:(n + 1) * 512],
                            av[i][dsts[i], :],
                            rcbs[i][dsts[i], :],
                        )

            return tail

        # ---- emission = per-engine execution order.  Warmup dummies bridge
        # the DMA head up to qT(0,0); kT(0) kv-half 0 slots into the gap as
        # soon as its DMA lands; everything else (v, kT second half, later
        # k/q projections, finals) interleaves into attention-block PE slack.
        v_memsets()
        warmup(CFG["W1"])
        run_gen(gen_proj_q(0, 0))
        run_gen(gen_proj_q(1, 0))
        run_gen(gen_proj_k(0, nns=(0,)))
        run_gen(gen_proj_q(2, 0))
        run_gen(gen_proj_q(3, 0))
        tail = attn(
            0, 0, split_exp_ts=(0,),
            extras=gen_chain(
                gen_proj_v(0), gen_proj_v(1),
                gen_proj_k(0, nns=(1,)),
                *[gen_proj_v(t) for t in range(2, NT)],
                gen_proj_k(1, nns=(0,)),
            ),
            pre_pop=CFG["B0_PRE"], per_iter=CFG["B0_PER"],
        )
        tail = attn(1, 0, extras=gen_chain(gen_proj_k(1, nns=(1,)),
                                           gen_proj_q(0, 1),
                                           gen_proj_k(2), gen_proj_q(1, 1)),
                    pre_pop=CFG["BK_PRE"], per_iter=CFG["BK_PER"],
                    prev_tail=tail)
        tail = attn(2, 0, extras=gen_chain(gen_proj_k(3), gen_proj_q(2, 1)),
                    pre_pop=CFG["B2_PRE"], per_iter=CFG["B2_PER"],
                    prev_tail=tail)
        tail = attn(3, 0, extras=gen_proj_q(3, 1), pre_pop=0, per_iter=1,
                    prev_tail=tail)
        f01 = gen_chain(gen_final(0), gen_final(1))
        tail = attn(0, 1, extras=f01, pre_pop=0, per_iter=1,
                    prev_tail=tail, drain=False)
        tail = attn(1, 1, extras=gen_chain(f01, gen_final(2)),
                    pre_pop=0, per_iter=1, prev_tail=tail)
        tail = attn(2, 1,
                    extras=gen_chain(gen_final(3), gen_final_a(4)),
                    pre_pop=0, per_iter=CFG["B21_PER"], prev_tail=tail)
        tail = attn(3, 1, act_copy_norm=True, split_exp_ts=(NT - 1,),
                    tail_mid=lambda: (run_gen(gen_final_b(4)),
                                      run_gen(gen_final_b(5))),
                    extras=gen_chain(gen_final_a(5), gen_final_a(6),
                                     gen_final_a(7)),
                    pre_pop=0, per_iter=CFG["B31_PER"], prev_tail=tail)
        tail()
        for m in (6, 7):
            run_gen(gen_final_b(m))

    nc.finalize()
    return nc


def _get_nc():
    if "nc" not in _CACHE:
        _CACHE["nc"] = _build_nc()
    return _CACHE["nc"]


def make_in_maps(hidden_states, encoder_hidden_states, Wq, Wk, Wv, Wo, bo):
    f16 = np.float16
    hs = np.asarray(hidden_states, dtype=np.float32)
    enc = np.asarray(encoder_hidden_states, dtype=np.float32)
    import ml_dtypes

    f8 = ml_dtypes.float8_e4m3
    wq = np.ascontiguousarray(np.asarray(Wq, dtype=np.float32) * WSCALE).astype(f16)
    wk32 = np.asarray(Wk, dtype=np.float32) * WSCALE_K
    wk = np.ascontiguousarray(wk32[:, 0:P]).astype(f16)
    wk8 = np.ascontiguousarray(wk32).astype(f8)
    wv = np.ascontiguousarray(np.asarray(Wv, dtype=np.float32)).astype(f16)
    wo = np.ascontiguousarray(np.asarray(Wo, dtype=np.float32)).astype(f16)
    bo_ = np.ascontiguousarray(np.asarray(bo, dtype=np.float32)).reshape(1, QD)
    encT = [np.ascontiguousarray(enc[b].T).astype(f16) for b in range(B)]
    encT8 = []
    for b in range(B):
        e = enc[b].T.astype(np.float32)
        hi = e.astype(f8)
        lo = (e - hi.astype(np.float32)).astype(f8)
        encT8.append(np.ascontiguousarray(np.stack([hi, lo], axis=1)))
    in_maps = []
    for c in range(NCORES):
        b, s = divmod(c, 4)
        hsT = np.ascontiguousarray(hs[b, s * QSH:(s + 1) * QSH, :].T).astype(f16)
        in_maps.append(
            dict(hsT=hsT, encT=encT[b], wq=wq, wk=wk, wv=wv, wo=wo, bo=bo_,
                 wk8=wk8, encT8=encT8[b])
        )
    return in_maps


def kernel(hidden_states, encoder_hidden_states, Wq, Wk, Wv, Wo, bo):
    global LAST_RESULTS
    from concourse.bass_utils import run_bass_kernel_spmd

    nc = _get_nc()
    in_maps = make_in_maps(
        hidden_states, encoder_hidden_states, Wq, Wk, Wv, Wo, bo
    )
    res = run_bass_kernel_spmd(nc, in_maps, core_ids=list(range(NCORES)))
    LAST_RESULTS = res
    out = np.empty((B, LQ, QD), dtype=np.float32)
    for c in range(NCORES):
        b, s = divmod(c, 4)
        out[b, s * QSH:(s + 1) * QSH, :] = res.results[c]["out"]
    return out

